# revision 1
# baseline (speedup 1.0000x reference)
"""DualPathTransformer Trainium2 kernel.

Sharding: 8 cores = batch(4) x query-half(2). Each core processes one batch
and 1024 query tokens; K/V work is duplicated within a batch pair. No
device collectives: partial pooled projections are summed on the host.

SPMD uniformity trick: each core receives its batch token-ROTATED so that
its query tokens sit at rotated positions [512, 1536). Global attention is
permutation-invariant over keys; the local band structure is encoded in
host-prepped per-core mask tiles in true original coordinates. The program
is identical on all cores; only input data differs.

Layouts: activations feature-major (hT = [feature partitions, tokens]) for
matmuls; token-major (tokens on partitions) for layernorm stages. Scores
are computed transposed (keys on partitions) so softmax denominators come
free from a ones-row appended to V, and the AV matmul needs no transposes.

Precision: residual stream and weights fp32/f32r; attention q/k/v/probs and
post-attention projections bf16 (error contribution ~1e-3 of the stream).
"""

import numpy as np
import ml_dtypes
from contextlib import ExitStack

import concourse.bass as bass
import concourse.bacc as bacc
import concourse.tile as tile
import concourse.mybir as mybir
from concourse.bass_utils import run_bass_kernel_spmd

F32R = mybir.dt.float32r
F32 = mybir.dt.float32
BF16 = mybir.dt.bfloat16
AF = mybir.ActivationFunctionType
ALU = mybir.AluOpType

B, S, DIN, D, H, DOUT, W = 4, 2048, 256, 512, 8, 128, 64
HD = D // H          # 64
DFF = 2 * D          # 1024
NQ = S // 2          # 1024 queries per core
N_CORES = 8
Q0 = 512             # rotated position of first query token (uniform)
KL0, KL1 = 384, 1664   # local K/V window in rotated coords (10 ptiles)
NKL = KL1 - KL0        # 1280
DELTAS = (-128, 0, 128, 256, 384, 512)   # local kblock offsets rel. to qblock
# stripe (bounding qq range) per delta, qblock-relative
STRIPE = {-128: (0, 32), 0: (0, 160), 128: (96, 288),
          256: (224, 416), 384: (352, 512), 512: (480, 512)}
EDGE_DELTAS = (-128, 512)          # AV mms sliced to the stripe
SCALE = 1.0 / float(np.sqrt(HD))
EPS = 1e-5

_CACHE = {}
GLOBAL_KV_ON_ACT = False
LOCAL_KV_ON_ACT = True


def _build(flags, debug=False):
    (use_bqkv_l, use_bqkv_g, use_bo, use_gate_b, use_b1, use_b2,
     use_n1g, use_n1b, use_n2g, use_n2b, use_n3g) = flags

    nc = bacc.Bacc("TRN2", target_bir_lowering=False, debug=False)

    def din(name, shape, dt=F32R):
        return nc.dram_tensor(name, list(shape), dt, kind="ExternalInput").ap()

    xT = din("xT", [DIN, S])
    posb = din("posb", [D, S])
    win = din("win", [DIN, D])
    wqkv_l = din("wqkv_l", [3, D, D])
    wqkv_g = din("wqkv_g", [3, D, D])
    wo2 = din("wo2", [2, D, D], BF16)    # [0]=local, [1]=global
    gate_w = din("gate_w", [2 * D, D], BF16)
    w1 = din("w1", [D, DFF], BF16)
    w2 = din("w2", [DFF, D], BF16)
    outw = din("outw", [D, DOUT])
    masks_m = din("masks_m", [128, 4, 512], BF16)   # [kk, di, qq]
    masks_e = din("masks_e", [128, 2, 2, 32], BF16)  # [kk, de, qb, qq32]
    eye = din("eye", [128, 128], F32)
    poolw = din("poolw", [128, 1])
    if use_bqkv_l:
        bqkv_l = din("bqkv_l", [128, 3, 4], F32)
        bv_l = din("bv_l", [128, D], F32)
    if use_bqkv_g:
        bqkv_g = din("bqkv_g", [128, 3, 4], F32)
        bv_g = din("bv_g", [128, D], F32)
    if use_bo:
        bo2 = din("bo2", [128, 2, 4], F32)
    if use_gate_b:
        gate_b = din("gate_b", [128, 4], F32)
    if use_b1:
        b1 = din("b1", [128, 8], F32)
    if use_b2:
        b2b = din("b2b", [128, D], F32)
    if use_n1g:
        n1gb = din("n1gb", [128, D], F32)
    if use_n1b:
        n1bb = din("n1bb", [128, D], F32)
    if use_n2g:
        n2gb = din("n2gb", [128, D], F32)
    if use_n2b:
        n2bb = din("n2bb", [128, D], F32)
    if use_n3g:
        n3gb = din("n3gb", [128, D], F32)
    # n3_b handled on host (pooled mean is linear in it)

    po = nc.dram_tensor("po", [1, DOUT], F32, kind="ExternalOutput").ap()
    scratch = nc.dram_tensor("pool_scratch", [1, D], F32R).ap()

    dbg = {}
    if debug:
        for nm, shp, dt_ in [("d_hT", [128, S], F32), ("d_oTl", [128, NQ], BF16),
                             ("d_oTg", [128, NQ], BF16), ("d_gateT", [128, 512], BF16),
                             ("d_fusedT", [128, NQ], BF16), ("d_y1", [128, D], F32),
                             ("d_y3", [128, D], F32), ("d_pooled", [1, D], F32)]:
            dbg[nm] = nc.dram_tensor(nm, shp, dt_, kind="ExternalOutput").ap()

    f32 = lambda ap: ap.bitcast(F32)

    with tile.TileContext(nc) as tc, ExitStack() as top:
        # ---- psum pools (8 banks) ----
        ps = top.enter_context(tc.tile_pool(name="ps", bufs=2, space="PSUM"))
        ps2 = top.enter_context(tc.tile_pool(name="ps2", bufs=2, space="PSUM"))
        pso = top.enter_context(tc.tile_pool(name="pso", bufs=1, space="PSUM"))

        # ---- persistent pools (static tags, round-robin slot reuse) ----
        pers = top.enter_context(tc.tile_pool(name="pers", bufs=1))
        lnp = top.enter_context(tc.tile_pool(name="lnp", bufs=2))
        wp = top.enter_context(tc.tile_pool(name="wp", bufs=1))
        s4 = top.enter_context(tc.tile_pool(name="s4", bufs=1))     # [128,1024] bf16 tags
        s2 = top.enter_context(tc.tile_pool(name="s2", bufs=11))    # [128,512] f32
        qTp = top.enter_context(tc.tile_pool(name="qTp", bufs=4))   # [128,1024] bf16
        kTp = top.enter_context(tc.tile_pool(name="kTp", bufs=4))   # [128,2048] bf16
        hTp = top.enter_context(tc.tile_pool(name="hTp", bufs=1))
        Vp = top.enter_context(tc.tile_pool(name="Vp", bufs=16))    # [128,8,65] bf16
        ptgp = top.enter_context(tc.tile_pool(name="ptgp", bufs=3)) # pair bf16

        eye_sb = pers.tile([128, 128], F32, name="eye_sb")
        nc.sync.dma_start(eye_sb[:], eye[:])
        eyeb_sb = pers.tile([128, 128], BF16, name="eyeb_sb")
        nc.vector.tensor_copy(eyeb_sb[:], eye_sb[:])
        poolw_sb = pers.tile([128, 1], F32R, name="poolw_sb")
        nc.sync.dma_start(poolw_sb[:], poolw[:])
        eps_sb = pers.tile([128, 1], F32, name="eps_sb")
        nc.vector.memset(eps_sb[:], EPS)
        eps2_sb = pers.tile([128, 1], F32, name="eps2_sb")
        nc.vector.memset(eps2_sb[:], EPS * EPS)

        def load_bias(ap_dram, shape, name):
            t = pers.tile(shape, F32, name=name)
            nc.sync.dma_start(t[:], ap_dram[:])
            return t
        bqkv_l_sb = load_bias(bqkv_l, [128, 3, 4], "bqkv_l_sb") if use_bqkv_l else None
        bv_l_sb = load_bias(bv_l, [128, D], "bv_l_sb") if use_bqkv_l else None
        bqkv_g_sb = load_bias(bqkv_g, [128, 3, 4], "bqkv_g_sb") if use_bqkv_g else None
        bv_g_sb = load_bias(bv_g, [128, D], "bv_g_sb") if use_bqkv_g else None
        bo2_sb = load_bias(bo2, [128, 2, 4], "bo2_sb") if use_bo else None
        gate_b_sb = load_bias(gate_b, [128, 4], "gate_b_sb") if use_gate_b else None
        b1_sb = load_bias(b1, [128, 8], "b1_sb") if use_b1 else None
        b2b_sb = load_bias(b2b, [128, D], "b2b_sb") if use_b2 else None
        n1gb_sb = load_bias(n1gb, [128, D], "n1gb_sb") if use_n1g else None
        n1bb_sb = load_bias(n1bb, [128, D], "n1bb_sb") if use_n1b else None
        n2gb_sb = load_bias(n2gb, [128, D], "n2gb_sb") if use_n2g else None
        n2bb_sb = load_bias(n2bb, [128, D], "n2bb_sb") if use_n2b else None
        n3gb_sb = load_bias(n3gb, [128, D], "n3gb_sb") if use_n3g else None

        # long-lived stream tiles
        hT = [hTp.tile([128, S], F32R, name=f"hT{m}", tag="hT", bufs=4)
              for m in range(4)]
        h_sb = [s2.tile([128, D], F32R, name=f"h{t}", tag="s2") for t in range(8)]

        # ============ Phase A: hT + h ======================================
        # posb lands directly in hT via DMA; matmul results accumulate into it
        for m in range(4):
            nc.sync.dma_start(
                hT[m][:], posb.rearrange("(t p) n -> p t n", p=128)[:, m, :])
        with ExitStack() as sA:
            pA = sA.enter_context(tc.tile_pool(name="pA", bufs=2))
            win_sb = pA.tile([128, 2, D], F32R, name="win_sb", tag="win", bufs=1)
            nc.sync.dma_start(win_sb[:], win.rearrange("(t p) n -> p t n", p=128))
            for c in range(2):
                xTc = pA.tile([128, 2, 1024], F32R, name=f"xTc{c}", tag="xTc")
                nc.sync.dma_start(
                    xTc[:], xT.rearrange("(t p) n -> p t n", p=128)
                    [:, :, c * 1024:(c + 1) * 1024])
                for m in range(4):
                    for hh in range(2):
                        acc = ps.tile([128, 512], F32, name=f"psA{m}{c}{hh}",
                                      tag="ps")
                        for kt in range(2):
                            nc.tensor.matmul(
                                acc[:], win_sb[:, kt, m * 128:(m + 1) * 128],
                                xTc[:, kt, hh * 512:(hh + 1) * 512],
                                start=(kt == 0), stop=(kt == 1))
                        sl = hT[m][:, c * 1024 + hh * 512:
                                   c * 1024 + (hh + 1) * 512]
                        nc.vector.tensor_tensor(sl, acc[:], sl, op=ALU.add)
        # token-major h for core's tokens (rotated [512, 1536))
        for t in range(8):
            for m in range(4):
                ptr = ps.tile([128, 128], F32, name=f"ptrA{t}{m}", tag="ps")
                nc.tensor.transpose(
                    ptr[:], f32(hT[m][:, Q0 + t * 128: Q0 + (t + 1) * 128]),
                    eye_sb[:])
                nc.vector.tensor_copy(
                    h_sb[t][:, m * 128:(m + 1) * 128], ptr[:])
        if debug:
            nc.sync.dma_start(dbg["d_hT"][:], f32(hT[0][:]))

        # ============ helper: qkv projection ================================
        def project_qkv(wqkv_sb, bias_sb, bv_sb, q_tiles, kT_tiles, v_tiles,
                        kT_lo, kT_hi, v_pt_lo, pfx, kv_on_act=True):
            for m in range(4):
                for n in range(2):
                    acc = ps.tile([128, 512], F32, name=f"{pfx}q{m}{n}", tag="ps")
                    for kt in range(4):
                        nc.tensor.matmul(
                            acc[:], wqkv_sb[:, 0, kt, m * 128:(m + 1) * 128],
                            hT[kt][:, Q0 + n * 512: Q0 + (n + 1) * 512],
                            start=(kt == 0), stop=(kt == 3))
                    dst = q_tiles[m].bitcast(BF16)[:, n * 512:(n + 1) * 512]
                    if bias_sb is not None:
                        nc.vector.tensor_scalar(
                            dst, acc[:], bias_sb[:, 0, m:m + 1], None,
                            op0=ALU.add)
                    else:
                        nc.vector.tensor_copy(dst, acc[:])
            nk = kT_hi - kT_lo
            for m in range(4):
                for off in range(0, nk, 512):
                    w_ = min(512, nk - off)
                    acc = ps.tile([128, 512], F32, name=f"{pfx}k{m}{off}",
                                  tag="ps")
                    for kt in range(4):
                        nc.tensor.matmul(
                            acc[:, 0:w_], wqkv_sb[:, 1, kt, m * 128:(m + 1) * 128],
                            hT[kt][:, kT_lo + off: kT_lo + off + w_],
                            start=(kt == 0), stop=(kt == 3))
                    dst = kT_tiles[m].bitcast(BF16)[:, off:off + w_]
                    if bias_sb is not None:
                        if kv_on_act:
                            nc.scalar.activation(dst, acc[:, 0:w_], AF.Identity,
                                                 bias=bias_sb[:, 1, m:m + 1])
                        else:
                            nc.vector.tensor_scalar(
                                dst, acc[:, 0:w_], bias_sb[:, 1, m:m + 1], None,
                                op0=ALU.add)
                    elif kv_on_act:
                        nc.scalar.copy(dst, acc[:, 0:w_])
                    else:
                        nc.vector.tensor_copy(dst, acc[:, 0:w_])
            for i, vt in enumerate(v_tiles):
                pt = v_pt_lo + i
                acc = ps.tile([128, 512], F32, name=f"{pfx}v{pt}", tag="ps")
                for kt in range(4):
                    nc.tensor.matmul(
                        acc[:], hT[kt][:, pt * 128:(pt + 1) * 128],
                        wqkv_sb[:, 2, kt, :], start=(kt == 0), stop=(kt == 3))
                dst3 = vt.bitcast(BF16)[:, :, 0:64]
                src3 = acc[:].rearrange("p (h e) -> p h e", h=8)
                if bv_sb is not None:
                    nc.vector.tensor_tensor(
                        dst3, src3,
                        f32(bv_sb[:]).rearrange("p (h e) -> p h e", h=8),
                        op=ALU.add)
                elif kv_on_act:
                    nc.scalar.copy(dst3, src3)
                else:
                    nc.vector.tensor_copy(dst3, src3)
                nc.gpsimd.memset(vt.bitcast(BF16)[:, :, 64:65], 1.0)

        # ============ helper: softmax-normalize attention head ==============
        def normalize(ps_o, oT_tile, r0, c0, pfx):
            recip = lnp.tile([1, 512], F32, name=f"{pfx}r", tag="recip")
            nc.vector.reciprocal(recip[:], ps_o[64:65, :])
            rb = lnp.tile([64, 512], F32, name=f"{pfx}rb", tag="rb")
            nc.gpsimd.partition_broadcast(rb[:], recip[:])
            nc.vector.tensor_tensor(
                oT_tile.bitcast(BF16)[r0:r0 + 64, c0:c0 + 512],
                ps_o[0:64, :], rb[:], op=ALU.mult)

        # ============ helper: out-projection (feature-major) ================
        def out_proj(oT, outT, wo_sb, li, pfx):
            for m in range(4):
                for n in range(2):
                    acc = ps.tile([128, 512], F32, name=f"{pfx}{m}{n}", tag="ps")
                    for kt in range(4):
                        nc.tensor.matmul(
                            acc[:], wo_sb[:, li, kt, m * 128:(m + 1) * 128],
                            oT[kt].bitcast(BF16)[:, n * 512:(n + 1) * 512],
                            start=(kt == 0), stop=(kt == 3))
                    dst = outT[m].bitcast(BF16)[:, n * 512:(n + 1) * 512]
                    if use_bo:
                        nc.scalar.activation(dst, acc[:], AF.Identity,
                                             bias=bo2_sb[:, li, m:m + 1])
                    else:
                        nc.scalar.copy(dst, acc[:])

        # ============ Phase B: local qkv ====================================
        qT_l = [qTp.tile([128, NQ], BF16, name=f"qTl{m}", tag="qT")
                for m in range(4)]
        kT_l = [kTp.tile([128, S], BF16, name=f"kTl{m}", tag="kT")
                for m in range(4)]
        V_l = [Vp.tile([128, 8, 65], BF16, name=f"Vl{pt}", tag="V")
               for pt in range(KL0 // 128, KL1 // 128)]
        wqkv_l_sb = wp.tile([128, 3, 4, D], F32R, name="wqkv_l_sb", tag="wbig")
        nc.sync.dma_start(
            wqkv_l_sb[:], wqkv_l.rearrange("w (t p) d -> p w t d", p=128))
        project_qkv(wqkv_l_sb, bqkv_l_sb, bv_l_sb, qT_l, kT_l, V_l,
                    KL0, KL1, KL0 // 128, "Bl", kv_on_act=LOCAL_KV_ON_ACT)

        # ============ Phase C: local (band) attention + out-proj ============
        oT_l = [s4.tile([128, NQ], BF16, name=f"oTl{m}", tag="s4a", bufs=4)
                for m in range(4)]
        with ExitStack() as sC:
            pC = sC.enter_context(tc.tile_pool(name="pC", bufs=1))
            masks_m_sb = pC.tile([128, 4, 512], BF16, name="masks_m_sb")
            nc.scalar.dma_start(masks_m_sb[:], masks_m[:])
            masks_e_sb = pC.tile([128, 2, 2, 32], BF16, name="masks_e_sb")
            nc.sync.dma_start(masks_e_sb[:], masks_e[:])
            MAIN_DELTAS = (0, 128, 256, 384)
            PT = {}
            for di, dd in enumerate(MAIN_DELTAS):
                t = pC.tile([128, 2, 512], BF16, name=f"PTl{di}")
                nc.gpsimd.memset(t[:], 0.0)
                PT[dd] = t
            for de_i, de in enumerate(EDGE_DELTAS):
                PT[de] = pC.tile([128, 2, 32], BF16, name=f"PTe{de_i}")
            for qb in range(2):
                q0 = Q0 + qb * 512
                for hp in range(4):
                    for di, dd in enumerate(MAIN_DELTAS):
                        qq0, qq1 = STRIPE[dd]
                        rel = q0 + dd - KL0
                        sc2 = ps2.tile([128, 2, 512], F32,
                                       name=f"psC{qb}{hp}{di}", tag="ps2")
                        for ab in range(2):
                            r0 = ab * 64
                            nc.tensor.matmul(
                                sc2[:, ab, qq0:qq1],
                                kT_l[hp].bitcast(BF16)[r0:r0 + 64, rel:rel + 128],
                                qT_l[hp].bitcast(BF16)
                                [r0:r0 + 64, qb * 512 + qq0: qb * 512 + qq1],
                                start=True, stop=True, tile_position=(r0, 0))
                        pt_t = PT[dd]
                        nc.scalar.activation(
                            pt_t[:, :, qq0:qq1], sc2[:, :, qq0:qq1],
                            AF.Exp, scale=SCALE)
                        nc.vector.tensor_tensor(
                            pt_t[:, :, qq0:qq1], pt_t[:, :, qq0:qq1],
                            masks_m_sb[:, di, qq0:qq1].unsqueeze(1)
                            .to_broadcast((128, 2, qq1 - qq0)), op=ALU.mult)
                    for de_i, de in enumerate(EDGE_DELTAS):
                        qq0, qq1 = STRIPE[de]
                        rel = q0 + de - KL0
                        sc2 = ps2.tile([128, 2, 512], F32,
                                       name=f"psCe{qb}{hp}{de_i}", tag="ps2")
                        for ab in range(2):
                            r0 = ab * 64
                            nc.tensor.matmul(
                                sc2[:, ab, 0:32],
                                kT_l[hp].bitcast(BF16)[r0:r0 + 64, rel:rel + 128],
                                qT_l[hp].bitcast(BF16)
                                [r0:r0 + 64, qb * 512 + qq0: qb * 512 + qq1],
                                start=True, stop=True, tile_position=(r0, 0))
                        pt_t = PT[de]
                        nc.scalar.activation(
                            pt_t[:], sc2[:, :, 0:32], AF.Exp, scale=SCALE)
                        nc.vector.tensor_tensor(
                            pt_t[:], pt_t[:],
                            masks_e_sb[:, de_i, qb, :].unsqueeze(1)
                            .to_broadcast((128, 2, 32)), op=ALU.mult)
                    for ab in range(2):
                        head = 2 * hp + ab
                        po_t = pso.tile([65, 512], F32, name=f"psoC{qb}{hp}{ab}",
                                        tag=f"pso{ab}", bufs=1)
                        nc.tensor.matmul(
                            po_t[:], V_l[(q0 - KL0) // 128].bitcast(BF16)[:, head, :],
                            PT[0][:, ab, :], start=True, stop=False,
                            skip_group_check=True)
                        for de in EDGE_DELTAS:
                            qq0, qq1 = STRIPE[de]
                            nc.tensor.matmul(
                                po_t[:, qq0:qq1],
                                V_l[(q0 + de - KL0) // 128].bitcast(BF16)[:, head, :],
                                PT[de][:, ab, :],
                                start=False, stop=False, skip_group_check=True)
                        for dd in (128, 256, 384):
                            nc.tensor.matmul(
                                po_t[:],
                                V_l[(q0 + dd - KL0) // 128].bitcast(BF16)[:, head, :],
                                PT[dd][:, ab, :], start=False, stop=(dd == 384),
                                skip_group_check=True)
                        normalize(po_t, oT_l[hp], ab * 64, qb * 512,
                                  f"nC{qb}{hp}{ab}")
        if debug:
            nc.sync.dma_start(dbg["d_oTl"][:], oT_l[0].bitcast(BF16)[:])

        wo_sb = wp.tile([128, 2, 4, D], BF16, name="wo_sb", tag="wo2nd")
        nc.scalar.dma_start(wo_sb[:], wo2.rearrange("w (t p) d -> p w t d", p=128))
        localT = [s4.tile([128, NQ], BF16, name=f"localT{m}", tag="s4b", bufs=4)
                  for m in range(4)]
        out_proj(oT_l, localT, wo_sb, 0, "psFl")

        # ============ Phase D: global qkv ===================================
        qT_g = [qTp.tile([128, NQ], BF16, name=f"qTg{m}", tag="qT")
                for m in range(4)]
        kT_g = [kTp.tile([128, S], BF16, name=f"kTg{m}", tag="kT")
                for m in range(4)]
        V_g = [Vp.tile([128, 8, 65], BF16, name=f"Vg{pt}", tag="V")
               for pt in range(16)]
        wqkv_g_sb = wp.tile([128, 3, 4, D], F32R, name="wqkv_g_sb", tag="wbig")
        nc.scalar.dma_start(
            wqkv_g_sb[:], wqkv_g.rearrange("w (t p) d -> p w t d", p=128))
        project_qkv(wqkv_g_sb, bqkv_g_sb, bv_g_sb, qT_g, kT_g, V_g, 0, S, 0, "Dg", kv_on_act=GLOBAL_KV_ON_ACT)

        # ============ Phase E: global attention + out-proj ==================
        oT_g = [s4.tile([128, NQ], BF16, name=f"oTg{m}", tag="s4c", bufs=8)
                for m in range(4)]
        for qb in range(2):
            for hp in range(4):
                po_ts = [pso.tile([65, 512], F32, name=f"psoE{qb}{hp}{ab}",
                                  tag=f"pso{ab}", bufs=1) for ab in range(2)]
                for kt in range(16):
                    sc2 = ps2.tile([128, 2, 512], F32,
                                   name=f"psE{qb}{hp}{kt}", tag="ps2")
                    for ab in range(2):
                        r0 = ab * 64
                        nc.tensor.matmul(
                            sc2[:, ab, :], kT_g[hp].bitcast(BF16)
                            [r0:r0 + 64, kt * 128:(kt + 1) * 128],
                            qT_g[hp].bitcast(BF16)
                            [r0:r0 + 64, qb * 512:(qb + 1) * 512],
                            start=True, stop=True, tile_position=(r0, 0))
                    ptg = ptgp.tile([128, 2, 512], BF16,
                                    name=f"ptg{qb}{hp}{kt}", tag="ptg")
                    nc.scalar.activation(ptg[:], sc2[:], AF.Exp, scale=SCALE)
                    for ab in range(2):
                        nc.tensor.matmul(
                            po_ts[ab][:],
                            V_g[kt].bitcast(BF16)[:, 2 * hp + ab, :],
                            ptg[:, ab, :], start=(kt == 0), stop=(kt == 15),
                            skip_group_check=True)
                for ab in range(2):
                    normalize(po_ts[ab], oT_g[hp], ab * 64, qb * 512,
                              f"nE{qb}{hp}{ab}")
        if debug:
            nc.sync.dma_start(dbg["d_oTg"][:], oT_g[0].bitcast(BF16)[:])

        globalT = [s4.tile([128, NQ], BF16, name=f"globalT{m}", tag="s4c", bufs=8)
                   for m in range(4)]
        out_proj(oT_g, globalT, wo_sb, 1, "psFg")

        # ============ Phase G: gate + fuse ==================================
        fusedT = [s4.tile([128, NQ], BF16, name=f"fusedT{m}", tag="s4a", bufs=4)
                  for m in range(4)]
        gate_w_sb = wp.tile([128, 8, D], BF16, name="gate_w_sb", tag="wbig")
        nc.scalar.dma_start(gate_w_sb[:],
                          gate_w.rearrange("(t p) d -> p t d", p=128))
        cat = localT + globalT
        for m in range(4):
            for n in range(2):
                acc = ps.tile([128, 512], F32, name=f"psG{m}{n}", tag="ps")
                for kt in range(8):
                    nc.tensor.matmul(
                        acc[:], gate_w_sb[:, kt, m * 128:(m + 1) * 128],
                        cat[kt].bitcast(BF16)[:, n * 512:(n + 1) * 512],
                        start=(kt == 0), stop=(kt == 7))
                gt = lnp.tile([128, 512], BF16, name=f"gt{m}{n}", tag="gt", bufs=1)
                if use_gate_b:
                    nc.vector.tensor_scalar(
                        gt[:], acc[:], gate_b_sb[:, m:m + 1], 0.0,
                        op0=ALU.add, op1=ALU.max)
                else:
                    nc.vector.tensor_scalar(gt[:], acc[:], 0.0, None,
                                            op0=ALU.max)
                nc.scalar.activation(gt[:], gt[:], AF.Tanh)
                if debug and m == 0 and n == 0:
                    nc.sync.dma_start(dbg["d_gateT"][:], gt[:])
                # fused = global + gate*(local - global)
                lsl = localT[m].bitcast(BF16)[:, n * 512:(n + 1) * 512]
                gsl = globalT[m].bitcast(BF16)[:, n * 512:(n + 1) * 512]
                tmp = lnp.tile([128, 512], BF16, name=f"tmpG{m}{n}", tag="tmpG", bufs=1)
                nc.gpsimd.tensor_tensor(tmp[:], lsl, gsl, op=ALU.subtract)
                nc.vector.tensor_tensor(tmp[:], tmp[:], gt[:], op=ALU.mult)
                nc.vector.tensor_tensor(
                    fusedT[m].bitcast(BF16)[:, n * 512:(n + 1) * 512],
                    tmp[:], gsl, op=ALU.add)
        if debug:
            nc.sync.dma_start(dbg["d_fusedT"][:], fusedT[0].bitcast(BF16)[:])

        # ===== layernorm helper (token-major [128, D]) ======================
        def layernorm(dst, src_ap, g_sb, b_sb, pfx):
            stats = lnp.tile([128, 6], F32, name=f"{pfx}st", tag="lnst")
            nc.vector.bn_stats(stats[:], src_ap)
            mv = lnp.tile([128, 2], F32, name=f"{pfx}mv", tag="lnmv")
            nc.vector.bn_aggr(mv[:], stats[:])
            std = lnp.tile([128, 1], F32, name=f"{pfx}sd", tag="lnsd")
            nc.scalar.activation(std[:], mv[:, 1:2], AF.Sqrt, bias=eps_sb[:])
            rstd = lnp.tile([128, 1], F32, name=f"{pfx}rs", tag="lnrs")
            nc.vector.reciprocal(rstd[:], std[:])
            if g_sb is not None:
                tmp = lnp.tile([128, D], F32, name=f"{pfx}tmp", tag="lntmp")
                nc.vector.tensor_scalar(
                    tmp[:], src_ap, mv[:, 0:1], rstd[:],
                    op0=ALU.subtract, op1=ALU.mult)
                if b_sb is not None:
                    nc.vector.tensor_tensor(dst, tmp[:], g_sb[:], op=ALU.mult)
                    nc.vector.tensor_tensor(dst, dst, b_sb[:], op=ALU.add)
                else:
                    nc.vector.tensor_tensor(dst, tmp[:], g_sb[:], op=ALU.mult)
            else:
                nc.vector.tensor_scalar(
                    dst, src_ap, mv[:, 0:1], rstd[:],
                    op0=ALU.subtract, op1=ALU.mult)
                if b_sb is not None:
                    nc.vector.tensor_tensor(dst, dst, b_sb[:], op=ALU.add)

        # ============ Phase H: x1 = h + fused^T; y1 = LN1 ===================
        y1 = [s2.tile([128, D], F32R, name=f"y1_{t}", tag="s2") for t in range(8)]
        for t in range(8):
            x1 = lnp.tile([128, D], F32, name=f"x1_{t}", tag="x1")
            for m in range(4):
                ptr = ps.tile([128, 128], BF16, name=f"ptrH{t}{m}", tag="ps")
                nc.tensor.transpose(
                    ptr[:], fusedT[m].bitcast(BF16)[:, t * 128:(t + 1) * 128],
                    eyeb_sb[:])
                nc.vector.tensor_tensor(
                    x1[:, m * 128:(m + 1) * 128],
                    f32(h_sb[t][:, m * 128:(m + 1) * 128]), ptr[:], op=ALU.add)
            layernorm(y1[t][:], x1[:], n1gb_sb, n1bb_sb, f"ln1_{t}")
        if debug:
            nc.sync.dma_start(dbg["d_y1"][:], f32(y1[0][:]))

        # ============ Phase I: y1T ==========================================
        y1T = [s4.tile([128, NQ], BF16, name=f"y1T{m}", tag="s4b", bufs=4)
               for m in range(4)]
        for t in range(8):
            for m in range(4):
                ptr = ps.tile([128, 128], F32, name=f"ptrI{t}{m}", tag="ps")
                nc.tensor.transpose(ptr[:], f32(y1[t][:, m * 128:(m + 1) * 128]),
                                    eye_sb[:])
                nc.scalar.copy(
                    y1T[m].bitcast(BF16)[:, t * 128:(t + 1) * 128], ptr[:])

        # ============ Phase J: FFN + LN2 + LN3; Phase K: pool + out =========
        w1_sb = wp.tile([128, 4, DFF], BF16, name="w1_sb", tag="wbig")
        nc.scalar.dma_start(w1_sb[:], w1.rearrange("(t p) d -> p t d", p=128))
        w2_sb = wp.tile([128, 8, D], BF16, name="w2_sb", tag="wo2nd")
        nc.scalar.dma_start(w2_sb[:], w2.rearrange("(t p) d -> p t d", p=128))
        z1T = [s4.tile([128, NQ], BF16, name=f"z1T{m}", tag="s4c", bufs=8)
               for m in range(8)]
        for m in range(8):
            for n in range(2):
                acc = ps.tile([128, 512], F32, name=f"psJ1{m}{n}", tag="ps")
                for kt in range(4):
                    nc.tensor.matmul(
                        acc[:], w1_sb[:, kt, m * 128:(m + 1) * 128],
                        y1T[kt].bitcast(BF16)[:, n * 512:(n + 1) * 512],
                        start=(kt == 0), stop=(kt == 3))
                dst = z1T[m].bitcast(BF16)[:, n * 512:(n + 1) * 512]
                if use_b1:
                    nc.vector.tensor_scalar(
                        dst, acc[:], b1_sb[:, m:m + 1], 0.0,
                        op0=ALU.add, op1=ALU.max)
                else:
                    nc.vector.tensor_scalar(dst, acc[:], 0.0, None, op0=ALU.max)

        y3 = [s2.tile([128, D], F32R, name=f"y3_{t}", tag="s2") for t in range(8)]
        accp = pso.tile([1, 512], F32, name="pspool", tag="pso0", bufs=1)
        for t in range(8):
            acc = ps.tile([128, 512], F32, name=f"psJ2{t}", tag="ps")
            for kt in range(8):
                nc.tensor.matmul(
                    acc[:], z1T[kt].bitcast(BF16)[:, t * 128:(t + 1) * 128],
                    w2_sb[:, kt, :], start=(kt == 0), stop=(kt == 7))
            x2 = lnp.tile([128, D], F32, name=f"x2_{t}", tag="x2")
            nc.vector.tensor_tensor(x2[:], acc[:], f32(y1[t][:]), op=ALU.add)
            if use_b2:
                nc.vector.tensor_tensor(x2[:], x2[:], b2b_sb[:], op=ALU.add)
            if not (use_n2g or use_n2b or use_n3g):
                # LN3(LN2(x)) with unit gamma / zero beta collapses to one LN:
                # mean(LN2 out) == 0 exactly, var(LN2 out) = v/(v+eps), so
                # y3 = (x - m) / sqrt(v*(1+eps) + eps^2)
                pfx = f"ln23_{t}"
                stats = lnp.tile([128, 6], F32, name=f"{pfx}st", tag="lnst")
                nc.vector.bn_stats(stats[:], x2[:])
                mv = lnp.tile([128, 2], F32, name=f"{pfx}mv", tag="lnmv")
                nc.vector.bn_aggr(mv[:], stats[:])
                std = lnp.tile([128, 1], F32, name=f"{pfx}sd", tag="lnsd")
                nc.scalar.activation(std[:], mv[:, 1:2], AF.Sqrt,
                                     bias=eps2_sb[:], scale=1.0 + EPS)
                rstd = lnp.tile([128, 1], F32, name=f"{pfx}rs", tag="lnrs")
                nc.vector.reciprocal(rstd[:], std[:])
                nc.vector.tensor_scalar(
                    y3[t][:], x2[:], mv[:, 0:1], rstd[:],
                    op0=ALU.subtract, op1=ALU.mult)
            else:
                y2 = lnp.tile([128, D], F32, name=f"y2_{t}", tag="y2")
                layernorm(y2[:], x2[:], n2gb_sb, n2bb_sb, f"ln2_{t}")
                layernorm(y3[t][:], y2[:], n3gb_sb, None, f"ln3_{t}")
            nc.tensor.matmul(accp[:], poolw_sb[:], y3[t][:],
                             start=(t == 0), stop=(t == 7),
                             skip_group_check=True)
        if debug:
            nc.sync.dma_start(dbg["d_y3"][:], f32(y3[0][:]))

        outw_sb = lnp.tile([128, 4, DOUT], F32R, name="outw_sb", tag="x2",
                           bufs=2)
        nc.sync.dma_start(outw_sb[:], outw.rearrange("(t p) n -> p t n", p=128))
        pooled_sb = pers.tile([1, D], F32R, name="pooled_sb")
        nc.vector.tensor_copy(pooled_sb[:], accp[:])
        if debug:
            nc.sync.dma_start(dbg["d_pooled"][:], f32(pooled_sb[:]))
        nc.sync.dma_start(scratch[:], pooled_sb[:])
        pooledT = pers.tile([128, 4], F32R, name="pooledT")
        nc.sync.dma_start(pooledT[:],
                          scratch.rearrange("o (t p) -> p (o t)", p=128))
        accf = pso.tile([1, 128], F32, name="psfin", tag="pso1", bufs=1)
        for kt in range(4):
            nc.tensor.matmul(accf[:], pooledT[:, kt:kt + 1], outw_sb[:, kt, :],
                             start=(kt == 0), stop=(kt == 3))
        po_sb = pers.tile([1, DOUT], F32, name="po_sb")
        nc.vector.tensor_copy(po_sb[:], accf[:])
        nc.sync.dma_start(po[:], po_sb[:])

    nc.compile()
    return nc


def _prep_inputs(inputs):
    """Host-side prep: returns (flags, in_maps for 8 cores, host_const)."""
    g = {k: np.asarray(v, dtype=np.float32) for k, v in inputs.items()}
    x, pos = g["x"], g["pos"]
    win_w, win_b = g["win_w"], g["win_b"]

    flags = (
        bool(np.any(g["l_bqkv"] != 0)), bool(np.any(g["g_bqkv"] != 0)),
        bool(np.any(g["l_bo"] != 0) or np.any(g["g_bo"] != 0)),
        bool(np.any(g["gate_b"] != 0)), bool(np.any(g["ffn_b1"] != 0)),
        bool(np.any(g["ffn_b2"] != 0)),
        bool(np.any(g["n1_g"] != 1)), bool(np.any(g["n1_b"] != 0)),
        bool(np.any(g["n2_g"] != 1)), bool(np.any(g["n2_b"] != 0)),
        bool(np.any(g["n3_g"] != 1)),
    )
    (use_bqkv_l, use_bqkv_g, use_bo, use_gate_b, use_b1, use_b2,
     use_n1g, use_n1b, use_n2g, use_n2b, use_n3g) = flags

    posT = pos[0].T + win_b[:, None]                      # [D, S]
    common = {
        "win": np.ascontiguousarray(win_w),
        "wqkv_l": np.ascontiguousarray(g["l_wqkv"]),
        "wqkv_g": np.ascontiguousarray(g["g_wqkv"]),
        "wo2": np.stack([g["l_wo"], g["g_wo"]]).astype(ml_dtypes.bfloat16),
        "gate_w": g["gate_w"].astype(ml_dtypes.bfloat16),
        "w1": g["ffn_w1"].astype(ml_dtypes.bfloat16),
        "w2": g["ffn_w2"].astype(ml_dtypes.bfloat16),
        "outw": np.ascontiguousarray(g["out_w"]),
        "eye": np.eye(128, dtype=np.float32),
        "poolw": np.full((128, 1), 1.0 / S, dtype=np.float32),
    }
    perm = lambda b: b.reshape(-1, 4, 128).transpose(2, 0, 1).copy()
    if use_bqkv_l:
        common["bqkv_l"] = perm(g["l_bqkv"])
        common["bv_l"] = np.tile(g["l_bqkv"][2], (128, 1))
    if use_bqkv_g:
        common["bqkv_g"] = perm(g["g_bqkv"])
        common["bv_g"] = np.tile(g["g_bqkv"][2], (128, 1))
    if use_bo:
        common["bo2"] = perm(np.stack([g["l_bo"], g["g_bo"]]))
    if use_gate_b:
        common["gate_b"] = g["gate_b"].reshape(4, 128).T.copy()
    if use_b1:
        common["b1"] = g["ffn_b1"].reshape(8, 128).T.copy()
    if use_b2:
        common["b2b"] = np.tile(g["ffn_b2"], (128, 1))
    if use_n1g:
        common["n1gb"] = np.tile(g["n1_g"], (128, 1))
    if use_n1b:
        common["n1bb"] = np.tile(g["n1_b"], (128, 1))
    if use_n2g:
        common["n2gb"] = np.tile(g["n2_g"], (128, 1))
    if use_n2b:
        common["n2bb"] = np.tile(g["n2_b"], (128, 1))
    if use_n3g:
        common["n3gb"] = np.tile(g["n3_g"], (128, 1))

    # universal interior band masks (pure Toeplitz, no seam crossing)
    kk = np.arange(128)
    qq = np.arange(512)
    mk_m = np.zeros((128, 4, 512), dtype=np.float32)
    for di, d in enumerate((0, 128, 256, 384)):
        mk_m[:, di, :] = (np.abs(kk[:, None] + d - qq[None, :]) <= W // 2)
    mk_m = mk_m.astype(ml_dtypes.bfloat16)

    hf_data = []
    for hf in range(2):
        q0c = NQ * hf
        shift = Q0 - q0c
        posb_rot = np.ascontiguousarray(np.roll(posT, shift, axis=1))
        mk_e = np.zeros((128, 2, 2, 32), dtype=np.float32)
        for qb in range(2):
            q0 = Q0 + qb * 512
            for de_i, d in enumerate(EDGE_DELTAS):
                qq0, qq1 = STRIPE[d]
                k_rot = q0 + d + kk[:, None]
                q_rot = q0 + np.arange(qq0, qq1)[None, :]
                orig_k = (k_rot - shift) % S
                orig_q = (q_rot - shift) % S
                mk_e[:, de_i, qb, :] = (np.abs(orig_k - orig_q) <= W // 2)
        hf_data.append((posb_rot, mk_e.astype(ml_dtypes.bfloat16)))

    in_maps = []
    for core in range(N_CORES):
        b, hf = core // 2, core % 2
        shift = Q0 - NQ * hf
        posb_rot, mk_e = hf_data[hf]
        m = dict(common)
        m["xT"] = np.ascontiguousarray(np.roll(x[b].T, shift, axis=1))
        m["posb"] = posb_rot
        m["masks_m"] = mk_m
        m["masks_e"] = mk_e
        in_maps.append(m)

    host_const = g["n3_b"] @ g["out_w"] + g["out_b"]
    return flags, in_maps, host_const


def kernel(**inputs):
    flags, in_maps, host_const = _prep_inputs(inputs)
    if flags not in _CACHE:
        _CACHE[flags] = _build(flags)
    nc = _CACHE[flags]
    res = run_bass_kernel_spmd(nc, in_maps, core_ids=list(range(N_CORES)))
    out = np.zeros((B, DOUT), dtype=np.float32)
    for b in range(B):
        out[b] = (res.results[2 * b]["po"][0] + res.results[2 * b + 1]["po"][0]
                  + host_const)
    return out



# revision 6
# speedup vs baseline: 1.0167x; 1.0167x over previous
"""DualPathTransformer Trainium2 kernel.

Sharding: 8 cores = batch(4) x query-half(2). Each core processes one batch
and 1024 query tokens; K/V work is duplicated within a batch pair. No
device collectives: partial pooled projections are summed on the host.

SPMD uniformity trick: each core receives its batch token-ROTATED so that
its query tokens sit at rotated positions [512, 1536). Global attention is
permutation-invariant over keys; the local band structure is encoded in
host-prepped per-core mask tiles in true original coordinates. The program
is identical on all cores; only input data differs.

Layouts: activations feature-major (hT = [feature partitions, tokens]) for
matmuls; token-major (tokens on partitions) for layernorm stages. Scores
are computed transposed (keys on partitions) so softmax denominators come
free from a ones-row appended to V, and the AV matmul needs no transposes.

Precision: residual stream and weights fp32/f32r; attention q/k/v/probs and
post-attention projections bf16 (error contribution ~1e-3 of the stream).
"""

import numpy as np
import ml_dtypes
from contextlib import ExitStack

import concourse.bass as bass
import concourse.bacc as bacc
import concourse.tile as tile
import concourse.mybir as mybir
from concourse.bass_utils import run_bass_kernel_spmd

F32R = mybir.dt.float32r
F32 = mybir.dt.float32
BF16 = mybir.dt.bfloat16
AF = mybir.ActivationFunctionType
ALU = mybir.AluOpType

B, S, DIN, D, H, DOUT, W = 4, 2048, 256, 512, 8, 128, 64
HD = D // H          # 64
DFF = 2 * D          # 1024
NQ = S // 2          # 1024 queries per core
N_CORES = 8
Q0 = 512             # rotated position of first query token (uniform)
KL0, KL1 = 384, 1664   # local K/V window in rotated coords (10 ptiles)
NKL = KL1 - KL0        # 1280
DELTAS = (-128, 0, 128, 256, 384, 512)   # local kblock offsets rel. to qblock
# stripe (bounding qq range) per delta, qblock-relative
STRIPE = {-128: (0, 32), 0: (0, 160), 128: (96, 288),
          256: (224, 416), 384: (352, 512), 512: (480, 512)}
EDGE_DELTAS = (-128, 512)          # AV mms sliced to the stripe
SCALE = 1.0 / float(np.sqrt(HD))
EPS = 1e-5

_CACHE = {}
GLOBAL_KV_ON_ACT = False
LOCAL_KV_ON_ACT = True


def _build(flags, debug=False):
    (use_bqkv_l, use_bqkv_g, use_bo, use_gate_b, use_b1, use_b2,
     use_n1g, use_n1b, use_n2g, use_n2b, use_n3g) = flags

    nc = bacc.Bacc("TRN2", target_bir_lowering=False, debug=False)

    def din(name, shape, dt=F32R):
        return nc.dram_tensor(name, list(shape), dt, kind="ExternalInput").ap()

    xT = din("xT", [DIN, S])
    posb = din("posb", [D, S])
    win = din("win", [DIN, D])
    wqkv_l = din("wqkv_l", [3, D, D])
    wqkv_g = din("wqkv_g", [3, D, D])
    wo2 = din("wo2", [2, D, D], BF16)    # [0]=local, [1]=global
    gate_w = din("gate_w", [2 * D, D], BF16)
    w1 = din("w1", [D, DFF], BF16)
    w2 = din("w2", [DFF, D], BF16)
    masks_m = din("masks_m", [128, 4, 512], BF16)   # [kk, di, qq]
    masks_e = din("masks_e", [128, 2, 2, 32], BF16)  # [kk, de, qb, qq32]
    eye = din("eye", [128, 128], F32)
    poolw = din("poolw", [128, 1])
    if use_bqkv_l:
        bqkv_l = din("bqkv_l", [128, 3, 4], F32)
        bv_l = din("bv_l", [128, D], F32)
    if use_bqkv_g:
        bqkv_g = din("bqkv_g", [128, 3, 4], F32)
        bv_g = din("bv_g", [128, D], F32)
    if use_bo:
        bo2 = din("bo2", [128, 2, 4], F32)
    if use_gate_b:
        gate_b = din("gate_b", [128, 4], F32)
    if use_b1:
        b1 = din("b1", [128, 8], F32)
    if use_b2:
        b2b = din("b2b", [128, D], F32)
    if use_n1g:
        n1gb = din("n1gb", [128, D], F32)
    if use_n1b:
        n1bb = din("n1bb", [128, D], F32)
    if use_n2g:
        n2gb = din("n2gb", [128, D], F32)
    if use_n2b:
        n2bb = din("n2bb", [128, D], F32)
    if use_n3g:
        n3gb = din("n3gb", [128, D], F32)
    # n3_b handled on host (pooled mean is linear in it)

    po = nc.dram_tensor("po", [1, D], F32, kind="ExternalOutput").ap()

    dbg = {}
    if debug:
        for nm, shp, dt_ in [("d_hT", [128, S], F32), ("d_oTl", [128, NQ], BF16),
                             ("d_oTg", [128, NQ], BF16), ("d_gateT", [128, 512], BF16),
                             ("d_fusedT", [128, NQ], BF16), ("d_y1", [128, D], F32),
                             ("d_y3", [128, D], F32), ("d_pooled", [1, D], F32)]:
            dbg[nm] = nc.dram_tensor(nm, shp, dt_, kind="ExternalOutput").ap()

    f32 = lambda ap: ap.bitcast(F32)

    with tile.TileContext(nc) as tc, ExitStack() as top:
        # ---- psum pools (8 banks) ----
        ps = top.enter_context(tc.tile_pool(name="ps", bufs=2, space="PSUM"))
        ps2 = top.enter_context(tc.tile_pool(name="ps2", bufs=2, space="PSUM"))
        pso = top.enter_context(tc.tile_pool(name="pso", bufs=1, space="PSUM"))

        # ---- persistent pools (static tags, round-robin slot reuse) ----
        pers = top.enter_context(tc.tile_pool(name="pers", bufs=1))
        lnp = top.enter_context(tc.tile_pool(name="lnp", bufs=2))
        wp = top.enter_context(tc.tile_pool(name="wp", bufs=1))
        s4 = top.enter_context(tc.tile_pool(name="s4", bufs=1))     # [128,1024] bf16 tags
        s2 = top.enter_context(tc.tile_pool(name="s2", bufs=11))    # [128,512] f32
        qTp = top.enter_context(tc.tile_pool(name="qTp", bufs=4))   # [128,1024] bf16
        kTp = top.enter_context(tc.tile_pool(name="kTp", bufs=4))   # [128,2048] bf16
        hTp = top.enter_context(tc.tile_pool(name="hTp", bufs=1))
        Vp = top.enter_context(tc.tile_pool(name="Vp", bufs=16))    # [128,8,65] bf16
        ptgp = top.enter_context(tc.tile_pool(name="ptgp", bufs=3)) # pair bf16

        eye_sb = pers.tile([128, 128], F32, name="eye_sb")
        nc.sync.dma_start(eye_sb[:], eye[:])
        eyeb_sb = pers.tile([128, 128], BF16, name="eyeb_sb")
        nc.vector.tensor_copy(eyeb_sb[:], eye_sb[:])
        poolw_sb = pers.tile([128, 1], F32R, name="poolw_sb")
        nc.sync.dma_start(poolw_sb[:], poolw[:])
        eps_sb = pers.tile([128, 1], F32, name="eps_sb")
        nc.vector.memset(eps_sb[:], EPS)
        eps2_sb = pers.tile([128, 1], F32, name="eps2_sb")
        nc.vector.memset(eps2_sb[:], EPS * EPS)

        def load_bias(ap_dram, shape, name):
            t = pers.tile(shape, F32, name=name)
            nc.sync.dma_start(t[:], ap_dram[:])
            return t
        bqkv_l_sb = load_bias(bqkv_l, [128, 3, 4], "bqkv_l_sb") if use_bqkv_l else None
        bv_l_sb = load_bias(bv_l, [128, D], "bv_l_sb") if use_bqkv_l else None
        bqkv_g_sb = load_bias(bqkv_g, [128, 3, 4], "bqkv_g_sb") if use_bqkv_g else None
        bv_g_sb = load_bias(bv_g, [128, D], "bv_g_sb") if use_bqkv_g else None
        bo2_sb = load_bias(bo2, [128, 2, 4], "bo2_sb") if use_bo else None
        gate_b_sb = load_bias(gate_b, [128, 4], "gate_b_sb") if use_gate_b else None
        b1_sb = load_bias(b1, [128, 8], "b1_sb") if use_b1 else None
        b2b_sb = load_bias(b2b, [128, D], "b2b_sb") if use_b2 else None
        n1gb_sb = load_bias(n1gb, [128, D], "n1gb_sb") if use_n1g else None
        n1bb_sb = load_bias(n1bb, [128, D], "n1bb_sb") if use_n1b else None
        n2gb_sb = load_bias(n2gb, [128, D], "n2gb_sb") if use_n2g else None
        n2bb_sb = load_bias(n2bb, [128, D], "n2bb_sb") if use_n2b else None
        n3gb_sb = load_bias(n3gb, [128, D], "n3gb_sb") if use_n3g else None

        # long-lived stream tiles
        hT = [hTp.tile([128, S], F32R, name=f"hT{m}", tag="hT", bufs=4)
              for m in range(4)]
        h_sb = [s2.tile([128, D], F32R, name=f"h{t}", tag="s2") for t in range(8)]

        # ============ Phase A: hT + h ======================================
        # posb lands directly in hT via DMA; matmul results accumulate into it
        for m in range(4):
            nc.sync.dma_start(
                hT[m][:], posb.rearrange("(t p) n -> p t n", p=128)[:, m, :])
        with ExitStack() as sA:
            pA = sA.enter_context(tc.tile_pool(name="pA", bufs=2))
            win_sb = pA.tile([128, 2, D], F32R, name="win_sb", tag="win", bufs=1)
            nc.sync.dma_start(win_sb[:], win.rearrange("(t p) n -> p t n", p=128))
            for c in range(2):
                xTc = pA.tile([128, 2, 1024], F32R, name=f"xTc{c}", tag="xTc")
                nc.sync.dma_start(
                    xTc[:], xT.rearrange("(t p) n -> p t n", p=128)
                    [:, :, c * 1024:(c + 1) * 1024])
                for m in range(4):
                    for hh in range(2):
                        acc = ps.tile([128, 512], F32, name=f"psA{m}{c}{hh}",
                                      tag="ps")
                        for kt in range(2):
                            nc.tensor.matmul(
                                acc[:], win_sb[:, kt, m * 128:(m + 1) * 128],
                                xTc[:, kt, hh * 512:(hh + 1) * 512],
                                start=(kt == 0), stop=(kt == 1))
                        sl = hT[m][:, c * 1024 + hh * 512:
                                   c * 1024 + (hh + 1) * 512]
                        nc.vector.tensor_tensor(sl, acc[:], sl, op=ALU.add)
        # token-major h for core's tokens (rotated [512, 1536))
        for t in range(8):
            for m in range(4):
                ptr = ps.tile([128, 128], F32, name=f"ptrA{t}{m}", tag="ps")
                nc.tensor.transpose(
                    ptr[:], f32(hT[m][:, Q0 + t * 128: Q0 + (t + 1) * 128]),
                    eye_sb[:])
                nc.vector.tensor_copy(
                    h_sb[t][:, m * 128:(m + 1) * 128], ptr[:])
        if debug:
            nc.sync.dma_start(dbg["d_hT"][:], f32(hT[0][:]))

        # ============ helper: qkv projection ================================
        def project_qkv(wqkv_sb, bias_sb, bv_sb, q_tiles, kT_tiles, v_tiles,
                        kT_lo, kT_hi, v_pt_lo, pfx, kv_on_act=True):
            for m in range(4):
                for n in range(2):
                    acc = ps.tile([128, 512], F32, name=f"{pfx}q{m}{n}", tag="ps")
                    for kt in range(4):
                        nc.tensor.matmul(
                            acc[:], wqkv_sb[:, 0, kt, m * 128:(m + 1) * 128],
                            hT[kt][:, Q0 + n * 512: Q0 + (n + 1) * 512],
                            start=(kt == 0), stop=(kt == 3))
                    dst = q_tiles[m].bitcast(BF16)[:, n * 512:(n + 1) * 512]
                    if bias_sb is not None:
                        nc.vector.tensor_scalar(
                            dst, acc[:], bias_sb[:, 0, m:m + 1], None,
                            op0=ALU.add)
                    else:
                        nc.vector.tensor_copy(dst, acc[:])
            nk = kT_hi - kT_lo
            for m in range(4):
                for off in range(0, nk, 512):
                    w_ = min(512, nk - off)
                    acc = ps.tile([128, 512], F32, name=f"{pfx}k{m}{off}",
                                  tag="ps")
                    for kt in range(4):
                        nc.tensor.matmul(
                            acc[:, 0:w_], wqkv_sb[:, 1, kt, m * 128:(m + 1) * 128],
                            hT[kt][:, kT_lo + off: kT_lo + off + w_],
                            start=(kt == 0), stop=(kt == 3))
                    dst = kT_tiles[m].bitcast(BF16)[:, off:off + w_]
                    if bias_sb is not None:
                        if kv_on_act:
                            nc.scalar.activation(dst, acc[:, 0:w_], AF.Identity,
                                                 bias=bias_sb[:, 1, m:m + 1])
                        else:
                            nc.vector.tensor_scalar(
                                dst, acc[:, 0:w_], bias_sb[:, 1, m:m + 1], None,
                                op0=ALU.add)
                    elif kv_on_act:
                        nc.scalar.copy(dst, acc[:, 0:w_])
                    else:
                        nc.vector.tensor_copy(dst, acc[:, 0:w_])
            for i, vt in enumerate(v_tiles):
                pt = v_pt_lo + i
                acc = ps.tile([128, 512], F32, name=f"{pfx}v{pt}", tag="ps")
                for kt in range(4):
                    nc.tensor.matmul(
                        acc[:], hT[kt][:, pt * 128:(pt + 1) * 128],
                        wqkv_sb[:, 2, kt, :], start=(kt == 0), stop=(kt == 3))
                dst3 = vt.bitcast(BF16)[:, :, 0:64]
                src3 = acc[:].rearrange("p (h e) -> p h e", h=8)
                if bv_sb is not None:
                    nc.vector.tensor_tensor(
                        dst3, src3,
                        f32(bv_sb[:]).rearrange("p (h e) -> p h e", h=8),
                        op=ALU.add)
                elif kv_on_act:
                    nc.scalar.copy(dst3, src3)
                else:
                    nc.vector.tensor_copy(dst3, src3)
                nc.gpsimd.memset(vt.bitcast(BF16)[:, :, 64:65], 1.0)

        # ============ helper: softmax-normalize attention head ==============
        def normalize(ps_o, oT_tile, r0, c0, pfx):
            recip = lnp.tile([1, 512], F32, name=f"{pfx}r", tag="recip")
            nc.vector.reciprocal(recip[:], ps_o[64:65, :])
            rb = lnp.tile([64, 512], F32, name=f"{pfx}rb", tag="rb")
            nc.gpsimd.partition_broadcast(rb[:], recip[:])
            nc.vector.tensor_tensor(
                oT_tile.bitcast(BF16)[r0:r0 + 64, c0:c0 + 512],
                ps_o[0:64, :], rb[:], op=ALU.mult)

        # ============ helper: out-projection (feature-major) ================
        def out_proj(oT, outT, wo_sb, li, pfx):
            for m in range(4):
                for n in range(2):
                    acc = ps.tile([128, 512], F32, name=f"{pfx}{m}{n}", tag="ps")
                    for kt in range(4):
                        nc.tensor.matmul(
                            acc[:], wo_sb[:, li, kt, m * 128:(m + 1) * 128],
                            oT[kt].bitcast(BF16)[:, n * 512:(n + 1) * 512],
                            start=(kt == 0), stop=(kt == 3))
                    dst = outT[m].bitcast(BF16)[:, n * 512:(n + 1) * 512]
                    if use_bo:
                        nc.scalar.activation(dst, acc[:], AF.Identity,
                                             bias=bo2_sb[:, li, m:m + 1])
                    else:
                        nc.scalar.copy(dst, acc[:])

        # ============ Phase B: local qkv ====================================
        qT_l = [qTp.tile([128, NQ], BF16, name=f"qTl{m}", tag="qT")
                for m in range(4)]
        kT_l = [kTp.tile([128, S], BF16, name=f"kTl{m}", tag="kT")
                for m in range(4)]
        V_l = [Vp.tile([128, 8, 65], BF16, name=f"Vl{pt}", tag="V")
               for pt in range(KL0 // 128, KL1 // 128)]
        wqkv_l_sb = wp.tile([128, 3, 4, D], F32R, name="wqkv_l_sb", tag="wbig")
        nc.sync.dma_start(
            wqkv_l_sb[:], wqkv_l.rearrange("w (t p) d -> p w t d", p=128))
        project_qkv(wqkv_l_sb, bqkv_l_sb, bv_l_sb, qT_l, kT_l, V_l,
                    KL0, KL1, KL0 // 128, "Bl", kv_on_act=LOCAL_KV_ON_ACT)

        # ============ Phase C: local (band) attention + out-proj ============
        oT_l = [s4.tile([128, NQ], BF16, name=f"oTl{m}", tag="s4a", bufs=4)
                for m in range(4)]
        with ExitStack() as sC:
            pC = sC.enter_context(tc.tile_pool(name="pC", bufs=1))
            masks_m_sb = pC.tile([128, 4, 512], BF16, name="masks_m_sb")
            nc.scalar.dma_start(masks_m_sb[:], masks_m[:])
            masks_e_sb = pC.tile([128, 2, 2, 32], BF16, name="masks_e_sb")
            nc.sync.dma_start(masks_e_sb[:], masks_e[:])
            MAIN_DELTAS = (0, 128, 256, 384)
            PT = {}
            for di, dd in enumerate(MAIN_DELTAS):
                t = pC.tile([128, 2, 512], BF16, name=f"PTl{di}")
                nc.gpsimd.memset(t[:], 0.0)
                PT[dd] = t
            for de_i, de in enumerate(EDGE_DELTAS):
                PT[de] = pC.tile([128, 2, 32], BF16, name=f"PTe{de_i}")
            for qb in range(2):
                q0 = Q0 + qb * 512
                for hp in range(4):
                    for di, dd in enumerate(MAIN_DELTAS):
                        qq0, qq1 = STRIPE[dd]
                        rel = q0 + dd - KL0
                        sc2 = ps2.tile([128, 2, 512], F32,
                                       name=f"psC{qb}{hp}{di}", tag="ps2")
                        for ab in range(2):
                            r0 = ab * 64
                            nc.tensor.matmul(
                                sc2[:, ab, qq0:qq1],
                                kT_l[hp].bitcast(BF16)[r0:r0 + 64, rel:rel + 128],
                                qT_l[hp].bitcast(BF16)
                                [r0:r0 + 64, qb * 512 + qq0: qb * 512 + qq1],
                                start=True, stop=True, tile_position=(r0, 0))
                        pt_t = PT[dd]
                        nc.scalar.activation(
                            pt_t[:, :, qq0:qq1], sc2[:, :, qq0:qq1],
                            AF.Exp, scale=SCALE)
                        nc.vector.tensor_tensor(
                            pt_t[:, :, qq0:qq1], pt_t[:, :, qq0:qq1],
                            masks_m_sb[:, di, qq0:qq1].unsqueeze(1)
                            .to_broadcast((128, 2, qq1 - qq0)), op=ALU.mult)
                    for de_i, de in enumerate(EDGE_DELTAS):
                        qq0, qq1 = STRIPE[de]
                        rel = q0 + de - KL0
                        sc2 = ps2.tile([128, 2, 512], F32,
                                       name=f"psCe{qb}{hp}{de_i}", tag="ps2")
                        for ab in range(2):
                            r0 = ab * 64
                            nc.tensor.matmul(
                                sc2[:, ab, 0:32],
                                kT_l[hp].bitcast(BF16)[r0:r0 + 64, rel:rel + 128],
                                qT_l[hp].bitcast(BF16)
                                [r0:r0 + 64, qb * 512 + qq0: qb * 512 + qq1],
                                start=True, stop=True, tile_position=(r0, 0))
                        pt_t = PT[de]
                        nc.scalar.activation(
                            pt_t[:], sc2[:, :, 0:32], AF.Exp, scale=SCALE)
                        nc.vector.tensor_tensor(
                            pt_t[:], pt_t[:],
                            masks_e_sb[:, de_i, qb, :].unsqueeze(1)
                            .to_broadcast((128, 2, 32)), op=ALU.mult)
                    for ab in range(2):
                        head = 2 * hp + ab
                        po_t = pso.tile([65, 512], F32, name=f"psoC{qb}{hp}{ab}",
                                        tag=f"pso{ab}", bufs=1)
                        nc.tensor.matmul(
                            po_t[:], V_l[(q0 - KL0) // 128].bitcast(BF16)[:, head, :],
                            PT[0][:, ab, :], start=True, stop=False,
                            skip_group_check=True)
                        for de in EDGE_DELTAS:
                            qq0, qq1 = STRIPE[de]
                            nc.tensor.matmul(
                                po_t[:, qq0:qq1],
                                V_l[(q0 + de - KL0) // 128].bitcast(BF16)[:, head, :],
                                PT[de][:, ab, :],
                                start=False, stop=False, skip_group_check=True)
                        for dd in (128, 256, 384):
                            nc.tensor.matmul(
                                po_t[:],
                                V_l[(q0 + dd - KL0) // 128].bitcast(BF16)[:, head, :],
                                PT[dd][:, ab, :], start=False, stop=(dd == 384),
                                skip_group_check=True)
                        normalize(po_t, oT_l[hp], ab * 64, qb * 512,
                                  f"nC{qb}{hp}{ab}")
        if debug:
            nc.sync.dma_start(dbg["d_oTl"][:], oT_l[0].bitcast(BF16)[:])

        wo_sb = wp.tile([128, 2, 4, D], BF16, name="wo_sb", tag="wo2nd")
        nc.scalar.dma_start(wo_sb[:], wo2.rearrange("w (t p) d -> p w t d", p=128))
        localT = [s4.tile([128, NQ], BF16, name=f"localT{m}", tag="s4b", bufs=4)
                  for m in range(4)]
        out_proj(oT_l, localT, wo_sb, 0, "psFl")

        # ============ Phase D: global qkv ===================================
        qT_g = [qTp.tile([128, NQ], BF16, name=f"qTg{m}", tag="qT")
                for m in range(4)]
        kT_g = [kTp.tile([128, S], BF16, name=f"kTg{m}", tag="kT")
                for m in range(4)]
        V_g = [Vp.tile([128, 8, 65], BF16, name=f"Vg{pt}", tag="V")
               for pt in range(16)]
        wqkv_g_sb = wp.tile([128, 3, 4, D], F32R, name="wqkv_g_sb", tag="wbig")
        nc.scalar.dma_start(
            wqkv_g_sb[:], wqkv_g.rearrange("w (t p) d -> p w t d", p=128))
        project_qkv(wqkv_g_sb, bqkv_g_sb, bv_g_sb, qT_g, kT_g, V_g, 0, S, 0, "Dg", kv_on_act=GLOBAL_KV_ON_ACT)

        # ============ Phase E: global attention + out-proj ==================
        oT_g = [s4.tile([128, NQ], BF16, name=f"oTg{m}", tag="s4c", bufs=8)
                for m in range(4)]
        for qb in range(2):
            for hp in range(4):
                po_ts = [pso.tile([65, 512], F32, name=f"psoE{qb}{hp}{ab}",
                                  tag=f"pso{ab}", bufs=1) for ab in range(2)]
                for kt in range(16):
                    sc2 = ps2.tile([128, 2, 512], F32,
                                   name=f"psE{qb}{hp}{kt}", tag="ps2")
                    for ab in range(2):
                        r0 = ab * 64
                        nc.tensor.matmul(
                            sc2[:, ab, :], kT_g[hp].bitcast(BF16)
                            [r0:r0 + 64, kt * 128:(kt + 1) * 128],
                            qT_g[hp].bitcast(BF16)
                            [r0:r0 + 64, qb * 512:(qb + 1) * 512],
                            start=True, stop=True, tile_position=(r0, 0))
                    ptg = ptgp.tile([128, 2, 512], BF16,
                                    name=f"ptg{qb}{hp}{kt}", tag="ptg")
                    nc.scalar.activation(ptg[:], sc2[:], AF.Exp, scale=SCALE)
                    for ab in range(2):
                        nc.tensor.matmul(
                            po_ts[ab][:],
                            V_g[kt].bitcast(BF16)[:, 2 * hp + ab, :],
                            ptg[:, ab, :], start=(kt == 0), stop=(kt == 15),
                            skip_group_check=True)
                for ab in range(2):
                    normalize(po_ts[ab], oT_g[hp], ab * 64, qb * 512,
                              f"nE{qb}{hp}{ab}")
        if debug:
            nc.sync.dma_start(dbg["d_oTg"][:], oT_g[0].bitcast(BF16)[:])

        globalT = [s4.tile([128, NQ], BF16, name=f"globalT{m}", tag="s4c", bufs=8)
                   for m in range(4)]
        out_proj(oT_g, globalT, wo_sb, 1, "psFg")

        # ============ Phase G: gate + fuse ==================================
        fusedT = [s4.tile([128, NQ], BF16, name=f"fusedT{m}", tag="s4a", bufs=4)
                  for m in range(4)]
        gate_w_sb = wp.tile([128, 8, D], BF16, name="gate_w_sb", tag="wbig")
        nc.scalar.dma_start(gate_w_sb[:],
                          gate_w.rearrange("(t p) d -> p t d", p=128))
        cat = localT + globalT
        for m in range(4):
            for n in range(2):
                acc = ps.tile([128, 512], F32, name=f"psG{m}{n}", tag="ps")
                for kt in range(8):
                    nc.tensor.matmul(
                        acc[:], gate_w_sb[:, kt, m * 128:(m + 1) * 128],
                        cat[kt].bitcast(BF16)[:, n * 512:(n + 1) * 512],
                        start=(kt == 0), stop=(kt == 7))
                gt = lnp.tile([128, 512], BF16, name=f"gt{m}{n}", tag="gt", bufs=1)
                if use_gate_b:
                    nc.vector.tensor_scalar(
                        gt[:], acc[:], gate_b_sb[:, m:m + 1], 0.0,
                        op0=ALU.add, op1=ALU.max)
                else:
                    nc.vector.tensor_scalar(gt[:], acc[:], 0.0, None,
                                            op0=ALU.max)
                nc.scalar.activation(gt[:], gt[:], AF.Tanh)
                if debug and m == 0 and n == 0:
                    nc.sync.dma_start(dbg["d_gateT"][:], gt[:])
                # fused = global + gate*(local - global)
                lsl = localT[m].bitcast(BF16)[:, n * 512:(n + 1) * 512]
                gsl = globalT[m].bitcast(BF16)[:, n * 512:(n + 1) * 512]
                tmp = lnp.tile([128, 512], BF16, name=f"tmpG{m}{n}", tag="tmpG", bufs=1)
                nc.gpsimd.tensor_tensor(tmp[:], lsl, gsl, op=ALU.subtract)
                nc.vector.tensor_tensor(tmp[:], tmp[:], gt[:], op=ALU.mult)
                nc.vector.tensor_tensor(
                    fusedT[m].bitcast(BF16)[:, n * 512:(n + 1) * 512],
                    tmp[:], gsl, op=ALU.add)
        if debug:
            nc.sync.dma_start(dbg["d_fusedT"][:], fusedT[0].bitcast(BF16)[:])

        # ===== layernorm helper (token-major [128, D]) ======================
        def layernorm(dst, src_ap, g_sb, b_sb, pfx):
            stats = lnp.tile([128, 6], F32, name=f"{pfx}st", tag="lnst")
            nc.vector.bn_stats(stats[:], src_ap)
            mv = lnp.tile([128, 2], F32, name=f"{pfx}mv", tag="lnmv")
            nc.vector.bn_aggr(mv[:], stats[:])
            std = lnp.tile([128, 1], F32, name=f"{pfx}sd", tag="lnsd")
            nc.scalar.activation(std[:], mv[:, 1:2], AF.Sqrt, bias=eps_sb[:])
            rstd = lnp.tile([128, 1], F32, name=f"{pfx}rs", tag="lnrs")
            nc.vector.reciprocal(rstd[:], std[:])
            if g_sb is not None:
                tmp = lnp.tile([128, D], F32, name=f"{pfx}tmp", tag="lntmp")
                nc.vector.tensor_scalar(
                    tmp[:], src_ap, mv[:, 0:1], rstd[:],
                    op0=ALU.subtract, op1=ALU.mult)
                if b_sb is not None:
                    nc.vector.tensor_tensor(dst, tmp[:], g_sb[:], op=ALU.mult)
                    nc.vector.tensor_tensor(dst, dst, b_sb[:], op=ALU.add)
                else:
                    nc.vector.tensor_tensor(dst, tmp[:], g_sb[:], op=ALU.mult)
            else:
                nc.vector.tensor_scalar(
                    dst, src_ap, mv[:, 0:1], rstd[:],
                    op0=ALU.subtract, op1=ALU.mult)
                if b_sb is not None:
                    nc.vector.tensor_tensor(dst, dst, b_sb[:], op=ALU.add)

        # ============ Phase H: x1 = h + fused^T; y1 = LN1 ===================
        y1 = [s2.tile([128, D], F32R, name=f"y1_{t}", tag="s2") for t in range(8)]
        for t in range(8):
            x1 = lnp.tile([128, D], F32, name=f"x1_{t}", tag="x1")
            for m in range(4):
                ptr = ps.tile([128, 128], BF16, name=f"ptrH{t}{m}", tag="ps")
                nc.tensor.transpose(
                    ptr[:], fusedT[m].bitcast(BF16)[:, t * 128:(t + 1) * 128],
                    eyeb_sb[:])
                nc.vector.tensor_tensor(
                    x1[:, m * 128:(m + 1) * 128],
                    f32(h_sb[t][:, m * 128:(m + 1) * 128]), ptr[:], op=ALU.add)
            layernorm(y1[t][:], x1[:], n1gb_sb, n1bb_sb, f"ln1_{t}")
        if debug:
            nc.sync.dma_start(dbg["d_y1"][:], f32(y1[0][:]))

        # ============ Phase I: y1T ==========================================
        y1T = [s4.tile([128, NQ], BF16, name=f"y1T{m}", tag="s4b", bufs=4)
               for m in range(4)]
        for t in range(8):
            for m in range(4):
                ptr = ps.tile([128, 128], F32, name=f"ptrI{t}{m}", tag="ps")
                nc.tensor.transpose(ptr[:], f32(y1[t][:, m * 128:(m + 1) * 128]),
                                    eye_sb[:])
                nc.scalar.copy(
                    y1T[m].bitcast(BF16)[:, t * 128:(t + 1) * 128], ptr[:])

        # ============ Phase J: FFN + LN2 + LN3; Phase K: pool + out =========
        w1_sb = wp.tile([128, 4, DFF], BF16, name="w1_sb", tag="wbig")
        nc.scalar.dma_start(w1_sb[:], w1.rearrange("(t p) d -> p t d", p=128))
        w2_sb = wp.tile([128, 8, D], BF16, name="w2_sb", tag="wo2nd")
        nc.scalar.dma_start(w2_sb[:], w2.rearrange("(t p) d -> p t d", p=128))
        z1T = [s4.tile([128, NQ], BF16, name=f"z1T{m}", tag="s4c", bufs=8)
               for m in range(8)]
        for m in range(8):
            for n in range(2):
                acc = ps.tile([128, 512], F32, name=f"psJ1{m}{n}", tag="ps")
                for kt in range(4):
                    nc.tensor.matmul(
                        acc[:], w1_sb[:, kt, m * 128:(m + 1) * 128],
                        y1T[kt].bitcast(BF16)[:, n * 512:(n + 1) * 512],
                        start=(kt == 0), stop=(kt == 3))
                dst = z1T[m].bitcast(BF16)[:, n * 512:(n + 1) * 512]
                if use_b1:
                    nc.vector.tensor_scalar(
                        dst, acc[:], b1_sb[:, m:m + 1], 0.0,
                        op0=ALU.add, op1=ALU.max)
                else:
                    nc.vector.tensor_scalar(dst, acc[:], 0.0, None, op0=ALU.max)

        y3 = [s2.tile([128, D], F32R, name=f"y3_{t}", tag="s2") for t in range(8)]
        accp = pso.tile([1, 512], F32, name="pspool", tag="pso0", bufs=1)
        for t in range(8):
            acc = ps.tile([128, 512], F32, name=f"psJ2{t}", tag="ps")
            for kt in range(8):
                nc.tensor.matmul(
                    acc[:], z1T[kt].bitcast(BF16)[:, t * 128:(t + 1) * 128],
                    w2_sb[:, kt, :], start=(kt == 0), stop=(kt == 7))
            x2 = lnp.tile([128, D], F32, name=f"x2_{t}", tag="x2")
            nc.vector.tensor_tensor(x2[:], acc[:], f32(y1[t][:]), op=ALU.add)
            if use_b2:
                nc.vector.tensor_tensor(x2[:], x2[:], b2b_sb[:], op=ALU.add)
            if not (use_n2g or use_n2b or use_n3g):
                # LN3(LN2(x)) with unit gamma / zero beta collapses to one LN:
                # mean(LN2 out) == 0 exactly, var(LN2 out) = v/(v+eps), so
                # y3 = (x - m) / sqrt(v*(1+eps) + eps^2)
                pfx = f"ln23_{t}"
                stats = lnp.tile([128, 6], F32, name=f"{pfx}st", tag="lnst")
                nc.vector.bn_stats(stats[:], x2[:])
                mv = lnp.tile([128, 2], F32, name=f"{pfx}mv", tag="lnmv")
                nc.vector.bn_aggr(mv[:], stats[:])
                std = lnp.tile([128, 1], F32, name=f"{pfx}sd", tag="lnsd")
                nc.scalar.activation(std[:], mv[:, 1:2], AF.Sqrt,
                                     bias=eps2_sb[:], scale=1.0 + EPS)
                rstd = lnp.tile([128, 1], F32, name=f"{pfx}rs", tag="lnrs")
                nc.vector.reciprocal(rstd[:], std[:])
                nc.vector.tensor_scalar(
                    y3[t][:], x2[:], mv[:, 0:1], rstd[:],
                    op0=ALU.subtract, op1=ALU.mult)
            else:
                y2 = lnp.tile([128, D], F32, name=f"y2_{t}", tag="y2")
                layernorm(y2[:], x2[:], n2gb_sb, n2bb_sb, f"ln2_{t}")
                layernorm(y3[t][:], y2[:], n3gb_sb, None, f"ln3_{t}")
            nc.tensor.matmul(accp[:], poolw_sb[:], y3[t][:],
                             start=(t == 0), stop=(t == 7),
                             skip_group_check=True)
        if debug:
            nc.sync.dma_start(dbg["d_y3"][:], f32(y3[0][:]))

        pooled_sb = pers.tile([1, D], F32, name="pooled_sb")
        nc.vector.tensor_copy(pooled_sb[:], accp[:])
        if debug:
            nc.sync.dma_start(dbg["d_pooled"][:], f32(pooled_sb[:]))
        nc.sync.dma_start(po[:], pooled_sb[:])

    nc.compile()
    return nc


def _prep_inputs(inputs):
    """Host-side prep: returns (flags, in_maps for 8 cores, host_const)."""
    g = {k: np.asarray(v, dtype=np.float32) for k, v in inputs.items()}
    x, pos = g["x"], g["pos"]
    win_w, win_b = g["win_w"], g["win_b"]

    flags = (
        bool(np.any(g["l_bqkv"] != 0)), bool(np.any(g["g_bqkv"] != 0)),
        bool(np.any(g["l_bo"] != 0) or np.any(g["g_bo"] != 0)),
        bool(np.any(g["gate_b"] != 0)), bool(np.any(g["ffn_b1"] != 0)),
        bool(np.any(g["ffn_b2"] != 0)),
        bool(np.any(g["n1_g"] != 1)), bool(np.any(g["n1_b"] != 0)),
        bool(np.any(g["n2_g"] != 1)), bool(np.any(g["n2_b"] != 0)),
        bool(np.any(g["n3_g"] != 1)),
    )
    (use_bqkv_l, use_bqkv_g, use_bo, use_gate_b, use_b1, use_b2,
     use_n1g, use_n1b, use_n2g, use_n2b, use_n3g) = flags

    posT = pos[0].T + win_b[:, None]                      # [D, S]
    common = {
        "win": np.ascontiguousarray(win_w),
        "wqkv_l": np.ascontiguousarray(g["l_wqkv"]),
        "wqkv_g": np.ascontiguousarray(g["g_wqkv"]),
        "wo2": np.stack([g["l_wo"], g["g_wo"]]).astype(ml_dtypes.bfloat16),
        "gate_w": g["gate_w"].astype(ml_dtypes.bfloat16),
        "w1": g["ffn_w1"].astype(ml_dtypes.bfloat16),
        "w2": g["ffn_w2"].astype(ml_dtypes.bfloat16),
        "eye": np.eye(128, dtype=np.float32),
        "poolw": np.full((128, 1), 1.0 / S, dtype=np.float32),
    }
    perm = lambda b: b.reshape(-1, 4, 128).transpose(2, 0, 1).copy()
    if use_bqkv_l:
        common["bqkv_l"] = perm(g["l_bqkv"])
        common["bv_l"] = np.tile(g["l_bqkv"][2], (128, 1))
    if use_bqkv_g:
        common["bqkv_g"] = perm(g["g_bqkv"])
        common["bv_g"] = np.tile(g["g_bqkv"][2], (128, 1))
    if use_bo:
        common["bo2"] = perm(np.stack([g["l_bo"], g["g_bo"]]))
    if use_gate_b:
        common["gate_b"] = g["gate_b"].reshape(4, 128).T.copy()
    if use_b1:
        common["b1"] = g["ffn_b1"].reshape(8, 128).T.copy()
    if use_b2:
        common["b2b"] = np.tile(g["ffn_b2"], (128, 1))
    if use_n1g:
        common["n1gb"] = np.tile(g["n1_g"], (128, 1))
    if use_n1b:
        common["n1bb"] = np.tile(g["n1_b"], (128, 1))
    if use_n2g:
        common["n2gb"] = np.tile(g["n2_g"], (128, 1))
    if use_n2b:
        common["n2bb"] = np.tile(g["n2_b"], (128, 1))
    if use_n3g:
        common["n3gb"] = np.tile(g["n3_g"], (128, 1))

    # universal interior band masks (pure Toeplitz, no seam crossing)
    kk = np.arange(128)
    qq = np.arange(512)
    mk_m = np.zeros((128, 4, 512), dtype=np.float32)
    for di, d in enumerate((0, 128, 256, 384)):
        mk_m[:, di, :] = (np.abs(kk[:, None] + d - qq[None, :]) <= W // 2)
    mk_m = mk_m.astype(ml_dtypes.bfloat16)

    hf_data = []
    for hf in range(2):
        q0c = NQ * hf
        shift = Q0 - q0c
        posb_rot = np.ascontiguousarray(np.roll(posT, shift, axis=1))
        mk_e = np.zeros((128, 2, 2, 32), dtype=np.float32)
        for qb in range(2):
            q0 = Q0 + qb * 512
            for de_i, d in enumerate(EDGE_DELTAS):
                qq0, qq1 = STRIPE[d]
                k_rot = q0 + d + kk[:, None]
                q_rot = q0 + np.arange(qq0, qq1)[None, :]
                orig_k = (k_rot - shift) % S
                orig_q = (q_rot - shift) % S
                mk_e[:, de_i, qb, :] = (np.abs(orig_k - orig_q) <= W // 2)
        hf_data.append((posb_rot, mk_e.astype(ml_dtypes.bfloat16)))

    in_maps = []
    for core in range(N_CORES):
        b, hf = core // 2, core % 2
        shift = Q0 - NQ * hf
        posb_rot, mk_e = hf_data[hf]
        m = dict(common)
        m["xT"] = np.ascontiguousarray(np.roll(x[b].T, shift, axis=1))
        m["posb"] = posb_rot
        m["masks_m"] = mk_m
        m["masks_e"] = mk_e
        in_maps.append(m)

    host_const = (g["n3_b"] @ g["out_w"] + g["out_b"],
                  np.ascontiguousarray(g["out_w"]))
    return flags, in_maps, host_const


def kernel(**inputs):
    flags, in_maps, host_const = _prep_inputs(inputs)
    const_vec, out_w = host_const
    if flags not in _CACHE:
        _CACHE[flags] = _build(flags)
    nc = _CACHE[flags]
    res = run_bass_kernel_spmd(nc, in_maps, core_ids=list(range(N_CORES)))
    out = np.zeros((B, DOUT), dtype=np.float32)
    for b in range(B):
        pooled = res.results[2 * b]["po"][0] + res.results[2 * b + 1]["po"][0]
        out[b] = pooled @ out_w + const_vec
    return out



# revision 13
# speedup vs baseline: 1.0791x; 1.0614x over previous
"""DualPathTransformer Trainium2 kernel.

Sharding: 8 cores = batch(4) x query-half(2). Each core processes one batch
and 1024 query tokens; K/V work is duplicated within a batch pair. No
device collectives: partial pooled projections are summed on the host.

SPMD uniformity trick: each core receives its batch token-ROTATED so that
its query tokens sit at rotated positions [512, 1536). Global attention is
permutation-invariant over keys; the local band structure is encoded in
host-prepped per-core mask tiles in true original coordinates. The program
is identical on all cores; only input data differs.

Layouts: activations feature-major (hT = [feature partitions, tokens]) for
matmuls; token-major (tokens on partitions) for layernorm stages. Scores
are computed transposed (keys on partitions) so softmax denominators come
free from a ones-row appended to V, and the AV matmul needs no transposes.

Precision: residual stream and weights fp32/f32r; attention q/k/v/probs and
post-attention projections bf16 (error contribution ~1e-3 of the stream).
"""

import numpy as np
import ml_dtypes
from contextlib import ExitStack

import concourse.bass as bass
import concourse.bacc as bacc
import concourse.tile as tile
import concourse.mybir as mybir
from concourse.bass_utils import run_bass_kernel_spmd

F32R = mybir.dt.float32r
F32 = mybir.dt.float32
BF16 = mybir.dt.bfloat16
FP8 = mybir.dt.float8e4
NPF8 = ml_dtypes.float8_e4m3
AF = mybir.ActivationFunctionType
ALU = mybir.AluOpType
DRM = mybir.MatmulPerfMode.DoubleRow
WS = 64.0          # fp8 weight pre-scale (host); compensated at psum drain
WSI = 1.0 / WS

B, S, DIN, D, H, DOUT, W = 4, 2048, 256, 512, 8, 128, 64
HD = D // H          # 64
DFF = 2 * D          # 1024
NQ = S // 2          # 1024 queries per core
N_CORES = 8
Q0 = 512             # rotated position of first query token (uniform)
KL0, KL1 = 384, 1664   # local K/V window in rotated coords (10 ptiles)
NKL = KL1 - KL0        # 1280
DELTAS = (-128, 0, 128, 256, 384, 512)   # local kblock offsets rel. to qblock
# stripe (bounding qq range) per delta, qblock-relative
STRIPE = {-128: (0, 32), 0: (0, 160), 128: (96, 288),
          256: (224, 416), 384: (352, 512), 512: (480, 512)}
EDGE_DELTAS = (-128, 512)          # AV mms sliced to the stripe
SCALE = 1.0 / float(np.sqrt(HD))
EPS = 1e-5

_CACHE = {}
GLOBAL_KV_ON_ACT = False
LOCAL_KV_ON_ACT = True


def _build(flags, debug=False):
    (use_bqkv_l, use_bqkv_g, use_bo, use_gate_b, use_b1, use_b2,
     use_n1g, use_n1b, use_n2g, use_n2b, use_n3g) = flags

    nc = bacc.Bacc("TRN2", target_bir_lowering=False, debug=False)

    def din(name, shape, dt=F32R):
        return nc.dram_tensor(name, list(shape), dt, kind="ExternalInput").ap()

    xT = din("xT", [DIN, S])
    posb = din("posb", [D, S])
    win = din("win", [DIN, D])
    wqkv8_l = din("wqkv8_l", [128, 3, 2, 2, D], FP8)
    wqkv8_g = din("wqkv8_g", [128, 3, 2, 2, D], FP8)
    wo2 = din("wo2", [2, D, D], BF16)    # [0]=local, [1]=global
    gate_w = din("gate_w", [2 * D, D], BF16)
    w1 = din("w1", [D, DFF], BF16)
    w2 = din("w2", [DFF, D], BF16)
    masks_m = din("masks_m", [128, 4, 512], BF16)   # [kk, di, qq]
    masks_e = din("masks_e", [128, 2, 2, 32], BF16)  # [kk, de, qb, qq32]
    eye = din("eye", [128, 128], F32)
    poolw = din("poolw", [128, 1])
    if use_bqkv_l:
        bqkv_l = din("bqkv_l", [128, 3, 4], F32)
        bv_l = din("bv_l", [128, D], F32)
    if use_bqkv_g:
        bqkv_g = din("bqkv_g", [128, 3, 4], F32)
        bv_g = din("bv_g", [128, D], F32)
    if use_bo:
        bo2 = din("bo2", [128, 2, 4], F32)
    if use_gate_b:
        gate_b = din("gate_b", [128, 4], F32)
    if use_b1:
        b1 = din("b1", [128, 8], F32)
    if use_b2:
        b2b = din("b2b", [128, D], F32)
    if use_n1g:
        n1gb = din("n1gb", [128, D], F32)
    if use_n1b:
        n1bb = din("n1bb", [128, D], F32)
    if use_n2g:
        n2gb = din("n2gb", [128, D], F32)
    if use_n2b:
        n2bb = din("n2bb", [128, D], F32)
    if use_n3g:
        n3gb = din("n3gb", [128, D], F32)
    # n3_b handled on host (pooled mean is linear in it)

    po = nc.dram_tensor("po", [1, D], F32, kind="ExternalOutput").ap()

    dbg = {}
    if debug:
        for nm, shp, dt_ in [("d_hT", [128, S], F32), ("d_oTl", [128, NQ], BF16),
                             ("d_oTg", [128, NQ], BF16), ("d_gateT", [128, 512], BF16),
                             ("d_fusedT", [128, NQ], BF16), ("d_y1", [128, D], F32),
                             ("d_y3", [128, D], F32), ("d_pooled", [1, D], F32)]:
            dbg[nm] = nc.dram_tensor(nm, shp, dt_, kind="ExternalOutput").ap()

    f32 = lambda ap: ap.bitcast(F32)

    with tile.TileContext(nc) as tc, ExitStack() as top:
        # ---- psum pools (8 banks) ----
        ps = top.enter_context(tc.tile_pool(name="ps", bufs=2, space="PSUM"))
        ps2 = top.enter_context(tc.tile_pool(name="ps2", bufs=2, space="PSUM"))
        pso = top.enter_context(tc.tile_pool(name="pso", bufs=1, space="PSUM"))

        # ---- persistent pools (static tags, round-robin slot reuse) ----
        pers = top.enter_context(tc.tile_pool(name="pers", bufs=1))
        lnp = top.enter_context(tc.tile_pool(name="lnp", bufs=2))
        wp = top.enter_context(tc.tile_pool(name="wp", bufs=1))
        s4 = top.enter_context(tc.tile_pool(name="s4", bufs=1))     # [128,1024] bf16 tags
        s2 = top.enter_context(tc.tile_pool(name="s2", bufs=11))    # [128,512] f32
        qTp = top.enter_context(tc.tile_pool(name="qTp", bufs=4))   # [128,1024] bf16
        kTp = top.enter_context(tc.tile_pool(name="kTp", bufs=4))   # [128,2048] bf16
        hTp = top.enter_context(tc.tile_pool(name="hTp", bufs=1))
        Vp = top.enter_context(tc.tile_pool(name="Vp", bufs=16))    # [128,8,65] bf16
        ptgp = top.enter_context(tc.tile_pool(name="ptgp", bufs=3)) # pair bf16

        eye_sb = pers.tile([128, 128], F32, name="eye_sb")
        nc.sync.dma_start(eye_sb[:], eye[:])
        eyeb_sb = pers.tile([128, 128], BF16, name="eyeb_sb")
        nc.vector.tensor_copy(eyeb_sb[:], eye_sb[:])
        poolw_sb = pers.tile([128, 1], F32R, name="poolw_sb")
        nc.sync.dma_start(poolw_sb[:], poolw[:])
        eps_sb = pers.tile([128, 1], F32, name="eps_sb")
        nc.vector.memset(eps_sb[:], EPS)
        eps2_sb = pers.tile([128, 1], F32, name="eps2_sb")
        nc.vector.memset(eps2_sb[:], EPS * EPS)

        def load_bias(ap_dram, shape, name):
            t = pers.tile(shape, F32, name=name)
            nc.sync.dma_start(t[:], ap_dram[:])
            return t
        bqkv_l_sb = load_bias(bqkv_l, [128, 3, 4], "bqkv_l_sb") if use_bqkv_l else None
        bv_l_sb = load_bias(bv_l, [128, D], "bv_l_sb") if use_bqkv_l else None
        bqkv_g_sb = load_bias(bqkv_g, [128, 3, 4], "bqkv_g_sb") if use_bqkv_g else None
        bv_g_sb = load_bias(bv_g, [128, D], "bv_g_sb") if use_bqkv_g else None
        bo2_sb = load_bias(bo2, [128, 2, 4], "bo2_sb") if use_bo else None
        gate_b_sb = load_bias(gate_b, [128, 4], "gate_b_sb") if use_gate_b else None
        b1_sb = load_bias(b1, [128, 8], "b1_sb") if use_b1 else None
        b2b_sb = load_bias(b2b, [128, D], "b2b_sb") if use_b2 else None
        n1gb_sb = load_bias(n1gb, [128, D], "n1gb_sb") if use_n1g else None
        n1bb_sb = load_bias(n1bb, [128, D], "n1bb_sb") if use_n1b else None
        n2gb_sb = load_bias(n2gb, [128, D], "n2gb_sb") if use_n2g else None
        n2bb_sb = load_bias(n2bb, [128, D], "n2bb_sb") if use_n2b else None
        n3gb_sb = load_bias(n3gb, [128, D], "n3gb_sb") if use_n3g else None

        # long-lived stream tiles
        hT = [hTp.tile([128, S], F32R, name=f"hT{m}", tag="hT", bufs=4)
              for m in range(4)]
        h_sb = [s2.tile([128, D], F32R, name=f"h{t}", tag="s2") for t in range(8)]

        # ============ Phase A: hT + h ======================================
        # posb lands directly in hT via DMA; matmul results accumulate into it
        for m in range(4):
            nc.sync.dma_start(
                hT[m][:], posb.rearrange("(t p) n -> p t n", p=128)[:, m, :])
        with ExitStack() as sA:
            pA = sA.enter_context(tc.tile_pool(name="pA", bufs=2))
            win_sb = pA.tile([128, 2, D], F32R, name="win_sb", tag="win", bufs=1)
            nc.sync.dma_start(win_sb[:], win.rearrange("(t p) n -> p t n", p=128))
            for c in range(2):
                xTc = pA.tile([128, 2, 1024], F32R, name=f"xTc{c}", tag="xTc")
                nc.sync.dma_start(
                    xTc[:], xT.rearrange("(t p) n -> p t n", p=128)
                    [:, :, c * 1024:(c + 1) * 1024])
                for m in range(4):
                    for hh in range(2):
                        acc = ps.tile([128, 512], F32, name=f"psA{m}{c}{hh}",
                                      tag="ps")
                        for kt in range(2):
                            nc.tensor.matmul(
                                acc[:], win_sb[:, kt, m * 128:(m + 1) * 128],
                                xTc[:, kt, hh * 512:(hh + 1) * 512],
                                start=(kt == 0), stop=(kt == 1))
                        sl = hT[m][:, c * 1024 + hh * 512:
                                   c * 1024 + (hh + 1) * 512]
                        nc.vector.tensor_tensor(sl, acc[:], sl, op=ALU.add)
        # token-major h for core's tokens (rotated [512, 1536))
        for t in range(8):
            for m in range(4):
                ptr = ps.tile([128, 128], F32, name=f"ptrA{t}{m}", tag="ps")
                nc.tensor.transpose(
                    ptr[:], f32(hT[m][:, Q0 + t * 128: Q0 + (t + 1) * 128]),
                    eye_sb[:])
                nc.vector.tensor_copy(
                    h_sb[t][:, m * 128:(m + 1) * 128], ptr[:])
        # fp8 contraction-folded copy of hT for DoubleRow projections:
        # hT2[pp][p, j, n] = h[pp*256 + j*128 + p, n]
        hT2 = [hTp.tile([128, 2, S], FP8, name=f"hT2_{pp}", tag="hT2", bufs=2)
               for pp in range(2)]
        for pp in range(2):
            for j in range(2):
                nc.scalar.copy(hT2[pp][:, j, :], f32(hT[2 * pp + j][:]))
        if debug:
            nc.sync.dma_start(dbg["d_hT"][:], f32(hT[0][:]))

        # ============ helper: qkv projection (fp8 DoubleRow) ================
        def project_qkv(w8_sb, bias_sb, bv_sb, q_tiles, kT_tiles, v_tiles,
                        kT_lo, kT_hi, v_pt_lo, pfx, kv_on_act=True):
            for m in range(4):
                for n in range(2):
                    acc = ps.tile([128, 512], F32, name=f"{pfx}q{m}{n}", tag="ps")
                    for pp in range(2):
                        nc.tensor.matmul(
                            acc[:], w8_sb[:, 0, pp, :, m * 128:(m + 1) * 128],
                            hT2[pp][:, :, Q0 + n * 512: Q0 + (n + 1) * 512],
                            start=(pp == 0), stop=(pp == 1), perf_mode=DRM)
                    dst = q_tiles[m].bitcast(BF16)[:, n * 512:(n + 1) * 512]
                    if bias_sb is not None:
                        nc.vector.tensor_scalar(
                            dst, acc[:], WSI, bias_sb[:, 0, m:m + 1],
                            op0=ALU.mult, op1=ALU.add)
                    else:
                        nc.vector.tensor_scalar(
                            dst, acc[:], WSI, None, op0=ALU.mult)
            nk = kT_hi - kT_lo
            for m in range(4):
                for off in range(0, nk, 512):
                    w_ = min(512, nk - off)
                    acc = ps.tile([128, 512], F32, name=f"{pfx}k{m}{off}",
                                  tag="ps")
                    for pp in range(2):
                        nc.tensor.matmul(
                            acc[:, 0:w_], w8_sb[:, 1, pp, :, m * 128:(m + 1) * 128],
                            hT2[pp][:, :, kT_lo + off: kT_lo + off + w_],
                            start=(pp == 0), stop=(pp == 1), perf_mode=DRM)
                    dst = kT_tiles[m].bitcast(BF16)[:, off:off + w_]
                    bias_ap = bias_sb[:, 1, m:m + 1] if bias_sb is not None else 0.0
                    if kv_on_act:
                        nc.scalar.activation(dst, acc[:, 0:w_], AF.Identity,
                                             bias=bias_ap, scale=WSI)
                    else:
                        nc.vector.tensor_scalar(
                            dst, acc[:, 0:w_], WSI,
                            None if bias_sb is None else bias_ap,
                            op0=ALU.mult,
                            **({} if bias_sb is None else dict(op1=ALU.add)))
            for i, vt in enumerate(v_tiles):
                pt = v_pt_lo + i
                acc = ps.tile([128, 512], F32, name=f"{pfx}v{pt}", tag="ps")
                for pp in range(2):
                    nc.tensor.matmul(
                        acc[:], hT2[pp][:, :, pt * 128:(pt + 1) * 128],
                        w8_sb[:, 2, pp, :, :],
                        start=(pp == 0), stop=(pp == 1), perf_mode=DRM)
                dst3 = vt.bitcast(BF16)[:, :, 0:64]
                src3 = acc[:].rearrange("p (h e) -> p h e", h=8)
                if bv_sb is not None:
                    nc.vector.scalar_tensor_tensor(
                        dst3, src3, WSI,
                        f32(bv_sb[:]).rearrange("p (h e) -> p h e", h=8),
                        op0=ALU.mult, op1=ALU.add)
                elif kv_on_act:
                    nc.scalar.activation(dst3, src3, AF.Identity, scale=WSI)
                else:
                    nc.vector.tensor_scalar(dst3, src3, WSI, None, op0=ALU.mult)
                nc.gpsimd.memset(vt.bitcast(BF16)[:, :, 64:65], 1.0)

        # ============ helper: softmax-normalize attention head ==============
        def normalize(ps_o, oT_tile, r0, c0, pfx):
            recip = lnp.tile([1, 512], F32, name=f"{pfx}r", tag="recip")
            nc.vector.reciprocal(recip[:], ps_o[64:65, :])
            rb = lnp.tile([64, 512], F32, name=f"{pfx}rb", tag="rb")
            nc.gpsimd.partition_broadcast(rb[:], recip[:])
            nc.vector.tensor_tensor(
                oT_tile.bitcast(BF16)[r0:r0 + 64, c0:c0 + 512],
                ps_o[0:64, :], rb[:], op=ALU.mult)

        # ============ helper: out-projection (feature-major) ================
        def out_proj(oT, outT, wo_sb, li, pfx):
            for m in range(4):
                for n in range(2):
                    acc = ps.tile([128, 512], F32, name=f"{pfx}{m}{n}", tag="ps")
                    for kt in range(4):
                        nc.tensor.matmul(
                            acc[:], wo_sb[:, li, kt, m * 128:(m + 1) * 128],
                            oT[kt].bitcast(BF16)[:, n * 512:(n + 1) * 512],
                            start=(kt == 0), stop=(kt == 3))
                    dst = outT[m].bitcast(BF16)[:, n * 512:(n + 1) * 512]
                    if use_bo:
                        nc.scalar.activation(dst, acc[:], AF.Identity,
                                             bias=bo2_sb[:, li, m:m + 1])
                    else:
                        nc.scalar.copy(dst, acc[:])

        # ============ Phase B: local qkv ====================================
        qT_l = [qTp.tile([128, NQ], BF16, name=f"qTl{m}", tag="qT")
                for m in range(4)]
        kT_l = [kTp.tile([128, S], BF16, name=f"kTl{m}", tag="kT")
                for m in range(4)]
        V_l = [Vp.tile([128, 8, 65], BF16, name=f"Vl{pt}", tag="V")
               for pt in range(KL0 // 128, KL1 // 128)]
        wqkv_l_sb = wp.tile([128, 3, 2, 2, D], FP8, name="wqkv_l_sb", tag="wbig")
        nc.sync.dma_start(wqkv_l_sb[:], wqkv8_l[:])
        project_qkv(wqkv_l_sb, bqkv_l_sb, bv_l_sb, qT_l, kT_l, V_l,
                    KL0, KL1, KL0 // 128, "Bl", kv_on_act=LOCAL_KV_ON_ACT)

        # ============ Phase C: local (band) attention + out-proj ============
        oT_l = [s4.tile([128, NQ], BF16, name=f"oTl{m}", tag="s4a", bufs=4)
                for m in range(4)]
        with ExitStack() as sC:
            pC = sC.enter_context(tc.tile_pool(name="pC", bufs=1))
            masks_m_sb = pC.tile([128, 4, 512], BF16, name="masks_m_sb")
            nc.scalar.dma_start(masks_m_sb[:], masks_m[:])
            masks_e_sb = pC.tile([128, 2, 2, 32], BF16, name="masks_e_sb")
            nc.sync.dma_start(masks_e_sb[:], masks_e[:])
            MAIN_DELTAS = (0, 128, 256, 384)
            PT = {}
            for di, dd in enumerate(MAIN_DELTAS):
                t = pC.tile([128, 2, 512], BF16, name=f"PTl{di}")
                nc.gpsimd.memset(t[:], 0.0)
                PT[dd] = t
            for de_i, de in enumerate(EDGE_DELTAS):
                PT[de] = pC.tile([128, 2, 32], BF16, name=f"PTe{de_i}")
            for qb in range(2):
                q0 = Q0 + qb * 512
                for hp in range(4):
                    for di, dd in enumerate(MAIN_DELTAS):
                        qq0, qq1 = STRIPE[dd]
                        rel = q0 + dd - KL0
                        sc2 = ps2.tile([128, 2, 512], F32,
                                       name=f"psC{qb}{hp}{di}", tag="ps2")
                        for ab in range(2):
                            r0 = ab * 64
                            nc.tensor.matmul(
                                sc2[:, ab, qq0:qq1],
                                kT_l[hp].bitcast(BF16)[r0:r0 + 64, rel:rel + 128],
                                qT_l[hp].bitcast(BF16)
                                [r0:r0 + 64, qb * 512 + qq0: qb * 512 + qq1],
                                start=True, stop=True, tile_position=(r0, 0))
                        pt_t = PT[dd]
                        nc.scalar.activation(
                            pt_t[:, :, qq0:qq1], sc2[:, :, qq0:qq1],
                            AF.Exp, scale=SCALE)
                        nc.vector.tensor_tensor(
                            pt_t[:, :, qq0:qq1], pt_t[:, :, qq0:qq1],
                            masks_m_sb[:, di, qq0:qq1].unsqueeze(1)
                            .to_broadcast((128, 2, qq1 - qq0)), op=ALU.mult)
                    for de_i, de in enumerate(EDGE_DELTAS):
                        qq0, qq1 = STRIPE[de]
                        rel = q0 + de - KL0
                        sc2 = ps2.tile([128, 2, 512], F32,
                                       name=f"psCe{qb}{hp}{de_i}", tag="ps2")
                        for ab in range(2):
                            r0 = ab * 64
                            nc.tensor.matmul(
                                sc2[:, ab, 0:32],
                                kT_l[hp].bitcast(BF16)[r0:r0 + 64, rel:rel + 128],
                                qT_l[hp].bitcast(BF16)
                                [r0:r0 + 64, qb * 512 + qq0: qb * 512 + qq1],
                                start=True, stop=True, tile_position=(r0, 0))
                        pt_t = PT[de]
                        nc.scalar.activation(
                            pt_t[:], sc2[:, :, 0:32], AF.Exp, scale=SCALE)
                        nc.vector.tensor_tensor(
                            pt_t[:], pt_t[:],
                            masks_e_sb[:, de_i, qb, :].unsqueeze(1)
                            .to_broadcast((128, 2, 32)), op=ALU.mult)
                    for ab in range(2):
                        head = 2 * hp + ab
                        po_t = pso.tile([65, 512], F32, name=f"psoC{qb}{hp}{ab}",
                                        tag=f"pso{ab}", bufs=1)
                        nc.tensor.matmul(
                            po_t[:], V_l[(q0 - KL0) // 128].bitcast(BF16)[:, head, :],
                            PT[0][:, ab, :], start=True, stop=False,
                            skip_group_check=True)
                        for de in EDGE_DELTAS:
                            qq0, qq1 = STRIPE[de]
                            nc.tensor.matmul(
                                po_t[:, qq0:qq1],
                                V_l[(q0 + de - KL0) // 128].bitcast(BF16)[:, head, :],
                                PT[de][:, ab, :],
                                start=False, stop=False, skip_group_check=True)
                        for dd in (128, 256, 384):
                            nc.tensor.matmul(
                                po_t[:],
                                V_l[(q0 + dd - KL0) // 128].bitcast(BF16)[:, head, :],
                                PT[dd][:, ab, :], start=False, stop=(dd == 384),
                                skip_group_check=True)
                        normalize(po_t, oT_l[hp], ab * 64, qb * 512,
                                  f"nC{qb}{hp}{ab}")
        if debug:
            nc.sync.dma_start(dbg["d_oTl"][:], oT_l[0].bitcast(BF16)[:])

        wo_sb = wp.tile([128, 2, 4, D], BF16, name="wo_sb", tag="wo2nd")
        nc.scalar.dma_start(wo_sb[:], wo2.rearrange("w (t p) d -> p w t d", p=128))
        localT = [s4.tile([128, NQ], BF16, name=f"localT{m}", tag="s4b", bufs=4)
                  for m in range(4)]
        out_proj(oT_l, localT, wo_sb, 0, "psFl")

        # ============ Phase D: global qkv ===================================
        qT_g = [qTp.tile([128, NQ], BF16, name=f"qTg{m}", tag="qT")
                for m in range(4)]
        kT_g = [kTp.tile([128, S], BF16, name=f"kTg{m}", tag="kT")
                for m in range(4)]
        V_g = [Vp.tile([128, 8, 65], BF16, name=f"Vg{pt}", tag="V")
               for pt in range(16)]
        wqkv_g_sb = wp.tile([128, 3, 2, 2, D], FP8, name="wqkv_g_sb", tag="wbig")
        nc.scalar.dma_start(wqkv_g_sb[:], wqkv8_g[:])
        project_qkv(wqkv_g_sb, bqkv_g_sb, bv_g_sb, qT_g, kT_g, V_g, 0, S, 0, "Dg", kv_on_act=GLOBAL_KV_ON_ACT)

        # ============ Phase E: global attention + out-proj ==================
        oT_g = [s4.tile([128, NQ], BF16, name=f"oTg{m}", tag="s4c", bufs=8)
                for m in range(4)]
        for qb in range(2):
            for hp in range(4):
                po_ts = [pso.tile([65, 512], F32, name=f"psoE{qb}{hp}{ab}",
                                  tag=f"pso{ab}", bufs=1) for ab in range(2)]
                for kt in range(16):
                    sc2 = ps2.tile([128, 2, 512], F32,
                                   name=f"psE{qb}{hp}{kt}", tag="ps2")
                    for ab in range(2):
                        r0 = ab * 64
                        nc.tensor.matmul(
                            sc2[:, ab, :], kT_g[hp].bitcast(BF16)
                            [r0:r0 + 64, kt * 128:(kt + 1) * 128],
                            qT_g[hp].bitcast(BF16)
                            [r0:r0 + 64, qb * 512:(qb + 1) * 512],
                            start=True, stop=True, tile_position=(r0, 0))
                    ptg = ptgp.tile([128, 2, 512], BF16,
                                    name=f"ptg{qb}{hp}{kt}", tag="ptg")
                    nc.scalar.activation(ptg[:], sc2[:], AF.Exp, scale=SCALE)
                    for ab in range(2):
                        nc.tensor.matmul(
                            po_ts[ab][:],
                            V_g[kt].bitcast(BF16)[:, 2 * hp + ab, :],
                            ptg[:, ab, :], start=(kt == 0), stop=(kt == 15),
                            skip_group_check=True)
                for ab in range(2):
                    normalize(po_ts[ab], oT_g[hp], ab * 64, qb * 512,
                              f"nE{qb}{hp}{ab}")
        if debug:
            nc.sync.dma_start(dbg["d_oTg"][:], oT_g[0].bitcast(BF16)[:])

        globalT = [s4.tile([128, NQ], BF16, name=f"globalT{m}", tag="s4c", bufs=8)
                   for m in range(4)]
        out_proj(oT_g, globalT, wo_sb, 1, "psFg")

        # ============ Phase G: gate + fuse ==================================
        fusedT = [s4.tile([128, NQ], BF16, name=f"fusedT{m}", tag="s4a", bufs=4)
                  for m in range(4)]
        gate_w_sb = wp.tile([128, 8, D], BF16, name="gate_w_sb", tag="wbig")
        nc.scalar.dma_start(gate_w_sb[:],
                          gate_w.rearrange("(t p) d -> p t d", p=128))
        cat = localT + globalT
        for m in range(4):
            for n in range(2):
                acc = ps.tile([128, 512], F32, name=f"psG{m}{n}", tag="ps")
                for kt in range(8):
                    nc.tensor.matmul(
                        acc[:], gate_w_sb[:, kt, m * 128:(m + 1) * 128],
                        cat[kt].bitcast(BF16)[:, n * 512:(n + 1) * 512],
                        start=(kt == 0), stop=(kt == 7))
                gt = lnp.tile([128, 512], BF16, name=f"gt{m}{n}", tag="gt", bufs=1)
                if use_gate_b:
                    nc.vector.tensor_scalar(
                        gt[:], acc[:], gate_b_sb[:, m:m + 1], 0.0,
                        op0=ALU.add, op1=ALU.max)
                else:
                    nc.vector.tensor_scalar(gt[:], acc[:], 0.0, None,
                                            op0=ALU.max)
                nc.scalar.activation(gt[:], gt[:], AF.Tanh)
                if debug and m == 0 and n == 0:
                    nc.sync.dma_start(dbg["d_gateT"][:], gt[:])
                # fused = global + gate*(local - global)
                lsl = localT[m].bitcast(BF16)[:, n * 512:(n + 1) * 512]
                gsl = globalT[m].bitcast(BF16)[:, n * 512:(n + 1) * 512]
                tmp = lnp.tile([128, 512], BF16, name=f"tmpG{m}{n}", tag="tmpG", bufs=1)
                nc.gpsimd.tensor_tensor(tmp[:], lsl, gsl, op=ALU.subtract)
                nc.vector.tensor_tensor(tmp[:], tmp[:], gt[:], op=ALU.mult)
                nc.vector.tensor_tensor(
                    fusedT[m].bitcast(BF16)[:, n * 512:(n + 1) * 512],
                    tmp[:], gsl, op=ALU.add)
        if debug:
            nc.sync.dma_start(dbg["d_fusedT"][:], fusedT[0].bitcast(BF16)[:])

        # ===== layernorm helper (token-major [128, D]) ======================
        def layernorm(dst, src_ap, g_sb, b_sb, pfx):
            stats = lnp.tile([128, 6], F32, name=f"{pfx}st", tag="lnst")
            nc.vector.bn_stats(stats[:], src_ap)
            mv = lnp.tile([128, 2], F32, name=f"{pfx}mv", tag="lnmv")
            nc.vector.bn_aggr(mv[:], stats[:])
            std = lnp.tile([128, 1], F32, name=f"{pfx}sd", tag="lnsd")
            nc.scalar.activation(std[:], mv[:, 1:2], AF.Sqrt, bias=eps_sb[:])
            rstd = lnp.tile([128, 1], F32, name=f"{pfx}rs", tag="lnrs")
            nc.vector.reciprocal(rstd[:], std[:])
            if g_sb is not None:
                tmp = lnp.tile([128, D], F32, name=f"{pfx}tmp", tag="lntmp")
                nc.vector.tensor_scalar(
                    tmp[:], src_ap, mv[:, 0:1], rstd[:],
                    op0=ALU.subtract, op1=ALU.mult)
                if b_sb is not None:
                    nc.vector.tensor_tensor(dst, tmp[:], g_sb[:], op=ALU.mult)
                    nc.vector.tensor_tensor(dst, dst, b_sb[:], op=ALU.add)
                else:
                    nc.vector.tensor_tensor(dst, tmp[:], g_sb[:], op=ALU.mult)
            else:
                nc.vector.tensor_scalar(
                    dst, src_ap, mv[:, 0:1], rstd[:],
                    op0=ALU.subtract, op1=ALU.mult)
                if b_sb is not None:
                    nc.vector.tensor_tensor(dst, dst, b_sb[:], op=ALU.add)

        # ============ Phase H: x1 = h + fused^T; y1 = LN1 ===================
        y1 = [s2.tile([128, D], F32R, name=f"y1_{t}", tag="s2") for t in range(8)]
        for t in range(8):
            x1 = lnp.tile([128, D], F32, name=f"x1_{t}", tag="x1")
            for m in range(4):
                ptr = ps.tile([128, 128], BF16, name=f"ptrH{t}{m}", tag="ps")
                nc.tensor.transpose(
                    ptr[:], fusedT[m].bitcast(BF16)[:, t * 128:(t + 1) * 128],
                    eyeb_sb[:])
                nc.vector.tensor_tensor(
                    x1[:, m * 128:(m + 1) * 128],
                    f32(h_sb[t][:, m * 128:(m + 1) * 128]), ptr[:], op=ALU.add)
            layernorm(y1[t][:], x1[:], n1gb_sb, n1bb_sb, f"ln1_{t}")
        if debug:
            nc.sync.dma_start(dbg["d_y1"][:], f32(y1[0][:]))

        # ============ Phase I: y1T ==========================================
        y1T = [s4.tile([128, NQ], BF16, name=f"y1T{m}", tag="s4b", bufs=4)
               for m in range(4)]
        for t in range(8):
            for m in range(4):
                ptr = ps.tile([128, 128], F32, name=f"ptrI{t}{m}", tag="ps")
                nc.tensor.transpose(ptr[:], f32(y1[t][:, m * 128:(m + 1) * 128]),
                                    eye_sb[:])
                nc.scalar.copy(
                    y1T[m].bitcast(BF16)[:, t * 128:(t + 1) * 128], ptr[:])

        # ============ Phase J: FFN + LN2 + LN3; Phase K: pool + out =========
        w1_sb = wp.tile([128, 4, DFF], BF16, name="w1_sb", tag="wbig")
        nc.scalar.dma_start(w1_sb[:], w1.rearrange("(t p) d -> p t d", p=128))
        w2_sb = wp.tile([128, 8, D], BF16, name="w2_sb", tag="wo2nd")
        nc.scalar.dma_start(w2_sb[:], w2.rearrange("(t p) d -> p t d", p=128))
        z1T = [s4.tile([128, NQ], BF16, name=f"z1T{m}", tag="s4c", bufs=8)
               for m in range(8)]
        for m in range(8):
            for n in range(2):
                acc = ps.tile([128, 512], F32, name=f"psJ1{m}{n}", tag="ps")
                for kt in range(4):
                    nc.tensor.matmul(
                        acc[:], w1_sb[:, kt, m * 128:(m + 1) * 128],
                        y1T[kt].bitcast(BF16)[:, n * 512:(n + 1) * 512],
                        start=(kt == 0), stop=(kt == 3))
                dst = z1T[m].bitcast(BF16)[:, n * 512:(n + 1) * 512]
                if use_b1:
                    nc.vector.tensor_scalar(
                        dst, acc[:], b1_sb[:, m:m + 1], 0.0,
                        op0=ALU.add, op1=ALU.max)
                else:
                    nc.vector.tensor_scalar(dst, acc[:], 0.0, None, op0=ALU.max)

        y3 = [s2.tile([128, D], F32R, name=f"y3_{t}", tag="s2") for t in range(8)]
        accp = pso.tile([1, 512], F32, name="pspool", tag="pso0", bufs=1)
        for t in range(8):
            acc = ps.tile([128, 512], F32, name=f"psJ2{t}", tag="ps")
            for kt in range(8):
                nc.tensor.matmul(
                    acc[:], z1T[kt].bitcast(BF16)[:, t * 128:(t + 1) * 128],
                    w2_sb[:, kt, :], start=(kt == 0), stop=(kt == 7))
            x2 = lnp.tile([128, D], F32, name=f"x2_{t}", tag="x2")
            nc.vector.tensor_tensor(x2[:], acc[:], f32(y1[t][:]), op=ALU.add)
            if use_b2:
                nc.vector.tensor_tensor(x2[:], x2[:], b2b_sb[:], op=ALU.add)
            if not (use_n2g or use_n2b or use_n3g):
                # LN3(LN2(x)) with unit gamma / zero beta collapses to one LN:
                # mean(LN2 out) == 0 exactly, var(LN2 out) = v/(v+eps), so
                # y3 = (x - m) / sqrt(v*(1+eps) + eps^2)
                pfx = f"ln23_{t}"
                stats = lnp.tile([128, 6], F32, name=f"{pfx}st", tag="lnst")
                nc.vector.bn_stats(stats[:], x2[:])
                mv = lnp.tile([128, 2], F32, name=f"{pfx}mv", tag="lnmv")
                nc.vector.bn_aggr(mv[:], stats[:])
                std = lnp.tile([128, 1], F32, name=f"{pfx}sd", tag="lnsd")
                nc.scalar.activation(std[:], mv[:, 1:2], AF.Sqrt,
                                     bias=eps2_sb[:], scale=1.0 + EPS)
                rstd = lnp.tile([128, 1], F32, name=f"{pfx}rs", tag="lnrs")
                nc.vector.reciprocal(rstd[:], std[:])
                nc.vector.tensor_scalar(
                    y3[t][:], x2[:], mv[:, 0:1], rstd[:],
                    op0=ALU.subtract, op1=ALU.mult)
            else:
                y2 = lnp.tile([128, D], F32, name=f"y2_{t}", tag="y2")
                layernorm(y2[:], x2[:], n2gb_sb, n2bb_sb, f"ln2_{t}")
                layernorm(y3[t][:], y2[:], n3gb_sb, None, f"ln3_{t}")
            nc.tensor.matmul(accp[:], poolw_sb[:], y3[t][:],
                             start=(t == 0), stop=(t == 7),
                             skip_group_check=True)
        if debug:
            nc.sync.dma_start(dbg["d_y3"][:], f32(y3[0][:]))

        pooled_sb = pers.tile([1, D], F32, name="pooled_sb")
        nc.vector.tensor_copy(pooled_sb[:], accp[:])
        if debug:
            nc.sync.dma_start(dbg["d_pooled"][:], f32(pooled_sb[:]))
        nc.sync.dma_start(po[:], pooled_sb[:])

    nc.compile()
    return nc


def _prep_inputs(inputs):
    """Host-side prep: returns (flags, in_maps for 8 cores, host_const)."""
    g = {k: np.asarray(v, dtype=np.float32) for k, v in inputs.items()}
    x, pos = g["x"], g["pos"]
    win_w, win_b = g["win_w"], g["win_b"]

    flags = (
        bool(np.any(g["l_bqkv"] != 0)), bool(np.any(g["g_bqkv"] != 0)),
        bool(np.any(g["l_bo"] != 0) or np.any(g["g_bo"] != 0)),
        bool(np.any(g["gate_b"] != 0)), bool(np.any(g["ffn_b1"] != 0)),
        bool(np.any(g["ffn_b2"] != 0)),
        bool(np.any(g["n1_g"] != 1)), bool(np.any(g["n1_b"] != 0)),
        bool(np.any(g["n2_g"] != 1)), bool(np.any(g["n2_b"] != 0)),
        bool(np.any(g["n3_g"] != 1)),
    )
    (use_bqkv_l, use_bqkv_g, use_bo, use_gate_b, use_b1, use_b2,
     use_n1g, use_n1b, use_n2g, use_n2b, use_n3g) = flags

    posT = pos[0].T + win_b[:, None]                      # [D, S]

    def fold8(w3):
        # [3, D, D] -> [128, 3, 2pair, 2j, D]: w[qkv, pair*256 + j*128 + p, :]
        return np.ascontiguousarray(
            (w3.reshape(3, 2, 2, 128, D) * WS).transpose(3, 0, 1, 2, 4)
        ).astype(NPF8)

    common = {
        "win": np.ascontiguousarray(win_w),
        "wqkv8_l": fold8(g["l_wqkv"]),
        "wqkv8_g": fold8(g["g_wqkv"]),
        "wo2": np.stack([g["l_wo"], g["g_wo"]]).astype(ml_dtypes.bfloat16),
        "gate_w": g["gate_w"].astype(ml_dtypes.bfloat16),
        "w1": g["ffn_w1"].astype(ml_dtypes.bfloat16),
        "w2": g["ffn_w2"].astype(ml_dtypes.bfloat16),
        "eye": np.eye(128, dtype=np.float32),
        "poolw": np.full((128, 1), 1.0 / S, dtype=np.float32),
    }
    perm = lambda b: b.reshape(-1, 4, 128).transpose(2, 0, 1).copy()
    if use_bqkv_l:
        common["bqkv_l"] = perm(g["l_bqkv"])
        common["bv_l"] = np.tile(g["l_bqkv"][2], (128, 1))
    if use_bqkv_g:
        common["bqkv_g"] = perm(g["g_bqkv"])
        common["bv_g"] = np.tile(g["g_bqkv"][2], (128, 1))
    if use_bo:
        common["bo2"] = perm(np.stack([g["l_bo"], g["g_bo"]]))
    if use_gate_b:
        common["gate_b"] = g["gate_b"].reshape(4, 128).T.copy()
    if use_b1:
        common["b1"] = g["ffn_b1"].reshape(8, 128).T.copy()
    if use_b2:
        common["b2b"] = np.tile(g["ffn_b2"], (128, 1))
    if use_n1g:
        common["n1gb"] = np.tile(g["n1_g"], (128, 1))
    if use_n1b:
        common["n1bb"] = np.tile(g["n1_b"], (128, 1))
    if use_n2g:
        common["n2gb"] = np.tile(g["n2_g"], (128, 1))
    if use_n2b:
        common["n2bb"] = np.tile(g["n2_b"], (128, 1))
    if use_n3g:
        common["n3gb"] = np.tile(g["n3_g"], (128, 1))

    # universal interior band masks (pure Toeplitz, no seam crossing)
    kk = np.arange(128)
    qq = np.arange(512)
    mk_m = np.zeros((128, 4, 512), dtype=np.float32)
    for di, d in enumerate((0, 128, 256, 384)):
        mk_m[:, di, :] = (np.abs(kk[:, None] + d - qq[None, :]) <= W // 2)
    mk_m = mk_m.astype(ml_dtypes.bfloat16)

    hf_data = []
    for hf in range(2):
        q0c = NQ * hf
        shift = Q0 - q0c
        posb_rot = np.ascontiguousarray(np.roll(posT, shift, axis=1))
        mk_e = np.zeros((128, 2, 2, 32), dtype=np.float32)
        for qb in range(2):
            q0 = Q0 + qb * 512
            for de_i, d in enumerate(EDGE_DELTAS):
                qq0, qq1 = STRIPE[d]
                k_rot = q0 + d + kk[:, None]
                q_rot = q0 + np.arange(qq0, qq1)[None, :]
                orig_k = (k_rot - shift) % S
                orig_q = (q_rot - shift) % S
                mk_e[:, de_i, qb, :] = (np.abs(orig_k - orig_q) <= W // 2)
        hf_data.append((posb_rot, mk_e.astype(ml_dtypes.bfloat16)))

    in_maps = []
    for core in range(N_CORES):
        b, hf = core // 2, core % 2
        shift = Q0 - NQ * hf
        posb_rot, mk_e = hf_data[hf]
        m = dict(common)
        m["xT"] = np.ascontiguousarray(np.roll(x[b].T, shift, axis=1))
        m["posb"] = posb_rot
        m["masks_m"] = mk_m
        m["masks_e"] = mk_e
        in_maps.append(m)

    host_const = (g["n3_b"] @ g["out_w"] + g["out_b"],
                  np.ascontiguousarray(g["out_w"]))
    return flags, in_maps, host_const


def kernel(**inputs):
    flags, in_maps, host_const = _prep_inputs(inputs)
    const_vec, out_w = host_const
    if flags not in _CACHE:
        _CACHE[flags] = _build(flags)
    nc = _CACHE[flags]
    res = run_bass_kernel_spmd(nc, in_maps, core_ids=list(range(N_CORES)))
    out = np.zeros((B, DOUT), dtype=np.float32)
    for b in range(B):
        pooled = res.results[2 * b]["po"][0] + res.results[2 * b + 1]["po"][0]
        out[b] = pooled @ out_w + const_vec
    return out



# revision 32
# speedup vs baseline: 1.1125x; 1.0309x over previous
"""DualPathTransformer Trainium2 kernel.

Sharding: 8 cores = batch(4) x query-half(2). Each core processes one batch
and 1024 query tokens; K/V work is duplicated within a batch pair. No
device collectives: partial pooled projections are summed on the host.

SPMD uniformity trick: each core receives its batch token-ROTATED so that
its query tokens sit at rotated positions [512, 1536). Global attention is
permutation-invariant over keys; the local band structure is encoded in
host-prepped per-core mask tiles in true original coordinates. The program
is identical on all cores; only input data differs.

Layouts: activations feature-major (hT = [feature partitions, tokens]) for
matmuls; token-major (tokens on partitions) for layernorm stages. Scores
are computed transposed (keys on partitions) so softmax denominators come
free from a ones-row appended to V, and the AV matmul needs no transposes.

Precision: residual stream and weights fp32/f32r; attention q/k/v/probs and
post-attention projections bf16 (error contribution ~1e-3 of the stream).
"""

import numpy as np
import ml_dtypes
from contextlib import ExitStack

import concourse.bass as bass
import concourse.bacc as bacc
import concourse.tile as tile
import concourse.mybir as mybir
from concourse.bass_utils import run_bass_kernel_spmd

F32R = mybir.dt.float32r
F32 = mybir.dt.float32
BF16 = mybir.dt.bfloat16
FP8 = mybir.dt.float8e4
NPF8 = ml_dtypes.float8_e4m3
AF = mybir.ActivationFunctionType
ALU = mybir.AluOpType
DRM = mybir.MatmulPerfMode.DoubleRow
WS = 64.0          # fp8 weight pre-scale (host); compensated at psum drain
WSI = 1.0 / WS
OS = 16.0          # fp8 scale on oT / catf (lift tiny values out of subnormals)
# oT2 = OS*o via ones-row = 1/OS; catf = OS*out_proj via drain scale
# CS_DRAIN: psum(out_proj) = OS*o @ WS*wo -> *OS/(OS*WS) = WSI keeps catf at OS x
GATE_SCALE = 1.0 / (OS * WS)   # gate psum = OS*cat @ WS*gw
FUSE_INV = 1.0 / OS            # fusedT carries OS x; divided out at x1 add

B, S, DIN, D, H, DOUT, W = 4, 2048, 256, 512, 8, 128, 64
HD = D // H          # 64
DFF = 2 * D          # 1024
NQ = S // 2          # 1024 queries per core
N_CORES = 8
Q0 = 512             # rotated position of first query token (uniform)
KL0, KL1 = 384, 1664   # local K/V window in rotated coords (10 ptiles)
NKL = KL1 - KL0        # 1280
DELTAS = (-128, 0, 128, 256, 384, 512)   # local kblock offsets rel. to qblock
# stripe (bounding qq range) per delta, qblock-relative
STRIPE = {-128: (0, 32), 0: (0, 160), 128: (96, 288),
          256: (224, 416), 384: (352, 512), 512: (480, 512)}
EDGE_DELTAS = (-128, 512)          # AV mms sliced to the stripe
SCALE = 1.0 / float(np.sqrt(HD))
EPS = 1e-5

_CACHE = {}
GLOBAL_KV_ON_ACT = False
LOCAL_KV_ON_ACT = True


def _build(flags, debug=False):
    (use_bqkv_l, use_bqkv_g, use_bo, use_gate_b, use_b1, use_b2,
     use_n1g, use_n1b, use_n2g, use_n2b, use_n3g) = flags

    nc = bacc.Bacc("TRN2", target_bir_lowering=False, debug=False)

    def din(name, shape, dt=F32R):
        return nc.dram_tensor(name, list(shape), dt, kind="ExternalInput").ap()

    xT = din("xT", [DIN, S])
    posb = din("posb", [D, S])
    win = din("win", [DIN, D])
    wqkv8_l = din("wqkv8_l", [128, 3, 2, 2, D], FP8)
    wqkv8_g = din("wqkv8_g", [128, 3, 2, 2, D], FP8)
    wo8 = din("wo8", [128, 2, 2, 2, D], FP8)   # [p, li, pair, j, fout]
    gw8 = din("gw8", [128, 4, 2, D], FP8)      # [p, pair, j, fout]
    w18 = din("w18", [128, 2, 2, DFF], FP8)
    w28 = din("w28", [128, 2, 4, 2, D], FP8)   # [p, hi/lo, pair, j, fout]
    masks_m = din("masks_m", [128, 4, 512], BF16)   # [kk, di, qq]
    masks_e = din("masks_e", [128, 2, 2, 32], BF16)  # [kk, de, qb, qq32]
    eye = din("eye", [128, 128], F32)
    poolw = din("poolw", [128, 1])
    if use_bqkv_l:
        bqkv_l = din("bqkv_l", [128, 3, 4], F32)
        bv_l = din("bv_l", [128, D], F32)
    if use_bqkv_g:
        bqkv_g = din("bqkv_g", [128, 3, 4], F32)
        bv_g = din("bv_g", [128, D], F32)
    if use_bo:
        bo2 = din("bo2", [128, 2, 4], F32)
    if use_gate_b:
        gate_b = din("gate_b", [128, 4], F32)
    if use_b1:
        b1 = din("b1", [128, 8], F32)
    if use_b2:
        b2b = din("b2b", [128, D], F32)
    if use_n1g:
        n1gb = din("n1gb", [128, D], F32)
    if use_n1b:
        n1bb = din("n1bb", [128, D], F32)
    if use_n2g:
        n2gb = din("n2gb", [128, D], F32)
    if use_n2b:
        n2bb = din("n2bb", [128, D], F32)
    if use_n3g:
        n3gb = din("n3gb", [128, D], F32)
    # n3_b handled on host (pooled mean is linear in it)

    po = nc.dram_tensor("po", [1, D], F32, kind="ExternalOutput").ap()

    dbg = {}
    if debug:
        for nm, shp, dt_ in [("d_hT", [128, S], F32), ("d_oTl", [128, NQ], BF16),
                             ("d_oTg", [128, NQ], BF16), ("d_gateT", [128, 512], BF16),
                             ("d_fusedT", [128, NQ], BF16), ("d_y1", [128, D], F32),
                             ("d_y3", [128, D], F32), ("d_pooled", [1, D], F32)]:
            dbg[nm] = nc.dram_tensor(nm, shp, dt_, kind="ExternalOutput").ap()

    f32 = lambda ap: ap.bitcast(F32)

    with tile.TileContext(nc) as tc, ExitStack() as top:
        # ---- psum pools (8 banks) ----
        ps = top.enter_context(tc.tile_pool(name="ps", bufs=2, space="PSUM"))
        ps2 = top.enter_context(tc.tile_pool(name="ps2", bufs=2, space="PSUM"))
        pso = top.enter_context(tc.tile_pool(name="pso", bufs=1, space="PSUM"))

        # ---- persistent pools (static tags, round-robin slot reuse) ----
        pers = top.enter_context(tc.tile_pool(name="pers", bufs=1))
        lnp = top.enter_context(tc.tile_pool(name="lnp", bufs=2))
        wp = top.enter_context(tc.tile_pool(name="wp", bufs=1))
        s4 = top.enter_context(tc.tile_pool(name="s4", bufs=1))     # [128,1024] bf16 tags
        s2 = top.enter_context(tc.tile_pool(name="s2", bufs=11))    # [128,512] f32
        qTp = top.enter_context(tc.tile_pool(name="qTp", bufs=4))   # [128,1024] bf16
        kTp = top.enter_context(tc.tile_pool(name="kTp", bufs=4))   # [128,2048] bf16
        hTp = top.enter_context(tc.tile_pool(name="hTp", bufs=1))
        Vp = top.enter_context(tc.tile_pool(name="Vp", bufs=16))    # [128,8,65] bf16
        ptgp = top.enter_context(tc.tile_pool(name="ptgp", bufs=3)) # pair bf16

        eye_sb = pers.tile([128, 128], F32, name="eye_sb")
        nc.sync.dma_start(eye_sb[:], eye[:])
        eyeb_sb = pers.tile([128, 128], BF16, name="eyeb_sb")
        nc.vector.tensor_copy(eyeb_sb[:], eye_sb[:])
        poolw_sb = pers.tile([128, 1], F32R, name="poolw_sb")
        nc.sync.dma_start(poolw_sb[:], poolw[:])
        eps_sb = pers.tile([128, 1], F32, name="eps_sb")
        nc.vector.memset(eps_sb[:], EPS)
        eps2_sb = pers.tile([128, 1], F32, name="eps2_sb")
        nc.vector.memset(eps2_sb[:], EPS * EPS)

        def load_bias(ap_dram, shape, name):
            t = pers.tile(shape, F32, name=name)
            nc.sync.dma_start(t[:], ap_dram[:])
            return t
        bqkv_l_sb = load_bias(bqkv_l, [128, 3, 4], "bqkv_l_sb") if use_bqkv_l else None
        bv_l_sb = load_bias(bv_l, [128, D], "bv_l_sb") if use_bqkv_l else None
        bqkv_g_sb = load_bias(bqkv_g, [128, 3, 4], "bqkv_g_sb") if use_bqkv_g else None
        bv_g_sb = load_bias(bv_g, [128, D], "bv_g_sb") if use_bqkv_g else None
        bo2_sb = load_bias(bo2, [128, 2, 4], "bo2_sb") if use_bo else None
        gate_b_sb = load_bias(gate_b, [128, 4], "gate_b_sb") if use_gate_b else None
        b1_sb = load_bias(b1, [128, 8], "b1_sb") if use_b1 else None
        b2b_sb = load_bias(b2b, [128, D], "b2b_sb") if use_b2 else None
        n1gb_sb = load_bias(n1gb, [128, D], "n1gb_sb") if use_n1g else None
        n1bb_sb = load_bias(n1bb, [128, D], "n1bb_sb") if use_n1b else None
        n2gb_sb = load_bias(n2gb, [128, D], "n2gb_sb") if use_n2g else None
        n2bb_sb = load_bias(n2bb, [128, D], "n2bb_sb") if use_n2b else None
        n3gb_sb = load_bias(n3gb, [128, D], "n3gb_sb") if use_n3g else None

        # long-lived stream tiles
        hT = [hTp.tile([128, S], F32R, name=f"hT{m}", tag="hT", bufs=4)
              for m in range(4)]
        h_sb = [s2.tile([128, D], F32R, name=f"h{t}", tag="s2") for t in range(8)]

        # ============ Phase A: hT + h ======================================
        # posb lands directly in hT via DMA; matmul results accumulate into it
        for m in range(4):
            nc.sync.dma_start(
                hT[m][:], posb.rearrange("(t p) n -> p t n", p=128)[:, m, :])
        with ExitStack() as sA:
            pA = sA.enter_context(tc.tile_pool(name="pA", bufs=2))
            win_sb = pA.tile([128, 2, D], F32R, name="win_sb", tag="win", bufs=1)
            nc.sync.dma_start(win_sb[:], win.rearrange("(t p) n -> p t n", p=128))
            for c in range(2):
                xTc = pA.tile([128, 2, 1024], F32R, name=f"xTc{c}", tag="xTc")
                nc.sync.dma_start(
                    xTc[:], xT.rearrange("(t p) n -> p t n", p=128)
                    [:, :, c * 1024:(c + 1) * 1024])
                for m in range(4):
                    for hh in range(2):
                        acc = ps.tile([128, 512], F32, name=f"psA{m}{c}{hh}",
                                      tag="ps")
                        for kt in range(2):
                            nc.tensor.matmul(
                                acc[:], win_sb[:, kt, m * 128:(m + 1) * 128],
                                xTc[:, kt, hh * 512:(hh + 1) * 512],
                                start=(kt == 0), stop=(kt == 1))
                        sl = hT[m][:, c * 1024 + hh * 512:
                                   c * 1024 + (hh + 1) * 512]
                        nc.vector.tensor_tensor(sl, acc[:], sl, op=ALU.add)
        # token-major h for core's tokens (rotated [512, 1536))
        for t in range(8):
            for m in range(4):
                ptr = ps.tile([128, 128], F32, name=f"ptrA{t}{m}", tag="ps")
                nc.tensor.transpose(
                    ptr[:], f32(hT[m][:, Q0 + t * 128: Q0 + (t + 1) * 128]),
                    eye_sb[:])
                nc.vector.tensor_copy(
                    h_sb[t][:, m * 128:(m + 1) * 128], ptr[:])
        # fp8 contraction-folded copy of hT for DoubleRow projections:
        # hT2[pp][p, j, n] = h[pp*256 + j*128 + p, n]
        hT2 = [hTp.tile([128, 2, S], FP8, name=f"hT2_{pp}", tag="hT2", bufs=2)
               for pp in range(2)]
        for pp in range(2):
            for j in range(2):
                nc.scalar.copy(hT2[pp][:, j, :], f32(hT[2 * pp + j][:]))
        if debug:
            nc.sync.dma_start(dbg["d_hT"][:], f32(hT[0][:]))

        # ============ helper: qkv projection (fp8 DoubleRow) ================
        def project_qkv(w8_sb, bias_sb, bv_sb, q_tiles, kT_tiles, v_tiles,
                        kT_lo, kT_hi, v_pt_lo, pfx, kv_on_act=True):
            for m in range(4):
                for n in range(2):
                    acc = ps.tile([128, 512], F32, name=f"{pfx}q{m}{n}", tag="ps")
                    for pp in range(2):
                        nc.tensor.matmul(
                            acc[:], w8_sb[:, 0, pp, :, m * 128:(m + 1) * 128],
                            hT2[pp][:, :, Q0 + n * 512: Q0 + (n + 1) * 512],
                            start=(pp == 0), stop=(pp == 1), perf_mode=DRM)
                    dst = q_tiles[m].bitcast(BF16)[:, n * 512:(n + 1) * 512]
                    if bias_sb is not None:
                        nc.vector.tensor_scalar(
                            dst, acc[:], WSI, bias_sb[:, 0, m:m + 1],
                            op0=ALU.mult, op1=ALU.add)
                    else:
                        nc.vector.tensor_scalar(
                            dst, acc[:], WSI, None, op0=ALU.mult)
            nk = kT_hi - kT_lo
            for m in range(4):
                for off in range(0, nk, 512):
                    w_ = min(512, nk - off)
                    acc = ps.tile([128, 512], F32, name=f"{pfx}k{m}{off}",
                                  tag="ps")
                    for pp in range(2):
                        nc.tensor.matmul(
                            acc[:, 0:w_], w8_sb[:, 1, pp, :, m * 128:(m + 1) * 128],
                            hT2[pp][:, :, kT_lo + off: kT_lo + off + w_],
                            start=(pp == 0), stop=(pp == 1), perf_mode=DRM)
                    dst = kT_tiles[m].bitcast(BF16)[:, off:off + w_]
                    bias_ap = bias_sb[:, 1, m:m + 1] if bias_sb is not None else 0.0
                    if kv_on_act:
                        nc.scalar.activation(dst, acc[:, 0:w_], AF.Identity,
                                             bias=bias_ap, scale=WSI)
                    else:
                        nc.vector.tensor_scalar(
                            dst, acc[:, 0:w_], WSI,
                            None if bias_sb is None else bias_ap,
                            op0=ALU.mult,
                            **({} if bias_sb is None else dict(op1=ALU.add)))
            for i, vt in enumerate(v_tiles):
                pt = v_pt_lo + i
                acc = ps.tile([128, 512], F32, name=f"{pfx}v{pt}", tag="ps")
                for pp in range(2):
                    nc.tensor.matmul(
                        acc[:], hT2[pp][:, :, pt * 128:(pt + 1) * 128],
                        w8_sb[:, 2, pp, :, :],
                        start=(pp == 0), stop=(pp == 1), perf_mode=DRM)
                dst3 = vt.bitcast(BF16)[:, :, 0:64]
                src3 = acc[:].rearrange("p (h e) -> p h e", h=8)
                if bv_sb is not None:
                    nc.vector.scalar_tensor_tensor(
                        dst3, src3, WSI,
                        f32(bv_sb[:]).rearrange("p (h e) -> p h e", h=8),
                        op0=ALU.mult, op1=ALU.add)
                elif kv_on_act:
                    nc.scalar.activation(dst3, src3, AF.Identity, scale=WSI)
                else:
                    nc.vector.tensor_scalar(dst3, src3, WSI, None, op0=ALU.mult)
                nc.gpsimd.memset(vt.bitcast(BF16)[:, :, 64:65], 1.0 / OS)

        # ============ helper: softmax-normalize attention head ==============
        # writes fp8 contraction-folded oT2: oT2[hp//2][ab*64+e, hp%2, q]
        def normalize(ps_o, oT2set, hp, r0, c0, pfx):
            recip = lnp.tile([1, 512], F32, name=f"{pfx}r", tag="recip")
            nc.vector.reciprocal(recip[:], ps_o[64:65, :])
            rb = lnp.tile([64, 512], F32, name=f"{pfx}rb", tag="rb")
            nc.gpsimd.partition_broadcast(rb[:], recip[:])
            nc.vector.tensor_tensor(
                oT2set[hp // 2][r0:r0 + 64, hp % 2, c0:c0 + 512],
                ps_o[0:64, :], rb[:], op=ALU.mult)

        # ============ helper: out-projection (fp8 DoubleRow) ================
        def out_proj(oT2, catf2, wo_sb, li, pfx):
            for m in range(4):
                for n in range(2):
                    acc = ps.tile([128, 512], F32, name=f"{pfx}{m}{n}", tag="ps")
                    for pp in range(2):
                        nc.tensor.matmul(
                            acc[:], wo_sb[:, li, pp, :, m * 128:(m + 1) * 128],
                            oT2[pp][:, :, n * 512:(n + 1) * 512],
                            start=(pp == 0), stop=(pp == 1), perf_mode=DRM)
                    dst = catf2[m // 2][:, m % 2, n * 512:(n + 1) * 512]
                    if use_bo:
                        nc.vector.tensor_scalar(
                            dst, acc[:], WSI, bo2_sb[:, li, m:m + 1],
                            op0=ALU.mult, op1=ALU.add)
                    else:
                        nc.vector.tensor_scalar(dst, acc[:], WSI, None,
                                                op0=ALU.mult)

        # ============ Phase B: local qkv ====================================
        qT_l = [qTp.tile([128, NQ], BF16, name=f"qTl{m}", tag="qT")
                for m in range(4)]
        kT_l = [kTp.tile([128, S], BF16, name=f"kTl{m}", tag="kT")
                for m in range(4)]
        V_l = [Vp.tile([128, 8, 65], BF16, name=f"Vl{pt}", tag="V")
               for pt in range(KL0 // 128, KL1 // 128)]
        wqkv_l_sb = wp.tile([128, 3, 2, 2, D], FP8, name="wqkv_l_sb", tag="wbig")
        nc.sync.dma_start(wqkv_l_sb[:], wqkv8_l[:])
        project_qkv(wqkv_l_sb, bqkv_l_sb, bv_l_sb, qT_l, kT_l, V_l,
                    KL0, KL1, KL0 // 128, "Bl", kv_on_act=LOCAL_KV_ON_ACT)

        # ============ Phase C: local (band) attention + out-proj ============
        oT2_l = [s4.tile([128, 2, NQ], FP8, name=f"oTl{mm}", tag="s4a", bufs=4)
                 for mm in range(2)]
        with ExitStack() as sC:
            pC = sC.enter_context(tc.tile_pool(name="pC", bufs=1))
            masks_m_sb = pC.tile([128, 4, 512], BF16, name="masks_m_sb")
            nc.scalar.dma_start(masks_m_sb[:], masks_m[:])
            masks_e_sb = pC.tile([128, 2, 2, 32], BF16, name="masks_e_sb")
            nc.sync.dma_start(masks_e_sb[:], masks_e[:])
            MAIN_DELTAS = (0, 128, 256, 384)
            PT = {}
            for di, dd in enumerate(MAIN_DELTAS):
                t = pC.tile([128, 2, 512], BF16, name=f"PTl{di}")
                nc.gpsimd.memset(t[:], 0.0)
                PT[dd] = t
            for de_i, de in enumerate(EDGE_DELTAS):
                PT[de] = pC.tile([128, 2, 32], BF16, name=f"PTe{de_i}")
            for qb in range(2):
                q0 = Q0 + qb * 512
                for hp in range(4):
                    for di, dd in enumerate(MAIN_DELTAS):
                        qq0, qq1 = STRIPE[dd]
                        rel = q0 + dd - KL0
                        sc2 = ps2.tile([128, 2, 512], F32,
                                       name=f"psC{qb}{hp}{di}", tag="ps2")
                        for ab in range(2):
                            r0 = ab * 64
                            nc.tensor.matmul(
                                sc2[:, ab, qq0:qq1],
                                kT_l[hp].bitcast(BF16)[r0:r0 + 64, rel:rel + 128],
                                qT_l[hp].bitcast(BF16)
                                [r0:r0 + 64, qb * 512 + qq0: qb * 512 + qq1],
                                start=True, stop=True, tile_position=(r0, 0))
                        pt_t = PT[dd]
                        nc.scalar.activation(
                            pt_t[:, :, qq0:qq1], sc2[:, :, qq0:qq1],
                            AF.Exp, scale=SCALE)
                        nc.vector.tensor_tensor(
                            pt_t[:, :, qq0:qq1], pt_t[:, :, qq0:qq1],
                            masks_m_sb[:, di, qq0:qq1].unsqueeze(1)
                            .to_broadcast((128, 2, qq1 - qq0)), op=ALU.mult)
                    for de_i, de in enumerate(EDGE_DELTAS):
                        qq0, qq1 = STRIPE[de]
                        rel = q0 + de - KL0
                        sc2 = ps2.tile([128, 2, 512], F32,
                                       name=f"psCe{qb}{hp}{de_i}", tag="ps2")
                        for ab in range(2):
                            r0 = ab * 64
                            nc.tensor.matmul(
                                sc2[:, ab, 0:32],
                                kT_l[hp].bitcast(BF16)[r0:r0 + 64, rel:rel + 128],
                                qT_l[hp].bitcast(BF16)
                                [r0:r0 + 64, qb * 512 + qq0: qb * 512 + qq1],
                                start=True, stop=True, tile_position=(r0, 0))
                        pt_t = PT[de]
                        nc.scalar.activation(
                            pt_t[:], sc2[:, :, 0:32], AF.Exp, scale=SCALE)
                        nc.vector.tensor_tensor(
                            pt_t[:], pt_t[:],
                            masks_e_sb[:, de_i, qb, :].unsqueeze(1)
                            .to_broadcast((128, 2, 32)), op=ALU.mult)
                    for ab in range(2):
                        head = 2 * hp + ab
                        po_t = pso.tile([65, 512], F32, name=f"psoC{qb}{hp}{ab}",
                                        tag=f"pso{ab}", bufs=1)
                        nc.tensor.matmul(
                            po_t[:], V_l[(q0 - KL0) // 128].bitcast(BF16)[:, head, :],
                            PT[0][:, ab, :], start=True, stop=False,
                            skip_group_check=True)
                        for de in EDGE_DELTAS:
                            qq0, qq1 = STRIPE[de]
                            nc.tensor.matmul(
                                po_t[:, qq0:qq1],
                                V_l[(q0 + de - KL0) // 128].bitcast(BF16)[:, head, :],
                                PT[de][:, ab, :],
                                start=False, stop=False, skip_group_check=True)
                        for dd in (128, 256, 384):
                            nc.tensor.matmul(
                                po_t[:],
                                V_l[(q0 + dd - KL0) // 128].bitcast(BF16)[:, head, :],
                                PT[dd][:, ab, :], start=False, stop=(dd == 384),
                                skip_group_check=True)
                        normalize(po_t, oT2_l, hp, ab * 64, qb * 512,
                                  f"nC{qb}{hp}{ab}")
        if debug:
            dbgc = pers.tile([128, NQ], BF16, name="dbg_oTl")
            nc.vector.tensor_copy(dbgc[:], oT2_l[0][:, 0, :])
            nc.sync.dma_start(dbg["d_oTl"][:], dbgc[:])

        wo_sb = wp.tile([128, 2, 2, 2, D], FP8, name="wo_sb", tag="wo2nd")
        nc.scalar.dma_start(wo_sb[:], wo8[:])
        # catf[0..1]=local out fp8-fold, catf[2..3]=global out fp8-fold
        catf = [s4.tile([128, 2, NQ], FP8, name=f"catf{i}", tag="s4b", bufs=4)
                for i in range(4)]
        out_proj(oT2_l, catf[0:2], wo_sb, 0, "psFl")

        # ============ Phase D: global qkv ===================================
        qT_g = [qTp.tile([128, NQ], BF16, name=f"qTg{m}", tag="qT")
                for m in range(4)]
        kT_g = [kTp.tile([128, S], BF16, name=f"kTg{m}", tag="kT")
                for m in range(4)]
        V_g = [Vp.tile([128, 8, 65], BF16, name=f"Vg{pt}", tag="V")
               for pt in range(16)]
        wqkv_g_sb = wp.tile([128, 3, 2, 2, D], FP8, name="wqkv_g_sb", tag="wbig")
        nc.scalar.dma_start(wqkv_g_sb[:], wqkv8_g[:])
        project_qkv(wqkv_g_sb, bqkv_g_sb, bv_g_sb, qT_g, kT_g, V_g, 0, S, 0, "Dg", kv_on_act=GLOBAL_KV_ON_ACT)

        # ============ Phase E: global attention + out-proj ==================
        oT2_g = [s4.tile([128, 2, NQ], FP8, name=f"oTg{mm}", tag="s4c", bufs=8)
                 for mm in range(2)]
        for qb in range(2):
            for hp in range(4):
                po_ts = [pso.tile([65, 512], F32, name=f"psoE{qb}{hp}{ab}",
                                  tag=f"pso{ab}", bufs=1) for ab in range(2)]
                for kt in range(16):
                    sc2 = ps2.tile([128, 2, 512], F32,
                                   name=f"psE{qb}{hp}{kt}", tag="ps2")
                    for ab in range(2):
                        r0 = ab * 64
                        nc.tensor.matmul(
                            sc2[:, ab, :], kT_g[hp].bitcast(BF16)
                            [r0:r0 + 64, kt * 128:(kt + 1) * 128],
                            qT_g[hp].bitcast(BF16)
                            [r0:r0 + 64, qb * 512:(qb + 1) * 512],
                            start=True, stop=True, tile_position=(r0, 0))
                    ptg = ptgp.tile([128, 2, 512], BF16,
                                    name=f"ptg{qb}{hp}{kt}", tag="ptg")
                    nc.scalar.activation(ptg[:], sc2[:], AF.Exp, scale=SCALE)
                    for ab in range(2):
                        nc.tensor.matmul(
                            po_ts[ab][:],
                            V_g[kt].bitcast(BF16)[:, 2 * hp + ab, :],
                            ptg[:, ab, :], start=(kt == 0), stop=(kt == 15),
                            skip_group_check=True)
                for ab in range(2):
                    normalize(po_ts[ab], oT2_g, hp, ab * 64, qb * 512,
                              f"nE{qb}{hp}{ab}")
        if debug:
            dbgc2 = pers.tile([128, NQ], BF16, name="dbg_oTg")
            nc.vector.tensor_copy(dbgc2[:], oT2_g[0][:, 0, :])
            nc.sync.dma_start(dbg["d_oTg"][:], dbgc2[:])

        out_proj(oT2_g, catf[2:4], wo_sb, 1, "psFg")

        # ============ Phase G: gate + fuse ==================================
        # gate = tanh(relu(z)); tanh monotone => tanh(relu(z)) = relu(tanh(z)),
        # so Act does Tanh(scale*acc+bias) and relu folds into the fuse mult.
        fusedT = [s4.tile([128, NQ], BF16, name=f"fusedT{m}", tag="s4a", bufs=4)
                  for m in range(4)]
        gate_w_sb = wp.tile([128, 4, 2, D], FP8, name="gate_w_sb", tag="wbig")
        nc.scalar.dma_start(gate_w_sb[:], gw8[:])
        for m in range(4):
            for n in range(2):
                acc = ps.tile([128, 512], F32, name=f"psG{m}{n}", tag="ps")
                for pp in range(4):
                    nc.tensor.matmul(
                        acc[:], gate_w_sb[:, pp, :, m * 128:(m + 1) * 128],
                        catf[pp][:, :, n * 512:(n + 1) * 512],
                        start=(pp == 0), stop=(pp == 3), perf_mode=DRM)
                gt = lnp.tile([128, 512], BF16, name=f"gt{m}{n}", tag="gt", bufs=1)
                nc.scalar.activation(
                    gt[:], acc[:], AF.Tanh, scale=GATE_SCALE,
                    bias=gate_b_sb[:, m:m + 1] if use_gate_b else 0.0)
                if debug and m == 0 and n == 0:
                    dbgg = pers.tile([128, 512], BF16, name="dbg_gt")
                    nc.vector.tensor_scalar(dbgg[:], gt[:], 0.0, None,
                                            op0=ALU.max)
                    nc.sync.dma_start(dbg["d_gateT"][:], dbgg[:])
                # fused = global + relu(gate)*(local - global)
                lsl = catf[m // 2][:, m % 2, n * 512:(n + 1) * 512]
                gsl = catf[2 + m // 2][:, m % 2, n * 512:(n + 1) * 512]
                tmp = lnp.tile([128, 512], BF16, name=f"tmpG{m}{n}", tag="tmpG", bufs=1)
                nc.gpsimd.tensor_tensor(tmp[:], lsl, gsl, op=ALU.subtract)
                nc.vector.scalar_tensor_tensor(
                    tmp[:], gt[:], 0.0, tmp[:], op0=ALU.max, op1=ALU.mult)
                nc.vector.tensor_tensor(
                    fusedT[m].bitcast(BF16)[:, n * 512:(n + 1) * 512],
                    tmp[:], gsl, op=ALU.add)
        if debug:
            nc.sync.dma_start(dbg["d_fusedT"][:], fusedT[0].bitcast(BF16)[:])

        # ===== layernorm helper (token-major [128, D]) ======================
        def layernorm(dst, src_ap, g_sb, b_sb, pfx):
            stats = lnp.tile([128, 6], F32, name=f"{pfx}st", tag="lnst")
            nc.vector.bn_stats(stats[:], src_ap)
            mv = lnp.tile([128, 2], F32, name=f"{pfx}mv", tag="lnmv")
            nc.vector.bn_aggr(mv[:], stats[:])
            std = lnp.tile([128, 1], F32, name=f"{pfx}sd", tag="lnsd")
            nc.scalar.activation(std[:], mv[:, 1:2], AF.Sqrt, bias=eps_sb[:])
            rstd = lnp.tile([128, 1], F32, name=f"{pfx}rs", tag="lnrs")
            nc.vector.reciprocal(rstd[:], std[:])
            if g_sb is not None:
                tmp = lnp.tile([128, D], F32, name=f"{pfx}tmp", tag="lntmp")
                nc.vector.tensor_scalar(
                    tmp[:], src_ap, mv[:, 0:1], rstd[:],
                    op0=ALU.subtract, op1=ALU.mult)
                if b_sb is not None:
                    nc.vector.tensor_tensor(dst, tmp[:], g_sb[:], op=ALU.mult)
                    nc.vector.tensor_tensor(dst, dst, b_sb[:], op=ALU.add)
                else:
                    nc.vector.tensor_tensor(dst, tmp[:], g_sb[:], op=ALU.mult)
            else:
                nc.vector.tensor_scalar(
                    dst, src_ap, mv[:, 0:1], rstd[:],
                    op0=ALU.subtract, op1=ALU.mult)
                if b_sb is not None:
                    nc.vector.tensor_tensor(dst, dst, b_sb[:], op=ALU.add)

        # ============ Phase H: x1 = h + fused^T; y1 = LN1 ===================
        y1 = [s2.tile([128, D], F32R, name=f"y1_{t}", tag="s2") for t in range(8)]
        for t in range(8):
            x1 = lnp.tile([128, D], F32, name=f"x1_{t}", tag="x1")
            for m in range(4):
                ptr = ps.tile([128, 128], BF16, name=f"ptrH{t}{m}", tag="ps")
                nc.tensor.transpose(
                    ptr[:], fusedT[m].bitcast(BF16)[:, t * 128:(t + 1) * 128],
                    eyeb_sb[:])
                nc.vector.scalar_tensor_tensor(
                    x1[:, m * 128:(m + 1) * 128], ptr[:], FUSE_INV,
                    f32(h_sb[t][:, m * 128:(m + 1) * 128]),
                    op0=ALU.mult, op1=ALU.add)
            layernorm(y1[t][:], x1[:], n1gb_sb, n1bb_sb, f"ln1_{t}")
        if debug:
            nc.sync.dma_start(dbg["d_y1"][:], f32(y1[0][:]))

        # ============ Phase I: y1T (fp8 contraction-fold) ===================
        y1T2 = [s4.tile([128, 2, NQ], FP8, name=f"y1T{mm}", tag="s4b", bufs=4)
                for mm in range(2)]
        for t in range(8):
            for m in range(4):
                ptr = ps.tile([128, 128], F32, name=f"ptrI{t}{m}", tag="ps")
                nc.tensor.transpose(ptr[:], f32(y1[t][:, m * 128:(m + 1) * 128]),
                                    eye_sb[:])
                nc.scalar.copy(
                    y1T2[m // 2][:, m % 2, t * 128:(t + 1) * 128], ptr[:])

        # ============ Phase J: FFN + LN2 + LN3; Phase K: pool + out =========
        w1_sb = wp.tile([128, 2, 2, DFF], FP8, name="w1_sb", tag="wbig")
        nc.scalar.dma_start(w1_sb[:], w18[:])
        w2_sb = wp.tile([128, 2, 4, 2, D], FP8, name="w2_sb", tag="wo2nd")
        nc.scalar.dma_start(w2_sb[:], w28[:])
        z8 = [s4.tile([128, 2, NQ], FP8, name=f"z1T{mm}", tag="s4c", bufs=8)
              for mm in range(4)]
        for m in range(8):
            for n in range(2):
                acc = ps.tile([128, 512], F32, name=f"psJ1{m}{n}", tag="ps")
                for pp in range(2):
                    nc.tensor.matmul(
                        acc[:], w1_sb[:, pp, :, m * 128:(m + 1) * 128],
                        y1T2[pp][:, :, n * 512:(n + 1) * 512],
                        start=(pp == 0), stop=(pp == 1), perf_mode=DRM)
                dst = z8[m // 2][:, m % 2, n * 512:(n + 1) * 512]
                nc.scalar.activation(
                    dst, acc[:], AF.Relu, scale=WSI,
                    bias=b1_sb[:, m:m + 1] if use_b1 else 0.0)

        y3 = [s2.tile([128, D], F32R, name=f"y3_{t}", tag="s2") for t in range(8)]
        accp = pso.tile([1, 512], F32, name="pspool", tag="pso0", bufs=1)
        for t in range(8):
            acc = ps.tile([128, 512], F32, name=f"psJ2{t}", tag="ps")
            for hl in range(2):       # w2 hi + lo fp8 terms (hi-lo split)
                for kk in range(4):
                    nc.tensor.matmul(
                        acc[:], z8[kk][:, :, t * 128:(t + 1) * 128],
                        w2_sb[:, hl, kk, :, :],
                        start=(hl == 0 and kk == 0),
                        stop=(hl == 1 and kk == 3), perf_mode=DRM)
            x2 = lnp.tile([128, D], F32, name=f"x2_{t}", tag="x2")
            nc.vector.scalar_tensor_tensor(
                x2[:], acc[:], WSI, f32(y1[t][:]), op0=ALU.mult, op1=ALU.add)
            if use_b2:
                nc.vector.tensor_tensor(x2[:], x2[:], b2b_sb[:], op=ALU.add)
            if not (use_n2g or use_n2b or use_n3g):
                # LN3(LN2(x)) with unit gamma / zero beta collapses to one LN:
                # mean(LN2 out) == 0 exactly, var(LN2 out) = v/(v+eps), so
                # y3 = (x - m) / sqrt(v*(1+eps) + eps^2)
                pfx = f"ln23_{t}"
                stats = lnp.tile([128, 6], F32, name=f"{pfx}st", tag="lnst")
                nc.vector.bn_stats(stats[:], x2[:])
                mv = lnp.tile([128, 2], F32, name=f"{pfx}mv", tag="lnmv")
                nc.vector.bn_aggr(mv[:], stats[:])
                std = lnp.tile([128, 1], F32, name=f"{pfx}sd", tag="lnsd")
                nc.scalar.activation(std[:], mv[:, 1:2], AF.Sqrt,
                                     bias=eps2_sb[:], scale=1.0 + EPS)
                rstd = lnp.tile([128, 1], F32, name=f"{pfx}rs", tag="lnrs")
                nc.vector.reciprocal(rstd[:], std[:])
                nc.vector.tensor_scalar(
                    y3[t][:], x2[:], mv[:, 0:1], rstd[:],
                    op0=ALU.subtract, op1=ALU.mult)
            else:
                y2 = lnp.tile([128, D], F32, name=f"y2_{t}", tag="y2")
                layernorm(y2[:], x2[:], n2gb_sb, n2bb_sb, f"ln2_{t}")
                layernorm(y3[t][:], y2[:], n3gb_sb, None, f"ln3_{t}")
            nc.tensor.matmul(accp[:], poolw_sb[:], y3[t][:],
                             start=(t == 0), stop=(t == 7),
                             skip_group_check=True)
        if debug:
            nc.sync.dma_start(dbg["d_y3"][:], f32(y3[0][:]))

        pooled_sb = pers.tile([1, D], F32, name="pooled_sb")
        nc.vector.tensor_copy(pooled_sb[:], accp[:])
        if debug:
            nc.sync.dma_start(dbg["d_pooled"][:], f32(pooled_sb[:]))
        nc.sync.dma_start(po[:], pooled_sb[:])

    nc.compile()
    return nc


def _prep_inputs(inputs):
    """Host-side prep: returns (flags, in_maps for 8 cores, host_const)."""
    g = {k: np.asarray(v, dtype=np.float32) for k, v in inputs.items()}
    x, pos = g["x"], g["pos"]
    win_w, win_b = g["win_w"], g["win_b"]

    flags = (
        bool(np.any(g["l_bqkv"] != 0)), bool(np.any(g["g_bqkv"] != 0)),
        bool(np.any(g["l_bo"] != 0) or np.any(g["g_bo"] != 0)),
        bool(np.any(g["gate_b"] != 0)), bool(np.any(g["ffn_b1"] != 0)),
        bool(np.any(g["ffn_b2"] != 0)),
        bool(np.any(g["n1_g"] != 1)), bool(np.any(g["n1_b"] != 0)),
        bool(np.any(g["n2_g"] != 1)), bool(np.any(g["n2_b"] != 0)),
        bool(np.any(g["n3_g"] != 1)),
    )
    (use_bqkv_l, use_bqkv_g, use_bo, use_gate_b, use_b1, use_b2,
     use_n1g, use_n1b, use_n2g, use_n2b, use_n3g) = flags

    posT = pos[0].T + win_b[:, None]                      # [D, S]

    def fold8(w3):
        # [3, D, D] -> [128, 3, 2pair, 2j, D]: w[qkv, pair*256 + j*128 + p, :]
        return np.ascontiguousarray(
            (w3.reshape(3, 2, 2, 128, D) * WS).transpose(3, 0, 1, 2, 4)
        ).astype(NPF8)

    def foldw(w, npair):
        # [K, N] -> [128, npair, 2, N]: w[pair*256 + j*128 + p, :] * WS
        kdim, n = w.shape
        assert kdim == npair * 256
        return np.ascontiguousarray(
            (w.reshape(npair, 2, 128, n) * WS).transpose(2, 0, 1, 3)
        ).astype(NPF8)

    common = {
        "win": np.ascontiguousarray(win_w),
        "wqkv8_l": fold8(g["l_wqkv"]),
        "wqkv8_g": fold8(g["g_wqkv"]),
        "wo8": np.ascontiguousarray(np.stack(
            [foldw(g["l_wo"], 2), foldw(g["g_wo"], 2)], axis=1)),
        "gw8": foldw(g["gate_w"], 4),
        "w18": foldw(g["ffn_w1"], 2),
        "eye": np.eye(128, dtype=np.float32),
        "poolw": np.full((128, 1), 1.0 / S, dtype=np.float32),
    }
    w2s = g["ffn_w2"] * WS
    w2hi = w2s.astype(NPF8).astype(np.float32)
    foldr = lambda w: np.ascontiguousarray(
        w.reshape(4, 2, 128, D).transpose(2, 0, 1, 3)).astype(NPF8)
    common["w28"] = np.ascontiguousarray(
        np.stack([foldr(w2hi), foldr(w2s - w2hi)], axis=1))
    perm = lambda b: b.reshape(-1, 4, 128).transpose(2, 0, 1).copy()
    if use_bqkv_l:
        common["bqkv_l"] = perm(g["l_bqkv"])
        common["bv_l"] = np.tile(g["l_bqkv"][2], (128, 1))
    if use_bqkv_g:
        common["bqkv_g"] = perm(g["g_bqkv"])
        common["bv_g"] = np.tile(g["g_bqkv"][2], (128, 1))
    if use_bo:
        common["bo2"] = perm(np.stack([g["l_bo"], g["g_bo"]])) * OS
    if use_gate_b:
        common["gate_b"] = g["gate_b"].reshape(4, 128).T.copy()
    if use_b1:
        common["b1"] = g["ffn_b1"].reshape(8, 128).T.copy()
    if use_b2:
        common["b2b"] = np.tile(g["ffn_b2"], (128, 1))
    if use_n1g:
        common["n1gb"] = np.tile(g["n1_g"], (128, 1))
    if use_n1b:
        common["n1bb"] = np.tile(g["n1_b"], (128, 1))
    if use_n2g:
        common["n2gb"] = np.tile(g["n2_g"], (128, 1))
    if use_n2b:
        common["n2bb"] = np.tile(g["n2_b"], (128, 1))
    if use_n3g:
        common["n3gb"] = np.tile(g["n3_g"], (128, 1))

    # universal interior band masks (pure Toeplitz, no seam crossing)
    kk = np.arange(128)
    qq = np.arange(512)
    mk_m = np.zeros((128, 4, 512), dtype=np.float32)
    for di, d in enumerate((0, 128, 256, 384)):
        mk_m[:, di, :] = (np.abs(kk[:, None] + d - qq[None, :]) <= W // 2)
    mk_m = mk_m.astype(ml_dtypes.bfloat16)

    hf_data = []
    for hf in range(2):
        q0c = NQ * hf
        shift = Q0 - q0c
        posb_rot = np.ascontiguousarray(np.roll(posT, shift, axis=1))
        mk_e = np.zeros((128, 2, 2, 32), dtype=np.float32)
        for qb in range(2):
            q0 = Q0 + qb * 512
            for de_i, d in enumerate(EDGE_DELTAS):
                qq0, qq1 = STRIPE[d]
                k_rot = q0 + d + kk[:, None]
                q_rot = q0 + np.arange(qq0, qq1)[None, :]
                orig_k = (k_rot - shift) % S
                orig_q = (q_rot - shift) % S
                mk_e[:, de_i, qb, :] = (np.abs(orig_k - orig_q) <= W // 2)
        hf_data.append((posb_rot, mk_e.astype(ml_dtypes.bfloat16)))

    in_maps = []
    for core in range(N_CORES):
        b, hf = core // 2, core % 2
        shift = Q0 - NQ * hf
        posb_rot, mk_e = hf_data[hf]
        m = dict(common)
        m["xT"] = np.ascontiguousarray(np.roll(x[b].T, shift, axis=1))
        m["posb"] = posb_rot
        m["masks_m"] = mk_m
        m["masks_e"] = mk_e
        in_maps.append(m)

    host_const = (g["n3_b"] @ g["out_w"] + g["out_b"],
                  np.ascontiguousarray(g["out_w"]))
    return flags, in_maps, host_const


def kernel(**inputs):
    flags, in_maps, host_const = _prep_inputs(inputs)
    const_vec, out_w = host_const
    if flags not in _CACHE:
        _CACHE[flags] = _build(flags)
    nc = _CACHE[flags]
    res = run_bass_kernel_spmd(nc, in_maps, core_ids=list(range(N_CORES)))
    out = np.zeros((B, DOUT), dtype=np.float32)
    for b in range(B):
        pooled = res.results[2 * b]["po"][0] + res.results[2 * b + 1]["po"][0]
        out[b] = pooled @ out_w + const_vec
    return out



# revision 45
# speedup vs baseline: 1.1703x; 1.0520x over previous
"""DualPathTransformer Trainium2 kernel.

Sharding: 8 cores = batch(4) x query-half(2). Each core processes one batch
and 1024 query tokens; K/V work is duplicated within a batch pair. No
device collectives: partial pooled projections are summed on the host.

SPMD uniformity trick: each core receives its batch token-ROTATED so that
its query tokens sit at rotated positions [512, 1536). Global attention is
permutation-invariant over keys; the local band structure is encoded in
host-prepped per-core mask tiles in true original coordinates. The program
is identical on all cores; only input data differs.

Layouts: activations feature-major (hT = [feature partitions, tokens]) for
matmuls; token-major (tokens on partitions) for layernorm stages. Scores
are computed transposed (keys on partitions) so softmax denominators come
free from a ones-row appended to V, and the AV matmul needs no transposes.

Precision: residual stream and weights fp32/f32r; attention q/k/v/probs and
post-attention projections bf16 (error contribution ~1e-3 of the stream).
"""

import numpy as np
import ml_dtypes
from contextlib import ExitStack

import concourse.bass as bass
import concourse.bacc as bacc
import concourse.tile as tile
import concourse.mybir as mybir
from concourse.bass_utils import run_bass_kernel_spmd

F32R = mybir.dt.float32r
F32 = mybir.dt.float32
BF16 = mybir.dt.bfloat16
FP8 = mybir.dt.float8e4
NPF8 = ml_dtypes.float8_e4m3
AF = mybir.ActivationFunctionType
ALU = mybir.AluOpType
DRM = mybir.MatmulPerfMode.DoubleRow
WS = 64.0          # fp8 weight pre-scale (host); compensated at psum drain
WSI = 1.0 / WS
OS = 16.0          # fp8 scale on oT / catf (lift tiny values out of subnormals)
# oT2 = OS*o via ones-row = 1/OS; catf = OS*out_proj via drain scale
# CS_DRAIN: psum(out_proj) = OS*o @ WS*wo -> *OS/(OS*WS) = WSI keeps catf at OS x
GATE_SCALE = 1.0 / (OS * WS)   # gate psum = OS*cat @ WS*gw
FUSE_INV = 1.0 / OS            # fusedT carries OS x; divided out at x1 add

B, S, DIN, D, H, DOUT, W = 4, 2048, 256, 512, 8, 128, 64
HD = D // H          # 64
DFF = 2 * D          # 1024
NQ = S // 2          # 1024 queries per core
N_CORES = 8
Q0 = 512             # rotated position of first query token (uniform)
KL0, KL1 = 384, 1664   # local K/V window in rotated coords (10 ptiles)
NKL = KL1 - KL0        # 1280
DELTAS = (-128, 0, 128, 256, 384, 512)   # local kblock offsets rel. to qblock
# stripe (bounding qq range) per delta, qblock-relative
STRIPE = {-128: (0, 32), 0: (0, 160), 128: (96, 288),
          256: (224, 416), 384: (352, 512), 512: (480, 512)}
EDGE_DELTAS = (-128, 512)          # AV mms sliced to the stripe
SCALE = 1.0 / float(np.sqrt(HD))
EPS = 1e-5

_CACHE = {}
GLOBAL_KV_ON_ACT = False
LOCAL_KV_ON_ACT = True


def _build(flags, debug=False):
    (use_bqkv_l, use_bqkv_g, use_bo, use_gate_b, use_b1, use_b2,
     use_n1g, use_n1b, use_n2g, use_n2b, use_n3g) = flags

    nc = bacc.Bacc("TRN2", target_bir_lowering=False, debug=False)

    def din(name, shape, dt=F32R):
        return nc.dram_tensor(name, list(shape), dt, kind="ExternalInput").ap()

    xT = din("xT", [DIN, S], BF16)
    posb = din("posb", [D, S], BF16)
    win = din("win", [DIN, D], BF16)
    wqkv8_l = din("wqkv8_l", [128, 3, 2, 2, D], FP8)
    wqkv8_g = din("wqkv8_g", [128, 3, 2, 2, D], FP8)
    wo8 = din("wo8", [128, 2, 2, 2, D], FP8)   # [p, li, pair, j, fout]
    gw8 = din("gw8", [128, 4, 2, D], FP8)      # [p, pair, j, fout]
    w18 = din("w18", [128, 2, 2, DFF], FP8)
    w28 = din("w28", [128, 2, 4, 2, D], FP8)   # [p, hi/lo, pair, j, fout]
    masks_m = din("masks_m", [128, 4, 512], BF16)   # [kk, di, qq]
    masks_e = din("masks_e", [128, 2, 2, 32], BF16)  # [kk, de, qb, qq32]
    eye = din("eye", [128, 128], F32)
    poolw = din("poolw", [128, 1])
    if use_bqkv_l:
        bqkv_l = din("bqkv_l", [128, 3, 4], F32)
        bv_l = din("bv_l", [128, D], F32)
    if use_bqkv_g:
        bqkv_g = din("bqkv_g", [128, 3, 4], F32)
        bv_g = din("bv_g", [128, D], F32)
    if use_bo:
        bo2 = din("bo2", [128, 2, 4], F32)
    if use_gate_b:
        gate_b = din("gate_b", [128, 4], F32)
    if use_b1:
        b1 = din("b1", [128, 8], F32)
    if use_b2:
        b2b = din("b2b", [128, D], F32)
    if use_n1g:
        n1gb = din("n1gb", [128, D], F32)
    if use_n1b:
        n1bb = din("n1bb", [128, D], F32)
    if use_n2g:
        n2gb = din("n2gb", [128, D], F32)
    if use_n2b:
        n2bb = din("n2bb", [128, D], F32)
    if use_n3g:
        n3gb = din("n3gb", [128, D], F32)
    # n3_b handled on host (pooled mean is linear in it)

    po = nc.dram_tensor("po", [1, D], F32, kind="ExternalOutput").ap()

    dbg = {}
    if debug:
        for nm, shp, dt_ in [("d_hT", [128, S], F32), ("d_oTl", [128, NQ], BF16),
                             ("d_oTg", [128, NQ], BF16), ("d_gateT", [128, 512], BF16),
                             ("d_fusedT", [128, NQ], BF16), ("d_y1", [128, D], F32),
                             ("d_y3", [128, D], F32), ("d_pooled", [1, D], F32)]:
            dbg[nm] = nc.dram_tensor(nm, shp, dt_, kind="ExternalOutput").ap()

    f32 = lambda ap: ap.bitcast(F32)

    with tile.TileContext(nc) as tc, ExitStack() as top:
        # ---- psum pools (8 banks) ----
        ps = top.enter_context(tc.tile_pool(name="ps", bufs=2, space="PSUM"))
        ps2 = top.enter_context(tc.tile_pool(name="ps2", bufs=2, space="PSUM"))
        pso = top.enter_context(tc.tile_pool(name="pso", bufs=1, space="PSUM"))

        # ---- persistent pools (static tags, round-robin slot reuse) ----
        pers = top.enter_context(tc.tile_pool(name="pers", bufs=1))
        lnp = top.enter_context(tc.tile_pool(name="lnp", bufs=2))
        wp = top.enter_context(tc.tile_pool(name="wp", bufs=1))
        s4 = top.enter_context(tc.tile_pool(name="s4", bufs=1))     # [128,1024] bf16 tags
        s2 = top.enter_context(tc.tile_pool(name="s2", bufs=11))    # [128,512] f32
        qTp = top.enter_context(tc.tile_pool(name="qTp", bufs=4))   # [128,1024] bf16
        kTp = top.enter_context(tc.tile_pool(name="kTp", bufs=4))   # [128,2048] bf16
        hTp = top.enter_context(tc.tile_pool(name="hTp", bufs=1))
        Vp = top.enter_context(tc.tile_pool(name="Vp", bufs=16))    # [128,8,65] bf16
        ptgp = top.enter_context(tc.tile_pool(name="ptgp", bufs=3)) # pair bf16

        eye_sb = pers.tile([128, 128], F32, name="eye_sb")
        nc.sync.dma_start(eye_sb[:], eye[:])
        eyeb_sb = pers.tile([128, 128], BF16, name="eyeb_sb")
        nc.vector.tensor_copy(eyeb_sb[:], eye_sb[:])
        poolw_sb = pers.tile([128, 1], F32R, name="poolw_sb")
        nc.sync.dma_start(poolw_sb[:], poolw[:])
        eps_sb = pers.tile([128, 1], F32, name="eps_sb")
        nc.vector.memset(eps_sb[:], EPS)
        eps2_sb = pers.tile([128, 1], F32, name="eps2_sb")
        nc.vector.memset(eps2_sb[:], EPS * EPS)

        def load_bias(ap_dram, shape, name):
            t = pers.tile(shape, F32, name=name)
            nc.sync.dma_start(t[:], ap_dram[:])
            return t
        bqkv_l_sb = load_bias(bqkv_l, [128, 3, 4], "bqkv_l_sb") if use_bqkv_l else None
        bv_l_sb = load_bias(bv_l, [128, D], "bv_l_sb") if use_bqkv_l else None
        bqkv_g_sb = load_bias(bqkv_g, [128, 3, 4], "bqkv_g_sb") if use_bqkv_g else None
        bv_g_sb = load_bias(bv_g, [128, D], "bv_g_sb") if use_bqkv_g else None
        bo2_sb = load_bias(bo2, [128, 2, 4], "bo2_sb") if use_bo else None
        gate_b_sb = load_bias(gate_b, [128, 4], "gate_b_sb") if use_gate_b else None
        b1_sb = load_bias(b1, [128, 8], "b1_sb") if use_b1 else None
        b2b_sb = load_bias(b2b, [128, D], "b2b_sb") if use_b2 else None
        n1gb_sb = load_bias(n1gb, [128, D], "n1gb_sb") if use_n1g else None
        n1bb_sb = load_bias(n1bb, [128, D], "n1bb_sb") if use_n1b else None
        n2gb_sb = load_bias(n2gb, [128, D], "n2gb_sb") if use_n2g else None
        n2bb_sb = load_bias(n2bb, [128, D], "n2bb_sb") if use_n2b else None
        n3gb_sb = load_bias(n3gb, [128, D], "n3gb_sb") if use_n3g else None

        # long-lived stream tiles
        hT = [hTp.tile([128, S], F32R, name=f"hT{m}", tag="hT", bufs=4)
              for m in range(4)]
        h_sb = [s2.tile([128, D], F32R, name=f"h{t}", tag="s2") for t in range(8)]

        # ============ Phase A: hT + h ======================================
        # DMA priority: win + first x chunk first so PE starts ASAP; pos
        # chunks land per-m right before their adds.
        with ExitStack() as sA:
            pA = sA.enter_context(tc.tile_pool(name="pA", bufs=2))
            win_sb = pA.tile([128, 2, D], BF16, name="win_sb", tag="win", bufs=1)
            nc.sync.dma_start(win_sb[:], win.rearrange("(t p) n -> p t n", p=128))
            xTc = [pA.tile([128, 2, 1024], BF16, name=f"xTc{c}", tag="xTc")
                   for c in range(2)]
            nc.sync.dma_start(
                xTc[0][:], xT.rearrange("(t p) n -> p t n", p=128)[:, :, 0:1024])
            hTpos = pA.tile([128, 4, S], BF16, name="hTpos", tag="hTpos", bufs=1)
            for m in range(4):
                nc.sync.dma_start(
                    hTpos[:, m, :],
                    posb.rearrange("(t p) n -> p t n", p=128)[:, m, :])
            nc.sync.dma_start(
                xTc[1][:], xT.rearrange("(t p) n -> p t n", p=128)[:, :, 1024:2048])
            for c in range(2):
                for m in range(4):
                    for hh in range(2):
                        acc = ps.tile([128, 512], F32, name=f"psA{m}{c}{hh}",
                                      tag="ps")
                        for kt in range(2):
                            nc.tensor.matmul(
                                acc[:], win_sb[:, kt, m * 128:(m + 1) * 128],
                                xTc[c][:, kt, hh * 512:(hh + 1) * 512],
                                start=(kt == 0), stop=(kt == 1))
                        cl = c * 1024 + hh * 512
                        nc.vector.tensor_tensor(
                            hT[m][:, cl:cl + 512], acc[:],
                            hTpos[:, m, cl:cl + 512], op=ALU.add)
        # fp8 contraction-folded copy of hT for DoubleRow projections:
        # hT2[pp][p, j, n] = h[pp*256 + j*128 + p, n]
        hT2 = [hTp.tile([128, 2, S], FP8, name=f"hT2_{pp}", tag="hT2", bufs=2)
               for pp in range(2)]
        for pp in range(2):
            for j in range(2):
                nc.vector.tensor_copy(hT2[pp][:, j, :], f32(hT[2 * pp + j][:]))
        if debug:
            nc.sync.dma_start(dbg["d_hT"][:], f32(hT[0][:]))

        # ============ helper: qkv projection (fp8 DoubleRow) ================
        def project_qkv(w8_sb, bias_sb, bv_sb, q_tiles, kT_tiles, v_tiles,
                        kT_lo, kT_hi, v_pt_lo, pfx, kv_on_act=True):
            for m in range(4):
                for n in range(2):
                    acc = ps.tile([128, 512], F32, name=f"{pfx}q{m}{n}", tag="ps")
                    for pp in range(2):
                        nc.tensor.matmul(
                            acc[:], w8_sb[:, 0, pp, :, m * 128:(m + 1) * 128],
                            hT2[pp][:, :, Q0 + n * 512: Q0 + (n + 1) * 512],
                            start=(pp == 0), stop=(pp == 1), perf_mode=DRM)
                    dst = q_tiles[m].bitcast(BF16)[:, n * 512:(n + 1) * 512]
                    if bias_sb is not None:
                        nc.vector.tensor_scalar(
                            dst, acc[:], WSI, bias_sb[:, 0, m:m + 1],
                            op0=ALU.mult, op1=ALU.add)
                    else:
                        nc.vector.tensor_scalar(
                            dst, acc[:], WSI, None, op0=ALU.mult)
            nk = kT_hi - kT_lo
            for m in range(4):
                for off in range(0, nk, 512):
                    w_ = min(512, nk - off)
                    acc = ps.tile([128, 512], F32, name=f"{pfx}k{m}{off}",
                                  tag="ps")
                    for pp in range(2):
                        nc.tensor.matmul(
                            acc[:, 0:w_], w8_sb[:, 1, pp, :, m * 128:(m + 1) * 128],
                            hT2[pp][:, :, kT_lo + off: kT_lo + off + w_],
                            start=(pp == 0), stop=(pp == 1), perf_mode=DRM)
                    dst = kT_tiles[m].bitcast(BF16)[:, off:off + w_]
                    if kv_on_act:
                        nc.scalar.activation(
                            dst, acc[:, 0:w_], AF.Identity, scale=WSI,
                            bias=bias_sb[:, 1, m:m + 1]
                            if bias_sb is not None else 0.0)
                    elif bias_sb is not None:
                        nc.vector.tensor_scalar(
                            dst, acc[:, 0:w_], WSI, bias_sb[:, 1, m:m + 1],
                            op0=ALU.mult, op1=ALU.add)
                    else:
                        nc.vector.tensor_scalar(dst, acc[:, 0:w_], WSI, None,
                                                op0=ALU.mult)
            for i, vt in enumerate(v_tiles):
                pt = v_pt_lo + i
                acc = ps.tile([128, 512], F32, name=f"{pfx}v{pt}", tag="ps")
                for pp in range(2):
                    nc.tensor.matmul(
                        acc[:], hT2[pp][:, :, pt * 128:(pt + 1) * 128],
                        w8_sb[:, 2, pp, :, :],
                        start=(pp == 0), stop=(pp == 1), perf_mode=DRM)
                dst3 = vt.bitcast(BF16)[:, :, 0:64]
                src3 = acc[:].rearrange("p (h e) -> p h e", h=8)
                if bv_sb is not None:
                    nc.vector.scalar_tensor_tensor(
                        dst3, src3, WSI,
                        f32(bv_sb[:]).rearrange("p (h e) -> p h e", h=8),
                        op0=ALU.mult, op1=ALU.add)
                elif kv_on_act:
                    nc.scalar.activation(dst3, src3, AF.Identity, scale=WSI)
                else:
                    nc.vector.tensor_scalar(dst3, src3, WSI, None, op0=ALU.mult)
                nc.gpsimd.memset(vt.bitcast(BF16)[:, :, 64:65], 1.0 / OS)

        # ============ helper: softmax-normalize attention head ==============
        # writes fp8 contraction-folded oT2: oT2[hp//2][ab*64+e, hp%2, q]
        def normalize(ps_o, oT2set, hp, r0, c0, pfx):
            recip = lnp.tile([1, 512], F32, name=f"{pfx}r", tag="recip")
            nc.vector.reciprocal(recip[:], ps_o[64:65, :])
            rb = lnp.tile([64, 512], F32, name=f"{pfx}rb", tag="rb")
            nc.gpsimd.partition_broadcast(rb[:], recip[:])
            nc.vector.tensor_tensor(
                oT2set[hp // 2][r0:r0 + 64, hp % 2, c0:c0 + 512],
                ps_o[0:64, :], rb[:], op=ALU.mult)

        # ============ helper: out-projection (fp8 DoubleRow) ================
        def out_proj(oT2, catf2, wo_sb, li, pfx):
            for m in range(4):
                for n in range(2):
                    acc = ps.tile([128, 512], F32, name=f"{pfx}{m}{n}", tag="ps")
                    for pp in range(2):
                        nc.tensor.matmul(
                            acc[:], wo_sb[:, li, pp, :, m * 128:(m + 1) * 128],
                            oT2[pp][:, :, n * 512:(n + 1) * 512],
                            start=(pp == 0), stop=(pp == 1), perf_mode=DRM)
                    dst = catf2[m // 2][:, m % 2, n * 512:(n + 1) * 512]
                    if use_bo:
                        nc.vector.tensor_scalar(
                            dst, acc[:], WSI, bo2_sb[:, li, m:m + 1],
                            op0=ALU.mult, op1=ALU.add)
                    else:
                        nc.vector.tensor_scalar(dst, acc[:], WSI, None,
                                                op0=ALU.mult)

        # ============ Phase B: local qkv ====================================
        qT_l = [qTp.tile([128, NQ], BF16, name=f"qTl{m}", tag="qT")
                for m in range(4)]
        kT_l = [kTp.tile([128, S], BF16, name=f"kTl{m}", tag="kT")
                for m in range(4)]
        V_l = [Vp.tile([128, 8, 65], BF16, name=f"Vl{pt}", tag="V")
               for pt in range(KL0 // 128, KL1 // 128)]
        wqkv_l_sb = wp.tile([128, 3, 2, 2, D], FP8, name="wqkv_l_sb", tag="wbig")
        nc.sync.dma_start(wqkv_l_sb[:], wqkv8_l[:])
        project_qkv(wqkv_l_sb, bqkv_l_sb, bv_l_sb, qT_l, kT_l, V_l,
                    KL0, KL1, KL0 // 128, "Bl", kv_on_act=LOCAL_KV_ON_ACT)

        # ============ Phase C: local (band) attention + out-proj ============
        oT2_l = [s4.tile([128, 2, NQ], FP8, name=f"oTl{mm}", tag="s4a", bufs=4)
                 for mm in range(2)]
        with ExitStack() as sC:
            pC = sC.enter_context(tc.tile_pool(name="pC", bufs=1))
            masks_m_sb = pC.tile([128, 4, 512], BF16, name="masks_m_sb")
            nc.scalar.dma_start(masks_m_sb[:], masks_m[:])
            masks_e_sb = pC.tile([128, 2, 2, 32], BF16, name="masks_e_sb")
            nc.sync.dma_start(masks_e_sb[:], masks_e[:])
            MAIN_DELTAS = (0, 128, 256, 384)
            PT = {}
            for di, dd in enumerate(MAIN_DELTAS):
                t = pC.tile([128, 2, 512], BF16, name=f"PTl{di}")
                nc.gpsimd.memset(t[:], 0.0)
                PT[dd] = t
            for de_i, de in enumerate(EDGE_DELTAS):
                PT[de] = pC.tile([128, 2, 32], BF16, name=f"PTe{de_i}")
            for qb in range(2):
                q0 = Q0 + qb * 512
                for hp in range(4):
                    for di, dd in enumerate(MAIN_DELTAS):
                        qq0, qq1 = STRIPE[dd]
                        rel = q0 + dd - KL0
                        sc2 = ps2.tile([128, 2, 512], F32,
                                       name=f"psC{qb}{hp}{di}", tag="ps2")
                        for ab in range(2):
                            r0 = ab * 64
                            nc.tensor.matmul(
                                sc2[:, ab, qq0:qq1],
                                kT_l[hp].bitcast(BF16)[r0:r0 + 64, rel:rel + 128],
                                qT_l[hp].bitcast(BF16)
                                [r0:r0 + 64, qb * 512 + qq0: qb * 512 + qq1],
                                start=True, stop=True, tile_position=(r0, 0))
                        pt_t = PT[dd]
                        nc.scalar.activation(
                            pt_t[:, :, qq0:qq1], sc2[:, :, qq0:qq1],
                            AF.Exp, scale=SCALE)
                        nc.vector.tensor_tensor(
                            pt_t[:, :, qq0:qq1], pt_t[:, :, qq0:qq1],
                            masks_m_sb[:, di, qq0:qq1].unsqueeze(1)
                            .to_broadcast((128, 2, qq1 - qq0)), op=ALU.mult)
                    for de_i, de in enumerate(EDGE_DELTAS):
                        qq0, qq1 = STRIPE[de]
                        rel = q0 + de - KL0
                        sc2 = ps2.tile([128, 2, 512], F32,
                                       name=f"psCe{qb}{hp}{de_i}", tag="ps2")
                        for ab in range(2):
                            r0 = ab * 64
                            nc.tensor.matmul(
                                sc2[:, ab, 0:32],
                                kT_l[hp].bitcast(BF16)[r0:r0 + 64, rel:rel + 128],
                                qT_l[hp].bitcast(BF16)
                                [r0:r0 + 64, qb * 512 + qq0: qb * 512 + qq1],
                                start=True, stop=True, tile_position=(r0, 0))
                        pt_t = PT[de]
                        nc.scalar.activation(
                            pt_t[:], sc2[:, :, 0:32], AF.Exp, scale=SCALE)
                        nc.vector.tensor_tensor(
                            pt_t[:], pt_t[:],
                            masks_e_sb[:, de_i, qb, :].unsqueeze(1)
                            .to_broadcast((128, 2, 32)), op=ALU.mult)
                    for ab in range(2):
                        head = 2 * hp + ab
                        po_t = pso.tile([65, 512], F32, name=f"psoC{qb}{hp}{ab}",
                                        tag=f"pso{ab}", bufs=1)
                        nc.tensor.matmul(
                            po_t[:], V_l[(q0 - KL0) // 128].bitcast(BF16)[:, head, :],
                            PT[0][:, ab, :], start=True, stop=False,
                            skip_group_check=True)
                        for de in EDGE_DELTAS:
                            qq0, qq1 = STRIPE[de]
                            nc.tensor.matmul(
                                po_t[:, qq0:qq1],
                                V_l[(q0 + de - KL0) // 128].bitcast(BF16)[:, head, :],
                                PT[de][:, ab, :],
                                start=False, stop=False, skip_group_check=True)
                        for dd in (128, 256, 384):
                            nc.tensor.matmul(
                                po_t[:],
                                V_l[(q0 + dd - KL0) // 128].bitcast(BF16)[:, head, :],
                                PT[dd][:, ab, :], start=False, stop=(dd == 384),
                                skip_group_check=True)
                        normalize(po_t, oT2_l, hp, ab * 64, qb * 512,
                                  f"nC{qb}{hp}{ab}")
        if debug:
            dbgc = pers.tile([128, NQ], BF16, name="dbg_oTl")
            nc.vector.tensor_copy(dbgc[:], oT2_l[0][:, 0, :])
            nc.sync.dma_start(dbg["d_oTl"][:], dbgc[:])

        wo_sb = wp.tile([128, 2, 2, 2, D], FP8, name="wo_sb", tag="wo2nd")
        nc.scalar.dma_start(wo_sb[:], wo8[:])
        # catf[0..1]=local out fp8-fold, catf[2..3]=global out fp8-fold
        catf = [s4.tile([128, 2, NQ], FP8, name=f"catf{i}", tag="s4b", bufs=4)
                for i in range(4)]
        out_proj(oT2_l, catf[0:2], wo_sb, 0, "psFl")

        # ============ Phase D: global qkv ===================================
        qT_g = [qTp.tile([128, NQ], BF16, name=f"qTg{m}", tag="qT")
                for m in range(4)]
        kT_g = [kTp.tile([128, S], BF16, name=f"kTg{m}", tag="kT")
                for m in range(4)]
        V_g = [Vp.tile([128, 8, 65], BF16, name=f"Vg{pt}", tag="V")
               for pt in range(16)]
        wqkv_g_sb = wp.tile([128, 3, 2, 2, D], FP8, name="wqkv_g_sb", tag="wbig")
        nc.scalar.dma_start(wqkv_g_sb[:], wqkv8_g[:])
        project_qkv(wqkv_g_sb, bqkv_g_sb, bv_g_sb, qT_g, kT_g, V_g, 0, S, 0, "Dg", kv_on_act=GLOBAL_KV_ON_ACT)

        # token-major h for core's tokens (rotated [512, 1536)); issued here
        # so the transposes fill PE slack while Act runs attention exps
        for t in range(8):
            for m in range(4):
                ptr = ps.tile([128, 128], F32, name=f"ptrA{t}{m}", tag="ps")
                nc.tensor.transpose(
                    ptr[:], f32(hT[m][:, Q0 + t * 128: Q0 + (t + 1) * 128]),
                    eye_sb[:])
                nc.vector.tensor_copy(
                    h_sb[t][:, m * 128:(m + 1) * 128], ptr[:])

        # ============ Phase E: global attention + out-proj ==================
        oT2_g = [s4.tile([128, 2, NQ], FP8, name=f"oTg{mm}", tag="s4c", bufs=8)
                 for mm in range(2)]
        for qb in range(2):
            for hp in range(4):
                po_ts = [pso.tile([65, 512], F32, name=f"psoE{qb}{hp}{ab}",
                                  tag=f"pso{ab}", bufs=1) for ab in range(2)]
                for kt in range(16):
                    sc2 = ps2.tile([128, 2, 512], F32,
                                   name=f"psE{qb}{hp}{kt}", tag="ps2")
                    for ab in range(2):
                        r0 = ab * 64
                        nc.tensor.matmul(
                            sc2[:, ab, :], kT_g[hp].bitcast(BF16)
                            [r0:r0 + 64, kt * 128:(kt + 1) * 128],
                            qT_g[hp].bitcast(BF16)
                            [r0:r0 + 64, qb * 512:(qb + 1) * 512],
                            start=True, stop=True, tile_position=(r0, 0))
                    ptg = ptgp.tile([128, 2, 512], BF16,
                                    name=f"ptg{qb}{hp}{kt}", tag="ptg")
                    nc.scalar.activation(ptg[:], sc2[:], AF.Exp, scale=SCALE)
                    for ab in range(2):
                        nc.tensor.matmul(
                            po_ts[ab][:],
                            V_g[kt].bitcast(BF16)[:, 2 * hp + ab, :],
                            ptg[:, ab, :], start=(kt == 0), stop=(kt == 15),
                            skip_group_check=True)
                for ab in range(2):
                    normalize(po_ts[ab], oT2_g, hp, ab * 64, qb * 512,
                              f"nE{qb}{hp}{ab}")
        if debug:
            dbgc2 = pers.tile([128, NQ], BF16, name="dbg_oTg")
            nc.vector.tensor_copy(dbgc2[:], oT2_g[0][:, 0, :])
            nc.sync.dma_start(dbg["d_oTg"][:], dbgc2[:])

        out_proj(oT2_g, catf[2:4], wo_sb, 1, "psFg")

        # ============ Phase G: gate + fuse ==================================
        # gate = tanh(relu(z)); tanh monotone => tanh(relu(z)) = relu(tanh(z)),
        # so Act does Tanh(scale*acc+bias) and relu folds into the fuse mult.
        fusedT = [s4.tile([128, NQ], BF16, name=f"fusedT{m}", tag="s4a", bufs=4)
                  for m in range(4)]
        gate_w_sb = wp.tile([128, 4, 2, D], FP8, name="gate_w_sb", tag="wbig")
        nc.scalar.dma_start(gate_w_sb[:], gw8[:])
        for m in range(4):
            for n in range(2):
                acc = ps.tile([128, 512], F32, name=f"psG{m}{n}", tag="ps")
                for pp in range(4):
                    nc.tensor.matmul(
                        acc[:], gate_w_sb[:, pp, :, m * 128:(m + 1) * 128],
                        catf[pp][:, :, n * 512:(n + 1) * 512],
                        start=(pp == 0), stop=(pp == 3), perf_mode=DRM)
                gt = lnp.tile([128, 512], BF16, name=f"gt{m}{n}", tag="gt", bufs=1)
                nc.scalar.activation(
                    gt[:], acc[:], AF.Tanh, scale=GATE_SCALE,
                    bias=gate_b_sb[:, m:m + 1] if use_gate_b else 0.0)
                if debug and m == 0 and n == 0:
                    dbgg = pers.tile([128, 512], BF16, name="dbg_gt")
                    nc.vector.tensor_scalar(dbgg[:], gt[:], 0.0, None,
                                            op0=ALU.max)
                    nc.sync.dma_start(dbg["d_gateT"][:], dbgg[:])
                # fused = global + relu(gate)*(local - global)
                lsl = catf[m // 2][:, m % 2, n * 512:(n + 1) * 512]
                gsl = catf[2 + m // 2][:, m % 2, n * 512:(n + 1) * 512]
                tmp = lnp.tile([128, 512], BF16, name=f"tmpG{m}{n}", tag="tmpG", bufs=1)
                nc.gpsimd.tensor_tensor(tmp[:], lsl, gsl, op=ALU.subtract)
                nc.vector.scalar_tensor_tensor(
                    tmp[:], gt[:], 0.0, tmp[:], op0=ALU.max, op1=ALU.mult)
                nc.vector.tensor_tensor(
                    fusedT[m].bitcast(BF16)[:, n * 512:(n + 1) * 512],
                    tmp[:], gsl, op=ALU.add)
        if debug:
            nc.sync.dma_start(dbg["d_fusedT"][:], fusedT[0].bitcast(BF16)[:])

        # ===== layernorm helper (token-major [128, D]) ======================
        def layernorm(dst, src_ap, g_sb, b_sb, pfx):
            stats = lnp.tile([128, 6], F32, name=f"{pfx}st", tag="lnst")
            nc.vector.bn_stats(stats[:], src_ap)
            mv = lnp.tile([128, 2], F32, name=f"{pfx}mv", tag="lnmv")
            nc.vector.bn_aggr(mv[:], stats[:])
            std = lnp.tile([128, 1], F32, name=f"{pfx}sd", tag="lnsd")
            nc.scalar.activation(std[:], mv[:, 1:2], AF.Sqrt, bias=eps_sb[:])
            rstd = lnp.tile([128, 1], F32, name=f"{pfx}rs", tag="lnrs")
            nc.vector.reciprocal(rstd[:], std[:])
            if g_sb is not None:
                tmp = lnp.tile([128, D], F32, name=f"{pfx}tmp", tag="lntmp")
                nc.vector.tensor_scalar(
                    tmp[:], src_ap, mv[:, 0:1], rstd[:],
                    op0=ALU.subtract, op1=ALU.mult)
                if b_sb is not None:
                    nc.vector.tensor_tensor(dst, tmp[:], g_sb[:], op=ALU.mult)
                    nc.vector.tensor_tensor(dst, dst, b_sb[:], op=ALU.add)
                else:
                    nc.vector.tensor_tensor(dst, tmp[:], g_sb[:], op=ALU.mult)
            else:
                nc.vector.tensor_scalar(
                    dst, src_ap, mv[:, 0:1], rstd[:],
                    op0=ALU.subtract, op1=ALU.mult)
                if b_sb is not None:
                    nc.vector.tensor_tensor(dst, dst, b_sb[:], op=ALU.add)

        # ============ Phase H: x1 = h + fused^T; y1 = LN1 ===================
        y1 = [s2.tile([128, D], F32R, name=f"y1_{t}", tag="s2") for t in range(8)]
        for t in range(8):
            x1 = lnp.tile([128, D], F32, name=f"x1_{t}", tag="x1")
            for m in range(4):
                ptr = ps.tile([128, 128], BF16, name=f"ptrH{t}{m}", tag="ps")
                nc.tensor.transpose(
                    ptr[:], fusedT[m].bitcast(BF16)[:, t * 128:(t + 1) * 128],
                    eyeb_sb[:])
                nc.vector.scalar_tensor_tensor(
                    x1[:, m * 128:(m + 1) * 128], ptr[:], FUSE_INV,
                    f32(h_sb[t][:, m * 128:(m + 1) * 128]),
                    op0=ALU.mult, op1=ALU.add)
            layernorm(y1[t][:], x1[:], n1gb_sb, n1bb_sb, f"ln1_{t}")
        if debug:
            nc.sync.dma_start(dbg["d_y1"][:], f32(y1[0][:]))

        # ============ Phase I: y1T (fp8 contraction-fold) ===================
        y1T2 = [s4.tile([128, 2, NQ], FP8, name=f"y1T{mm}", tag="s4b", bufs=4)
                for mm in range(2)]
        for t in range(8):
            for m in range(4):
                ptr = ps.tile([128, 128], F32, name=f"ptrI{t}{m}", tag="ps")
                nc.tensor.transpose(ptr[:], f32(y1[t][:, m * 128:(m + 1) * 128]),
                                    eye_sb[:])
                nc.vector.tensor_copy(
                    y1T2[m // 2][:, m % 2, t * 128:(t + 1) * 128], ptr[:])

        # ============ Phase J: FFN + LN2 + LN3; Phase K: pool + out =========
        w1_sb = wp.tile([128, 2, 2, DFF], FP8, name="w1_sb", tag="wbig")
        nc.scalar.dma_start(w1_sb[:], w18[:])
        w2_sb = wp.tile([128, 2, 4, 2, D], FP8, name="w2_sb", tag="wo2nd")
        nc.scalar.dma_start(w2_sb[:], w28[:])
        z8 = [s4.tile([128, 2, NQ], FP8, name=f"z1T{mm}", tag="s4c", bufs=8)
              for mm in range(4)]
        for m in range(8):
            for n in range(2):
                acc = ps.tile([128, 512], F32, name=f"psJ1{m}{n}", tag="ps")
                for pp in range(2):
                    nc.tensor.matmul(
                        acc[:], w1_sb[:, pp, :, m * 128:(m + 1) * 128],
                        y1T2[pp][:, :, n * 512:(n + 1) * 512],
                        start=(pp == 0), stop=(pp == 1), perf_mode=DRM)
                dst = z8[m // 2][:, m % 2, n * 512:(n + 1) * 512]
                nc.scalar.activation(
                    dst, acc[:], AF.Relu, scale=WSI,
                    bias=b1_sb[:, m:m + 1] if use_b1 else 0.0)

        y3 = [s2.tile([128, D], F32R, name=f"y3_{t}", tag="s2") for t in range(8)]
        accp = pso.tile([1, 512], F32, name="pspool", tag="pso0", bufs=1)
        for t in range(8):
            acc = ps.tile([128, 512], F32, name=f"psJ2{t}", tag="ps")
            for hl in range(2):       # w2 hi + lo fp8 terms (hi-lo split)
                for kk in range(4):
                    nc.tensor.matmul(
                        acc[:], z8[kk][:, :, t * 128:(t + 1) * 128],
                        w2_sb[:, hl, kk, :, :],
                        start=(hl == 0 and kk == 0),
                        stop=(hl == 1 and kk == 3), perf_mode=DRM)
            x2 = lnp.tile([128, D], F32, name=f"x2_{t}", tag="x2")
            nc.vector.scalar_tensor_tensor(
                x2[:], acc[:], WSI, f32(y1[t][:]), op0=ALU.mult, op1=ALU.add)
            if use_b2:
                nc.vector.tensor_tensor(x2[:], x2[:], b2b_sb[:], op=ALU.add)
            if not (use_n2g or use_n2b or use_n3g):
                # LN3(LN2(x)) with unit gamma / zero beta collapses to one LN:
                # mean(LN2 out) == 0 exactly, var(LN2 out) = v/(v+eps), so
                # y3 = (x - m) / sqrt(v*(1+eps) + eps^2)
                pfx = f"ln23_{t}"
                stats = lnp.tile([128, 6], F32, name=f"{pfx}st", tag="lnst")
                nc.vector.bn_stats(stats[:], x2[:])
                mv = lnp.tile([128, 2], F32, name=f"{pfx}mv", tag="lnmv")
                nc.vector.bn_aggr(mv[:], stats[:])
                std = lnp.tile([128, 1], F32, name=f"{pfx}sd", tag="lnsd")
                nc.scalar.activation(std[:], mv[:, 1:2], AF.Sqrt,
                                     bias=eps2_sb[:], scale=1.0 + EPS)
                rstd = lnp.tile([128, 1], F32, name=f"{pfx}rs", tag="lnrs")
                nc.vector.reciprocal(rstd[:], std[:])
                nc.vector.tensor_scalar(
                    y3[t][:], x2[:], mv[:, 0:1], rstd[:],
                    op0=ALU.subtract, op1=ALU.mult)
            else:
                y2 = lnp.tile([128, D], F32, name=f"y2_{t}", tag="y2")
                layernorm(y2[:], x2[:], n2gb_sb, n2bb_sb, f"ln2_{t}")
                layernorm(y3[t][:], y2[:], n3gb_sb, None, f"ln3_{t}")
            nc.tensor.matmul(accp[:], poolw_sb[:], y3[t][:],
                             start=(t == 0), stop=(t == 7),
                             skip_group_check=True)
        if debug:
            nc.sync.dma_start(dbg["d_y3"][:], f32(y3[0][:]))

        pooled_sb = pers.tile([1, D], F32, name="pooled_sb")
        nc.vector.tensor_copy(pooled_sb[:], accp[:])
        if debug:
            nc.sync.dma_start(dbg["d_pooled"][:], f32(pooled_sb[:]))
        nc.sync.dma_start(po[:], pooled_sb[:])

    nc.compile()
    return nc


def _prep_inputs(inputs):
    """Host-side prep: returns (flags, in_maps for 8 cores, host_const)."""
    g = {k: np.asarray(v, dtype=np.float32) for k, v in inputs.items()}
    x, pos = g["x"], g["pos"]
    win_w, win_b = g["win_w"], g["win_b"]

    flags = (
        bool(np.any(g["l_bqkv"] != 0)), bool(np.any(g["g_bqkv"] != 0)),
        bool(np.any(g["l_bo"] != 0) or np.any(g["g_bo"] != 0)),
        bool(np.any(g["gate_b"] != 0)), bool(np.any(g["ffn_b1"] != 0)),
        bool(np.any(g["ffn_b2"] != 0)),
        bool(np.any(g["n1_g"] != 1)), bool(np.any(g["n1_b"] != 0)),
        bool(np.any(g["n2_g"] != 1)), bool(np.any(g["n2_b"] != 0)),
        bool(np.any(g["n3_g"] != 1)),
    )
    (use_bqkv_l, use_bqkv_g, use_bo, use_gate_b, use_b1, use_b2,
     use_n1g, use_n1b, use_n2g, use_n2b, use_n3g) = flags

    posT = pos[0].T + win_b[:, None]                      # [D, S]

    def fold8(w3):
        # [3, D, D] -> [128, 3, 2pair, 2j, D]: w[qkv, pair*256 + j*128 + p, :]
        return np.ascontiguousarray(
            (w3.reshape(3, 2, 2, 128, D) * WS).transpose(3, 0, 1, 2, 4)
        ).astype(NPF8)

    def foldw(w, npair):
        # [K, N] -> [128, npair, 2, N]: w[pair*256 + j*128 + p, :] * WS
        kdim, n = w.shape
        assert kdim == npair * 256
        return np.ascontiguousarray(
            (w.reshape(npair, 2, 128, n) * WS).transpose(2, 0, 1, 3)
        ).astype(NPF8)

    common = {
        "win": np.ascontiguousarray(win_w).astype(ml_dtypes.bfloat16),
        "wqkv8_l": fold8(g["l_wqkv"]),
        "wqkv8_g": fold8(g["g_wqkv"]),
        "wo8": np.ascontiguousarray(np.stack(
            [foldw(g["l_wo"], 2), foldw(g["g_wo"], 2)], axis=1)),
        "gw8": foldw(g["gate_w"], 4),
        "w18": foldw(g["ffn_w1"], 2),
        "eye": np.eye(128, dtype=np.float32),
        "poolw": np.full((128, 1), 1.0 / S, dtype=np.float32),
    }
    w2s = g["ffn_w2"] * WS
    w2hi = w2s.astype(NPF8).astype(np.float32)
    foldr = lambda w: np.ascontiguousarray(
        w.reshape(4, 2, 128, D).transpose(2, 0, 1, 3)).astype(NPF8)
    common["w28"] = np.ascontiguousarray(
        np.stack([foldr(w2hi), foldr(w2s - w2hi)], axis=1))
    perm = lambda b: b.reshape(-1, 4, 128).transpose(2, 0, 1).copy()
    if use_bqkv_l:
        common["bqkv_l"] = perm(g["l_bqkv"])
        common["bv_l"] = np.tile(g["l_bqkv"][2], (128, 1))
    if use_bqkv_g:
        common["bqkv_g"] = perm(g["g_bqkv"])
        common["bv_g"] = np.tile(g["g_bqkv"][2], (128, 1))
    if use_bo:
        common["bo2"] = perm(np.stack([g["l_bo"], g["g_bo"]])) * OS
    if use_gate_b:
        common["gate_b"] = g["gate_b"].reshape(4, 128).T.copy()
    if use_b1:
        common["b1"] = g["ffn_b1"].reshape(8, 128).T.copy()
    if use_b2:
        common["b2b"] = np.tile(g["ffn_b2"], (128, 1))
    if use_n1g:
        common["n1gb"] = np.tile(g["n1_g"], (128, 1))
    if use_n1b:
        common["n1bb"] = np.tile(g["n1_b"], (128, 1))
    if use_n2g:
        common["n2gb"] = np.tile(g["n2_g"], (128, 1))
    if use_n2b:
        common["n2bb"] = np.tile(g["n2_b"], (128, 1))
    if use_n3g:
        common["n3gb"] = np.tile(g["n3_g"], (128, 1))

    # universal interior band masks (pure Toeplitz, no seam crossing)
    kk = np.arange(128)
    qq = np.arange(512)
    mk_m = np.zeros((128, 4, 512), dtype=np.float32)
    for di, d in enumerate((0, 128, 256, 384)):
        mk_m[:, di, :] = (np.abs(kk[:, None] + d - qq[None, :]) <= W // 2)
    mk_m = mk_m.astype(ml_dtypes.bfloat16)

    hf_data = []
    for hf in range(2):
        q0c = NQ * hf
        shift = Q0 - q0c
        posb_rot = np.ascontiguousarray(np.roll(posT, shift, axis=1))
        mk_e = np.zeros((128, 2, 2, 32), dtype=np.float32)
        for qb in range(2):
            q0 = Q0 + qb * 512
            for de_i, d in enumerate(EDGE_DELTAS):
                qq0, qq1 = STRIPE[d]
                k_rot = q0 + d + kk[:, None]
                q_rot = q0 + np.arange(qq0, qq1)[None, :]
                orig_k = (k_rot - shift) % S
                orig_q = (q_rot - shift) % S
                mk_e[:, de_i, qb, :] = (np.abs(orig_k - orig_q) <= W // 2)
        hf_data.append((posb_rot, mk_e.astype(ml_dtypes.bfloat16)))

    in_maps = []
    for core in range(N_CORES):
        b, hf = core // 2, core % 2
        shift = Q0 - NQ * hf
        posb_rot, mk_e = hf_data[hf]
        m = dict(common)
        m["xT"] = np.ascontiguousarray(
            np.roll(x[b].T, shift, axis=1)).astype(ml_dtypes.bfloat16)
        m["posb"] = posb_rot.astype(ml_dtypes.bfloat16)
        m["masks_m"] = mk_m
        m["masks_e"] = mk_e
        in_maps.append(m)

    host_const = (g["n3_b"] @ g["out_w"] + g["out_b"],
                  np.ascontiguousarray(g["out_w"]))
    return flags, in_maps, host_const


def kernel(**inputs):
    flags, in_maps, host_const = _prep_inputs(inputs)
    const_vec, out_w = host_const
    if flags not in _CACHE:
        _CACHE[flags] = _build(flags)
    nc = _CACHE[flags]
    res = run_bass_kernel_spmd(nc, in_maps, core_ids=list(range(N_CORES)))
    out = np.zeros((B, DOUT), dtype=np.float32)
    for b in range(B):
        pooled = res.results[2 * b]["po"][0] + res.results[2 * b + 1]["po"][0]
        out[b] = pooled @ out_w + const_vec
    return out

